# revision 14
# baseline (speedup 1.0000x reference)
"""Trainium2 Bass kernel for a single-layer "BiTRF" dense transformer block.

Math (see reference):
  posi[h,i,j] = p0*(exp(-sp1*|i-j|) + exp(-sp2*|i-j|)) + p3*(i<j)   (sp=softplus(p))
  attn[h,b,i,j] = kproj[b,i,h] + posi[h,i,j], diag masked, softmax over j.
  Because kproj[b,i,h] is constant along the softmax axis j, softmax is
  invariant to it, so the wk/bk projection drops out entirely and the
  attention weights W[h,i,:] are shared across the batch.
  out  = LN1(attnout @ fc_w.T + fc_b)
  out2 = LN2(relu(out @ w1.T + b1) @ w2.T + b2 + out)
  y    = log_softmax(out2 @ h2o_w.T + h2o_b)

Sharding: 8 cores, core c owns query rows i in [c*128,(c+1)*128) for BOTH
batches (256 row-instances).  v = x@wv.T is computed redundantly on every
core (avoids any collective); everything else is row-sharded, h2o is
row-sharded too (each core computes its rows x full 32000 vocab, so
log_softmax is fully local).

Layout: activations are kept feature-major [feat, row] so every linear is
matmul(lhsT=W.T tile, rhs=act) with weights pre-transposed on the host.
Nonzero biases ride as rank-1 augmentation: the contraction dim is padded
to K+128 with a ones row + zero rows (full 128-row block keeps K%128==0).
When a stage's bias is all-zero (the common case here) the augmentation is
skipped entirely at compile time.

dtypes: v/fc/attention matmuls run in float32r (fp32 data, FP22 multiply,
fp32 accumulate: full PE rate at free>=256, ~1e-4 error).  The FFN and the
h2o vocab projection stream their weights in bf16 (halves the dominant
HBM traffic; ~5e-4 output error, fp32 accumulation throughout).
"""

import contextlib
import math

import ml_dtypes
import numpy as np

import concourse.mybir as mybir
import concourse.tile as tile
from concourse import bacc
from concourse.bass_utils import run_bass_kernel_spmd
from concourse.kernels.tile_matmul import matmul_tile_kernel
from concourse.masks import make_identity

B, L, D, H, DV, HID, V = 2, 1024, 1024, 16, 64, 4096, 32000
NCORES = 8
IC = L // NCORES        # 128 query rows per core
ROWS = B * IC           # 256 row-instances per core
HD = H * DV             # 1024
P = 128
EPS = 1e-5
NEG_BIG = -1.0e9

F32 = mybir.dt.float32
F32R = mybir.dt.float32r
BF16 = mybir.dt.bfloat16
F16 = mybir.dt.float16
AF = mybir.ActivationFunctionType
ALU = mybir.AluOpType
AX = mybir.AxisListType

VS = V // NCORES        # 4000-vocab shard per core
# per-core h2o vocab tiling: 8 tiles of 500
VTILES = [(i * 500, 500) for i in range(8)]
NRT = B * L // P        # 16 gathered row tiles


def _r(ap):
    return ap.bitcast(F32R)


def _softplus(x):
    return np.logaddexp(0.0, x.astype(np.float64))


def _write_aug_block(nc, tc, dst_rows, dtype, tag):
    """Write [ones_row; zeros x 127] into dst_rows ([P, ROWS] DRAM view)."""
    with tc.tile_pool(name=f"aug_{tag}", bufs=1) as pool:
        blk = pool.tile([P, ROWS], dtype, name=f"augblk_{tag}")
        nc.any.memset(blk[:], 0.0)
        nc.any.memset(blk[0:1, :], 1.0)
        nc.sync.dma_start(dst_rows, blk[:])


def _layernorm(nc, tc, src_dram, g_dram, b_dram, dst_dram, ones_col, tag):
    """LN over the feature (partition) axis.  src [D, ROWS] f32 feature-major,
    g/b [P, D//P] per-partition scalars, dst [D, ROWS] (dst dtype = tile's)."""
    DC = D // P
    with contextlib.ExitStack() as ctx:
        lp = ctx.enter_context(tc.tile_pool(name=f"ln_{tag}", bufs=2))
        cp = ctx.enter_context(tc.tile_pool(name=f"lnc_{tag}", bufs=1))
        pp = ctx.enter_context(tc.tile_pool(name=f"lnp_{tag}", bufs=2, space="PSUM"))

        F_sb = lp.tile([P, DC, ROWS], F32R, name=f"F_{tag}")
        nc.sync.dma_start(F_sb[:], _r(src_dram.rearrange("(c p) r -> p c r", p=P)))
        SQ = lp.tile([P, DC, ROWS], F32R, name=f"SQ_{tag}")
        nc.vector.tensor_mul(SQ[:], F_sb[:], F_sb[:])

        g_sb = cp.tile([P, DC], F32, name=f"g_{tag}")
        nc.sync.dma_start(g_sb[:], g_dram.ap())
        b_sb = cp.tile([P, DC], F32, name=f"b_{tag}")
        nc.sync.dma_start(b_sb[:], b_dram.ap())

        ps_sum = pp.tile([2, ROWS], F32, name=f"pssum_{tag}")
        ps_sq = pp.tile([2, ROWS], F32, name=f"pssq_{tag}")
        for dc in range(DC):
            nc.tensor.matmul(ps_sum[:], ones_col[:], F_sb[:, dc],
                             start=(dc == 0), stop=(dc == DC - 1))
            nc.tensor.matmul(ps_sq[:], ones_col[:], SQ[:, dc],
                             start=(dc == 0), stop=(dc == DC - 1))

        mean = lp.tile([1, ROWS], F32, name=f"mean_{tag}")
        nc.vector.tensor_scalar(mean[:], ps_sum[0:1, :], 1.0 / D, None, ALU.mult)
        ex2 = lp.tile([1, ROWS], F32, name=f"ex2_{tag}")
        nc.vector.tensor_scalar(ex2[:], ps_sq[0:1, :], 1.0 / D, None, ALU.mult)
        var = lp.tile([1, ROWS], F32, name=f"var_{tag}")
        nc.vector.tensor_mul(var[:], mean[:], mean[:])
        nc.vector.tensor_sub(var[:], ex2[:], var[:])
        # veps = var + eps ; s = sqrt(veps) Newton-polished ; rstd = 1/s
        veps = lp.tile([1, ROWS], F32, name=f"veps_{tag}")
        nc.vector.tensor_scalar(veps[:], var[:], EPS, None, ALU.add)
        s0 = lp.tile([1, ROWS], F32, name=f"s0_{tag}")
        nc.scalar.activation(s0[:], veps[:], AF.Sqrt)
        r0 = lp.tile([1, ROWS], F32, name=f"r0_{tag}")
        nc.vector.reciprocal(r0[:], s0[:])
        s1 = lp.tile([1, ROWS], F32, name=f"s1_{tag}")
        nc.vector.tensor_mul(s1[:], veps[:], r0[:])
        nc.vector.tensor_add(s1[:], s1[:], s0[:])
        nc.vector.tensor_scalar(s1[:], s1[:], 0.5, None, ALU.mult)
        rstd = lp.tile([1, ROWS], F32, name=f"rstd_{tag}")
        nc.vector.reciprocal(rstd[:], s1[:])

        meanB = lp.tile([P, ROWS], F32, name=f"meanB_{tag}")
        nc.gpsimd.partition_broadcast(meanB[:], mean[:])
        rstdB = lp.tile([P, ROWS], F32, name=f"rstdB_{tag}")
        nc.gpsimd.partition_broadcast(rstdB[:], rstd[:])

        Y_sb = lp.tile([P, DC, ROWS], dst_dram.dtype, name=f"Y_{tag}")
        for dc in range(DC):
            t1 = lp.tile([P, ROWS], F32, name=f"t1_{tag}", bufs=3)
            nc.vector.tensor_sub(t1[:], F_sb[:, dc], meanB[:])
            nc.vector.tensor_mul(t1[:], t1[:], rstdB[:])
            nc.vector.tensor_scalar(Y_sb[:, dc], t1[:],
                                    g_sb[:, dc:dc + 1], b_sb[:, dc:dc + 1],
                                    ALU.mult, ALU.add)
        nc.sync.dma_start(dst_dram.rearrange("(c p) r -> p c r", p=P), Y_sb[:])


def _build(p0, sp1, sp2, p3, bias_on):
    """Build + compile the SPMD program.  p0/sp1/sp2/p3 are [H] host floats,
    baked into the NEFF as activation immediates.  bias_on: per-stage flags
    (v, fc, w1, w2, h2o) — augmentation emitted only for nonzero biases."""
    p3_zero = bool(np.all(p3 == 0.0))
    n_r = 1 if p3_zero else H

    KV = D + (P if bias_on["v"] else 0)
    KFC = HD + (P if bias_on["fc"] else 0)
    KW1 = D + (P if bias_on["w1"] else 0)
    KW2 = HID + (P if bias_on["w2"] else 0)

    nc = bacc.Bacc(None, target_bir_lowering=False, debug=False,
                   num_devices=NCORES)

    def inp(name, shape, dtype):
        return nc.dram_tensor(name, shape, dtype, kind="ExternalInput")

    xT = inp("xT", [KV, B * L], BF16)
    wvT = inp("wvT", [KV, HD], BF16)
    fcT = inp("fcT", [KFC, D], BF16)
    w1T = inp("w1T", [KW1, HID], BF16)
    w2T = inp("w2T", [KW2, D], BF16)
    h2oT = inp("h2oT", [D, VS], BF16)
    if bias_on["h2o"]:
        h2ob = inp("h2ob", [1, VS], BF16)
        onesr = inp("onesr", [1, ROWS], BF16)
    onesc = inp("onesc", [P, 2], F32R)           # ones columns (K-sum, even N)
    onesb = inp("onesb", [P, 2], BF16)           # bf16 ones columns
    ln1g = inp("ln1g", [P, D // P], F32)
    ln1b = inp("ln1b", [P, D // P], F32)
    ln2g = inp("ln2g", [P, D // P], F32)
    ln2b = inp("ln2b", [P, D // P], F32)
    S_in = inp("S_in", [P, 8, IC], F32)          # |i-j| tiled [jp, jc, i]
    expb = inp("expb", [P, H], F32)              # per-head exp bias ln(2*p0)
    R_in = inp("R_in", [n_r, P, 8, IC], F32)     # p3*(i<j) - BIG*eye, per head
    y = nc.dram_tensor("y", [B * L, VS], F32, kind="ExternalOutput")

    with tile.TileContext(nc) as tc:
        with tc.tile_pool(name="dram", bufs=1, space="DRAM") as dram:
            v_d = dram.tile([B * L, HD], BF16, name="v_d")
            O_aug = dram.tile([KFC, ROWS], BF16, name="O_aug")
            FC = dram.tile([D, ROWS], F32, name="FC")
            Y_aug = dram.tile([KW1, ROWS], BF16, name="Y_aug")
            H_aug = dram.tile([KW2, ROWS], BF16, name="H_aug")
            FF = dram.tile([D, ROWS], F32, name="FF")
            Z_d = dram.tile([D, ROWS], BF16, name="Z_d")
            Zg_d = dram.tile([NCORES * D, ROWS], BF16, name="Zg_d",
                             addr_space="Shared")
            sl_d = dram.tile([NRT, P], F32, name="sl_d")
            sg_d = dram.tile([NRT, P], F32, name="sg_d",
                             addr_space="Shared")

            @contextlib.contextmanager
            def _mm_pools(tag, bufs=4, cache=False):
                with tc.tile_pool(name=f"mmA_{tag}", bufs=bufs) as a, \
                     tc.tile_pool(name=f"mmB_{tag}", bufs=bufs) as b:
                    yield dict(kxm_pool=a, kxn_pool=b, cache_tiles=cache)

            with tc.tile_pool(name="const0", bufs=1) as c0:
                ones_col = c0.tile([P, 2], F32R, name="ones_col")
                nc.sync.dma_start(ones_col[:], onesc.ap())
                ones_colb = c0.tile([P, 2], BF16, name="ones_colb")
                nc.sync.dma_start(ones_colb[:], onesb.ap())

                # ---- stage A: v = x @ wv.T (+ bv) (row-major [B*L, HD]) ----
                with _mm_pools("v", cache=True) as mp:
                    matmul_tile_kernel(tc, xT.ap(), wvT.ap(), v_d[:], **mp)

                # ---- stage B: attention ----
                with contextlib.ExitStack() as ctx:
                    ap_ = ctx.enter_context(tc.tile_pool(name="attn", bufs=1))
                    up = ctx.enter_context(tc.tile_pool(name="attn_u", bufs=2))
                    sp_ = ctx.enter_context(tc.tile_pool(name="attn_s", bufs=3))
                    cp = ctx.enter_context(tc.tile_pool(name="attn_c", bufs=1))
                    pp = ctx.enter_context(
                        tc.tile_pool(name="attn_p", bufs=2, space="PSUM"))

                    v_sb = ap_.tile([P, B * L // P, HD], BF16, name="v_sb")
                    nc.sync.dma_start(
                        v_sb[:], v_d[:].rearrange("(c p) f -> p c f", p=P))
                    S_sb = cp.tile([P, 8, IC], F32, name="S_sb")
                    nc.sync.dma_start(S_sb[:], S_in.ap())
                    eb_sb = cp.tile([P, H], F32, name="eb_sb")
                    nc.sync.dma_start(eb_sb[:], expb.ap())
                    ident = cp.tile([P, P], F32, name="ident")
                    make_identity(nc, ident[:])
                    R_sb = None
                    O_sb = ap_.tile([P, B, HD], F32, name="O_sb")

                    for h in range(H):
                        if R_sb is None or n_r > 1:
                            R_sb = cp.tile([P, 8, IC], F32, name="R_sb", bufs=2)
                            nc.sync.dma_start(R_sb[:], R_in.ap()[min(h, n_r - 1)])
                        t_sb = up.tile([P, 8, IC], F32, name="t_sb")
                        if p0[h] > 0.0 and abs(sp1[h] - sp2[h]) < 1e-12:
                            nc.scalar.activation(t_sb[:], S_sb[:], AF.Exp,
                                                 scale=-sp1[h],
                                                 bias=eb_sb[:, h:h + 1])
                        elif p0[h] > 0.0:
                            e2 = up.tile([P, 8, IC], F32, name="e2_sb")
                            nc.scalar.activation(t_sb[:], S_sb[:], AF.Exp,
                                                 scale=-sp1[h],
                                                 bias=eb_sb[:, h:h + 1])
                            nc.scalar.activation(e2[:], S_sb[:], AF.Exp,
                                                 scale=-sp2[h],
                                                 bias=eb_sb[:, h:h + 1])
                            nc.vector.tensor_add(t_sb[:], t_sb[:], e2[:])
                        elif p0[h] == 0.0:
                            nc.any.memset(t_sb[:], 0.0)
                        else:
                            e2 = up.tile([P, 8, IC], F32, name="e2_sb")
                            nc.scalar.activation(t_sb[:], S_sb[:], AF.Exp,
                                                 scale=-sp1[h])
                            nc.scalar.activation(e2[:], S_sb[:], AF.Exp,
                                                 scale=-sp2[h])
                            nc.vector.tensor_add(t_sb[:], t_sb[:], e2[:])
                            nc.vector.tensor_scalar(t_sb[:], t_sb[:], p0[h],
                                                    None, ALU.mult)
                        nc.vector.tensor_add(t_sb[:], t_sb[:], R_sb[:])
                        u_sb = up.tile([P, 8, IC], BF16, name="u_sb")
                        nc.scalar.activation(u_sb[:], t_sb[:], AF.Exp)

                        ps_o = [pp.tile([P, DV], F32, name=f"ps_o{b}")
                                for b in range(B)]
                        ps_s = pp.tile([P, 2], F32, name="ps_s")
                        for jc in range(8):
                            lhsT = u_sb[:, jc]
                            for b in range(B):
                                nc.tensor.matmul(
                                    ps_o[b][:], lhsT,
                                    v_sb[:, b * 8 + jc, h * DV:(h + 1) * DV],
                                    start=(jc == 0), stop=(jc == 7))
                            nc.tensor.matmul(ps_s[:], lhsT, ones_colb[:],
                                             start=(jc == 0), stop=(jc == 7))
                        rs = sp_.tile([P, 1], F32, name="rs_t")
                        nc.vector.reciprocal(rs[:], ps_s[:, 0:1])
                        for b in range(B):
                            nc.vector.tensor_scalar(
                                O_sb[:, b, h * DV:(h + 1) * DV],
                                ps_o[b][:], rs[:], None, ALU.mult)

                    OT = ap_.tile([P, HD // P, ROWS], BF16, name="OT")
                    for b in range(B):
                        for hc in range(HD // P):
                            pt = pp.tile([P, P], F32, name="pt")
                            nc.tensor.transpose(
                                pt[:], O_sb[:, b, hc * P:(hc + 1) * P], ident[:])
                            nc.vector.tensor_copy(
                                OT[:, hc, b * IC:(b + 1) * IC], pt[:])
                    nc.sync.dma_start(
                        O_aug[0:HD, :].rearrange("(c p) r -> p c r", p=P), OT[:])
                    if bias_on["fc"]:
                        _write_aug_block(nc, tc, O_aug[HD:KFC, :], BF16, "O")

                # ---- stage C: fc + LN1 ----
                with _mm_pools("fc") as mp:
                    matmul_tile_kernel(tc, fcT.ap(), O_aug[:], FC[:], **mp)
                _layernorm(nc, tc, FC[:], ln1g, ln1b, Y_aug[0:D, :],
                           ones_col, "ln1")
                if bias_on["w1"]:
                    _write_aug_block(nc, tc, Y_aug[D:KW1, :], BF16, "Y")

                # ---- stage D: FFN ----
                with _mm_pools("w1") as mp:
                    matmul_tile_kernel(tc, w1T.ap(), Y_aug[:],
                                       H_aug[0:HID, :], use_relu=True, **mp)
                if bias_on["w2"]:
                    _write_aug_block(nc, tc, H_aug[HID:KW2, :], BF16, "H")
                with _mm_pools("w2") as mp:
                    matmul_tile_kernel(tc, w2T.ap(), H_aug[:], FF[:],
                                       accumulate_ap=Y_aug[0:D, :], **mp)
                _layernorm(nc, tc, FF[:], ln2g, ln2b, Z_d[:], ones_col, "ln2")

                # ---- stage E: h2o (vocab-sharded) + log_softmax ----
                nc.gpsimd.collective_compute(
                    "AllGather", ALU.bypass,
                    replica_groups=[list(range(NCORES))],
                    ins=[Z_d[:]], outs=[Zg_d[:]])
                with contextlib.ExitStack() as ctx:
                    zp = ctx.enter_context(tc.tile_pool(name="h2o_z", bufs=1))
                    wp = ctx.enter_context(tc.tile_pool(name="h2o_w", bufs=4))
                    ep = ctx.enter_context(tc.tile_pool(name="h2o_e", bufs=3))
                    op_ = ctx.enter_context(tc.tile_pool(name="h2o_o", bufs=4))
                    pp = ctx.enter_context(
                        tc.tile_pool(name="h2o_p", bufs=4, space="PSUM"))

                    Zg_sb = zp.tile([P, NCORES * D // P, ROWS], BF16,
                                    name="Zg_sb")
                    nc.sync.dma_start(
                        Zg_sb[:], Zg_d[:].rearrange("(g p) r -> p g r", p=P))
                    L16 = zp.tile([P, NRT, VS], F16, name="L16")       # 16 MB
                    parts = zp.tile([P, NRT, len(VTILES)], F32, name="parts")
                    if bias_on["h2o"]:
                        ones_row = zp.tile([1, ROWS], BF16, name="ones_row_z")
                        nc.sync.dma_start(ones_row[:], onesr.ap())

                    h2oT_t = h2oT.ap().rearrange("(c p) v -> p c v", p=P)
                    for vi, (vs, vsz) in enumerate(VTILES):
                        W_sb = wp.tile([P, D // P, 500], BF16, name="W_sb")
                        nc.sync.dma_start(W_sb[:, :, :vsz],
                                          h2oT_t[:, :, vs:vs + vsz])
                        if bias_on["h2o"]:
                            bias_sb = ep.tile([1, 500], BF16, name="bias_sb")
                            nc.sync.dma_start(bias_sb[:, :vsz],
                                              h2ob.ap()[:, vs:vs + vsz])
                        for rt in range(NRT):
                            crank, half = rt // B, rt % B
                            ps = pp.tile([P, 500], F32, name="ps_l")
                            for dc in range(D // P):
                                nc.tensor.matmul(
                                    ps[:, :vsz],
                                    Zg_sb[:, crank * (D // P) + dc,
                                          half * IC:(half + 1) * IC],
                                    W_sb[:, dc, :vsz],
                                    start=(dc == 0),
                                    stop=(dc == D // P - 1
                                          and not bias_on["h2o"]))
                            if bias_on["h2o"]:
                                nc.tensor.matmul(
                                    ps[:, :vsz],
                                    ones_row[:, half * IC:(half + 1) * IC],
                                    bias_sb[:, :vsz],
                                    start=False, stop=True)
                            nc.vector.tensor_copy(L16[:, rt, vs:vs + vsz],
                                                  ps[:, :vsz])
                            esc = ep.tile([P, 500], F32, name="esc", bufs=2)
                            nc.scalar.activation(
                                esc[:, :vsz], ps[:, :vsz], AF.Exp,
                                accum_out=parts[:, rt, vi:vi + 1])

                    # local sumexp over this core's vocab shard, then
                    # all-reduce across the vocab shards
                    s_loc = ep.tile([P, NRT], F32, name="s_loc")
                    nc.vector.reduce_sum(s_loc[:], parts[:], axis=AX.X)
                    nc.sync.dma_start(sl_d[:].rearrange("g p -> p g"), s_loc[:])
                    nc.gpsimd.collective_compute(
                        "AllReduce", ALU.add,
                        replica_groups=[list(range(NCORES))],
                        ins=[sl_d[:]], outs=[sg_d[:]])
                    s_glob = ep.tile([P, NRT], F32, name="s_glob")
                    nc.sync.dma_start(s_glob[:], sg_d[:].rearrange("g p -> p g"))
                    LSE = ep.tile([P, NRT], F32, name="LSE")
                    nc.scalar.activation(LSE[:], s_glob[:], AF.Ln)

                    for rt in range(NRT):
                        for (vs, vsz) in VTILES:
                            ot = op_.tile([P, 500], F32, name="ot")
                            nc.vector.tensor_scalar(
                                ot[:, :vsz], L16[:, rt, vs:vs + vsz],
                                LSE[:, rt:rt + 1], None, ALU.subtract)
                            nc.sync.dma_start(
                                y.ap()[rt * P:(rt + 1) * P, vs:vs + vsz],
                                ot[:, :vsz])

    nc.compile()
    return nc


_CACHE = {}


def _aug_pad(wT, bias, dtype=np.float32):
    """[K, M] + bias row + 127 zero rows -> [K+128, M]."""
    K, M = wT.shape
    out = np.zeros((K + P, M), dtype)
    out[:K] = wT
    out[K] = bias
    return out


def kernel(**inputs):
    f32 = np.float32
    bf16 = ml_dtypes.bfloat16
    x = np.asarray(inputs["x"], f32)
    wv = np.asarray(inputs["wv"], f32)
    bv = np.asarray(inputs["bv"], f32)
    fc_w = np.asarray(inputs["fc_w"], f32)
    fc_b = np.asarray(inputs["fc_b"], f32)
    ln1_g = np.asarray(inputs["ln1_g"], f32)
    ln1_b = np.asarray(inputs["ln1_b"], f32)
    w1 = np.asarray(inputs["w1"], f32)
    b1 = np.asarray(inputs["b1"], f32)
    w2 = np.asarray(inputs["w2"], f32)
    b2 = np.asarray(inputs["b2"], f32)
    ln2_g = np.asarray(inputs["ln2_g"], f32)
    ln2_b = np.asarray(inputs["ln2_b"], f32)
    h2o_w = np.asarray(inputs["h2o_w"], f32)
    h2o_b = np.asarray(inputs["h2o_b"], f32)
    p0 = np.asarray(inputs["p0"], np.float64)
    p1 = np.asarray(inputs["p1"], np.float64)
    p2 = np.asarray(inputs["p2"], np.float64)
    p3 = np.asarray(inputs["p3"], np.float64)
    # wk/bk deliberately unused: constant along the softmax axis.

    sp1 = np.float32(_softplus(p1)).astype(np.float64)
    sp2 = np.float32(_softplus(p2)).astype(np.float64)

    bias_on = {
        "v": bool(np.any(bv)),
        "fc": bool(np.any(fc_b)),
        "w1": bool(np.any(b1)),
        "w2": bool(np.any(b2)),
        "h2o": bool(np.any(h2o_b)),
    }

    key = (p0.tobytes(), sp1.tobytes(), sp2.tobytes(), p3.tobytes(),
           tuple(sorted(bias_on.items())))
    if key not in _CACHE:
        _CACHE[key] = _build(p0, sp1, sp2, p3, bias_on)
    nc = _CACHE[key]

    x2T = np.ascontiguousarray(x.reshape(B * L, D).T)
    if bias_on["v"]:
        ones_blk = np.zeros((P, B * L), f32)
        ones_blk[0] = 1.0
        xT_host = np.concatenate([x2T, ones_blk], axis=0)
        wvT_host = _aug_pad(wv.T, bv)
    else:
        xT_host = x2T
        wvT_host = np.ascontiguousarray(wv.T)
    shared = {
        "xT": xT_host,
        "wvT": wvT_host,
        "fcT": _aug_pad(fc_w.T, fc_b) if bias_on["fc"]
               else np.ascontiguousarray(fc_w.T),
        "onesc": np.ones((P, 2), f32),
        "ln1g": np.ascontiguousarray(ln1_g.reshape(D // P, P).T),
        "ln1b": np.ascontiguousarray(ln1_b.reshape(D // P, P).T),
        "ln2g": np.ascontiguousarray(ln2_g.reshape(D // P, P).T),
        "ln2b": np.ascontiguousarray(ln2_b.reshape(D // P, P).T),
    }
    shared = {k: np.ascontiguousarray(a, f32) for k, a in shared.items()}
    for k in ("xT", "wvT", "fcT"):
        shared[k] = np.ascontiguousarray(shared[k].astype(bf16))
    shared["onesb"] = np.ones((P, 2), bf16)
    shared["w1T"] = np.ascontiguousarray(
        _aug_pad(w1.T, b1, bf16) if bias_on["w1"] else w1.T.astype(bf16))
    shared["w2T"] = np.ascontiguousarray(
        _aug_pad(w2.T, b2, bf16) if bias_on["w2"] else w2.T.astype(bf16))
    h2oT_bf = h2o_w.T.astype(bf16)          # [D, V]
    if bias_on["h2o"]:
        shared["onesr"] = np.ones((1, ROWS), bf16)

    p3_zero = bool(np.all(p3 == 0.0))
    ebv = np.zeros(H, np.float64)
    for h in range(H):
        if p0[h] > 0.0 and abs(sp1[h] - sp2[h]) < 1e-12:
            ebv[h] = math.log(2.0 * p0[h])
        elif p0[h] > 0.0:
            ebv[h] = math.log(p0[h])
    expb_host = np.ascontiguousarray(
        np.broadcast_to(ebv.astype(f32)[None, :], (P, H)))

    j = np.arange(L)
    in_maps = []
    for c in range(NCORES):
        i_idx = c * IC + np.arange(IC)
        Sji = np.abs(j[:, None] - i_idx[None, :]).astype(f32)       # [L, IC]
        eye = (Sji == 0).astype(f32)
        if p3_zero:
            Rs = [NEG_BIG * eye]
        else:
            Aji = (i_idx[None, :] < j[:, None]).astype(f32)
            Rs = [np.float32(p3[h]) * Aji + NEG_BIG * eye for h in range(H)]

        def tile_ji(a):  # [L, IC] -> [jp, jc, IC]
            return np.ascontiguousarray(
                a.reshape(8, P, IC).transpose(1, 0, 2), f32)

        m = dict(shared)
        m["S_in"] = tile_ji(Sji)
        m["expb"] = expb_host
        m["R_in"] = np.stack([tile_ji(R) for R in Rs], axis=0)
        m["h2oT"] = np.ascontiguousarray(h2oT_bf[:, c * VS:(c + 1) * VS])
        if bias_on["h2o"]:
            m["h2ob"] = np.ascontiguousarray(
                h2o_b[None, c * VS:(c + 1) * VS].astype(bf16))
        in_maps.append(m)

    res = run_bass_kernel_spmd(nc, in_maps, core_ids=list(range(NCORES)))

    out = np.empty((B, L, V), f32)
    for c in range(NCORES):
        yc = res.results[c]["y"]        # [B*L, VS], gathered-row order
        for crank in range(NCORES):
            for b in range(B):
                out[b, crank * IC:(crank + 1) * IC, c * VS:(c + 1) * VS] = \
                    yc[crank * ROWS + b * IC:crank * ROWS + (b + 1) * IC, :]
    return out


# revision 16
# speedup vs baseline: 1.3530x; 1.3530x over previous
"""Trainium2 Bass kernel for a single-layer "BiTRF" dense transformer block.

Math (see reference):
  posi[h,i,j] = p0*(exp(-sp1*|i-j|) + exp(-sp2*|i-j|)) + p3*(i<j)   (sp=softplus(p))
  attn[h,b,i,j] = kproj[b,i,h] + posi[h,i,j], diag masked, softmax over j.
  Because kproj[b,i,h] is constant along the softmax axis j, softmax is
  invariant to it, so the wk/bk projection drops out entirely and the
  attention weights W[h,i,:] are shared across the batch.
  out  = LN1(attnout @ fc_w.T + fc_b)
  out2 = LN2(relu(out @ w1.T + b1) @ w2.T + b2 + out)
  y    = log_softmax(out2 @ h2o_w.T + h2o_b)

Sharding: 8 cores, core c owns query rows i in [c*128,(c+1)*128) for BOTH
batches (256 row-instances).  v = x@wv.T is computed redundantly on every
core (avoids any collective); everything else is row-sharded, h2o is
row-sharded too (each core computes its rows x full 32000 vocab, so
log_softmax is fully local).

Layout: activations are kept feature-major [feat, row] so every linear is
matmul(lhsT=W.T tile, rhs=act) with weights pre-transposed on the host.
Nonzero biases ride as rank-1 augmentation: the contraction dim is padded
to K+128 with a ones row + zero rows (full 128-row block keeps K%128==0).
When a stage's bias is all-zero (the common case here) the augmentation is
skipped entirely at compile time.

dtypes: v/fc/attention matmuls run in float32r (fp32 data, FP22 multiply,
fp32 accumulate: full PE rate at free>=256, ~1e-4 error).  The FFN and the
h2o vocab projection stream their weights in bf16 (halves the dominant
HBM traffic; ~5e-4 output error, fp32 accumulation throughout).
"""

import contextlib
import math

import ml_dtypes
import numpy as np

import concourse.mybir as mybir
import concourse.tile as tile
from concourse import bacc
from concourse.bass_utils import run_bass_kernel_spmd
from concourse.kernels.tile_matmul import matmul_tile_kernel
from concourse.masks import make_identity

B, L, D, H, DV, HID, V = 2, 1024, 1024, 16, 64, 4096, 32000
NCORES = 8
IC = L // NCORES        # 128 query rows per core
ROWS = B * IC           # 256 row-instances per core
HD = H * DV             # 1024
P = 128
EPS = 1e-5
NEG_BIG = -1.0e9

F32 = mybir.dt.float32
F32R = mybir.dt.float32r
BF16 = mybir.dt.bfloat16
F16 = mybir.dt.float16
AF = mybir.ActivationFunctionType
ALU = mybir.AluOpType
AX = mybir.AxisListType

# h2o vocab tiling: 62 tiles of 512 + 1 tile of 256
VTILES = [(i * 512, 512) for i in range(62)] + [(62 * 512, 256)]


def _r(ap):
    return ap.bitcast(F32R)


def _softplus(x):
    return np.logaddexp(0.0, x.astype(np.float64))


def _write_aug_block(nc, tc, dst_rows, dtype, tag):
    """Write [ones_row; zeros x 127] into dst_rows ([P, ROWS] DRAM view)."""
    with tc.tile_pool(name=f"aug_{tag}", bufs=1) as pool:
        blk = pool.tile([P, ROWS], dtype, name=f"augblk_{tag}")
        nc.any.memset(blk[:], 0.0)
        nc.any.memset(blk[0:1, :], 1.0)
        nc.sync.dma_start(dst_rows, blk[:])


def _layernorm(nc, tc, src_dram, g_dram, b_dram, dst_dram, ones_col, tag):
    """LN over the feature (partition) axis.  src [D, ROWS] f32 feature-major,
    g/b [P, D//P] per-partition scalars, dst [D, ROWS] (dst dtype = tile's)."""
    DC = D // P
    with contextlib.ExitStack() as ctx:
        lp = ctx.enter_context(tc.tile_pool(name=f"ln_{tag}", bufs=2))
        cp = ctx.enter_context(tc.tile_pool(name=f"lnc_{tag}", bufs=1))
        pp = ctx.enter_context(tc.tile_pool(name=f"lnp_{tag}", bufs=2, space="PSUM"))

        F_sb = lp.tile([P, DC, ROWS], F32R, name=f"F_{tag}")
        srct = _r(src_dram.rearrange("(c p) r -> p c r", p=P))
        for dc in range(DC):
            nc.sync.dma_start(F_sb[:, dc], srct[:, dc])
        SQ = lp.tile([P, DC, ROWS], F32R, name=f"SQ_{tag}")
        nc.vector.tensor_mul(SQ[:], F_sb[:], F_sb[:])

        g_sb = cp.tile([P, DC], F32, name=f"g_{tag}")
        nc.sync.dma_start(g_sb[:], g_dram.ap())
        b_sb = cp.tile([P, DC], F32, name=f"b_{tag}")
        nc.sync.dma_start(b_sb[:], b_dram.ap())

        ps_sum = pp.tile([2, ROWS], F32, name=f"pssum_{tag}")
        ps_sq = pp.tile([2, ROWS], F32, name=f"pssq_{tag}")
        for dc in range(DC):
            nc.tensor.matmul(ps_sum[:], ones_col[:], F_sb[:, dc],
                             start=(dc == 0), stop=(dc == DC - 1))
            nc.tensor.matmul(ps_sq[:], ones_col[:], SQ[:, dc],
                             start=(dc == 0), stop=(dc == DC - 1))

        mean = lp.tile([1, ROWS], F32, name=f"mean_{tag}")
        nc.vector.tensor_scalar(mean[:], ps_sum[0:1, :], 1.0 / D, None, ALU.mult)
        ex2 = lp.tile([1, ROWS], F32, name=f"ex2_{tag}")
        nc.vector.tensor_scalar(ex2[:], ps_sq[0:1, :], 1.0 / D, None, ALU.mult)
        var = lp.tile([1, ROWS], F32, name=f"var_{tag}")
        nc.vector.tensor_mul(var[:], mean[:], mean[:])
        nc.vector.tensor_sub(var[:], ex2[:], var[:])
        # veps = var + eps ; s = sqrt(veps) Newton-polished ; rstd = 1/s
        veps = lp.tile([1, ROWS], F32, name=f"veps_{tag}")
        nc.vector.tensor_scalar(veps[:], var[:], EPS, None, ALU.add)
        s0 = lp.tile([1, ROWS], F32, name=f"s0_{tag}")
        nc.scalar.activation(s0[:], veps[:], AF.Sqrt)
        r0 = lp.tile([1, ROWS], F32, name=f"r0_{tag}")
        nc.vector.reciprocal(r0[:], s0[:])
        s1 = lp.tile([1, ROWS], F32, name=f"s1_{tag}")
        nc.vector.tensor_mul(s1[:], veps[:], r0[:])
        nc.vector.tensor_add(s1[:], s1[:], s0[:])
        nc.vector.tensor_scalar(s1[:], s1[:], 0.5, None, ALU.mult)
        rstd = lp.tile([1, ROWS], F32, name=f"rstd_{tag}")
        nc.vector.reciprocal(rstd[:], s1[:])

        meanB = lp.tile([P, ROWS], F32, name=f"meanB_{tag}")
        nc.gpsimd.partition_broadcast(meanB[:], mean[:])
        rstdB = lp.tile([P, ROWS], F32, name=f"rstdB_{tag}")
        nc.gpsimd.partition_broadcast(rstdB[:], rstd[:])

        Y_sb = lp.tile([P, DC, ROWS], dst_dram.dtype, name=f"Y_{tag}")
        for dc in range(DC):
            t1 = lp.tile([P, ROWS], F32, name=f"t1_{tag}", bufs=3)
            nc.vector.tensor_sub(t1[:], F_sb[:, dc], meanB[:])
            nc.vector.tensor_mul(t1[:], t1[:], rstdB[:])
            nc.vector.tensor_scalar(Y_sb[:, dc], t1[:],
                                    g_sb[:, dc:dc + 1], b_sb[:, dc:dc + 1],
                                    ALU.mult, ALU.add)
        dstt = dst_dram.rearrange("(c p) r -> p c r", p=P)
        for dc in range(DC):
            nc.sync.dma_start(dstt[:, dc], Y_sb[:, dc])


def _build(p0, sp1, sp2, p3, bias_on):
    """Build + compile the SPMD program.  p0/sp1/sp2/p3 are [H] host floats,
    baked into the NEFF as activation immediates.  bias_on: per-stage flags
    (v, fc, w1, w2, h2o) — augmentation emitted only for nonzero biases."""
    p3_zero = bool(np.all(p3 == 0.0))
    n_r = 1 if p3_zero else H

    KV = D + (P if bias_on["v"] else 0)
    KFC = HD + (P if bias_on["fc"] else 0)
    KW1 = D + (P if bias_on["w1"] else 0)
    KW2 = HID + (P if bias_on["w2"] else 0)

    nc = bacc.Bacc(None, target_bir_lowering=False, debug=False,
                   num_devices=NCORES)

    def inp(name, shape, dtype):
        return nc.dram_tensor(name, shape, dtype, kind="ExternalInput")

    xT = inp("xT", [KV, B * L], BF16)
    wvT = inp("wvT", [KV, HD], BF16)
    fcT = inp("fcT", [KFC, D], BF16)
    w1T = inp("w1T", [KW1, HID], BF16)
    w2T = inp("w2T", [KW2, D], BF16)
    h2oT = inp("h2oT", [D, V], BF16)
    if bias_on["h2o"]:
        h2ob = inp("h2ob", [1, V], BF16)
        onesr = inp("onesr", [1, ROWS], BF16)
    onesc = inp("onesc", [P, 2], F32R)           # ones columns (K-sum, even N)
    onesb = inp("onesb", [P, 2], BF16)           # bf16 ones columns
    ln1g = inp("ln1g", [P, D // P], F32)
    ln1b = inp("ln1b", [P, D // P], F32)
    ln2g = inp("ln2g", [P, D // P], F32)
    ln2b = inp("ln2b", [P, D // P], F32)
    S_in = inp("S_in", [P, 8, IC], F32)          # |i-j| tiled [jp, jc, i]
    expb = inp("expb", [P, H], F32)              # per-head exp bias ln(2*p0)
    R_in = inp("R_in", [n_r, P, 8, IC], F32)     # p3*(i<j) - BIG*eye, per head
    y = nc.dram_tensor("y", [ROWS, V], F32, kind="ExternalOutput")

    with tile.TileContext(nc) as tc:
        with tc.tile_pool(name="dram", bufs=1, space="DRAM") as dram:
            v_d = dram.tile([B * L, HD], BF16, name="v_d")
            O_aug = dram.tile([KFC, ROWS], BF16, name="O_aug")
            FC = dram.tile([D, ROWS], F32, name="FC")
            Y_aug = dram.tile([KW1, ROWS], BF16, name="Y_aug")
            H_aug = dram.tile([KW2, ROWS], BF16, name="H_aug")
            FF = dram.tile([D, ROWS], F32, name="FF")
            Z_d = dram.tile([D, ROWS], BF16, name="Z_d")

            @contextlib.contextmanager
            def _mm_pools(tag, bufs=4, cache=False):
                with tc.tile_pool(name=f"mmA_{tag}", bufs=bufs) as a, \
                     tc.tile_pool(name=f"mmB_{tag}", bufs=bufs) as b:
                    yield dict(kxm_pool=a, kxn_pool=b, cache_tiles=cache)

            with tc.tile_pool(name="const0", bufs=1) as c0:
                ones_col = c0.tile([P, 2], F32R, name="ones_col")
                nc.sync.dma_start(ones_col[:], onesc.ap())
                ones_colb = c0.tile([P, 2], BF16, name="ones_colb")
                nc.sync.dma_start(ones_colb[:], onesb.ap())

                # ---- stage A: v = x @ wv.T (+ bv) (row-major [B*L, HD]) ----
                with _mm_pools("v", cache=True) as mp:
                    matmul_tile_kernel(tc, xT.ap(), wvT.ap(), v_d[:], **mp)

                # ---- stage B: attention ----
                with contextlib.ExitStack() as ctx:
                    ap_ = ctx.enter_context(tc.tile_pool(name="attn", bufs=1))
                    up = ctx.enter_context(tc.tile_pool(name="attn_u", bufs=2))
                    sp_ = ctx.enter_context(tc.tile_pool(name="attn_s", bufs=3))
                    cp = ctx.enter_context(tc.tile_pool(name="attn_c", bufs=1))
                    pp = ctx.enter_context(
                        tc.tile_pool(name="attn_p", bufs=2, space="PSUM"))

                    v_sb = ap_.tile([P, B * L // P, HD], BF16, name="v_sb")
                    v_dt = v_d[:].rearrange("(c p) f -> p c f", p=P)
                    for ck in range(B * L // P):
                        nc.sync.dma_start(v_sb[:, ck], v_dt[:, ck])
                    S_sb = cp.tile([P, 8, IC], F32, name="S_sb")
                    nc.sync.dma_start(S_sb[:], S_in.ap())
                    eb_sb = cp.tile([P, H], F32, name="eb_sb")
                    nc.sync.dma_start(eb_sb[:], expb.ap())
                    ident = cp.tile([P, P], F32, name="ident")
                    make_identity(nc, ident[:])
                    R_sb = None
                    O_sb = ap_.tile([P, B, HD], F32, name="O_sb")

                    hkeys = [(float(p0[h]), float(sp1[h]), float(sp2[h]),
                              float(p3[h])) for h in range(H)]
                    n_groups = len(set(hkeys))
                    gup = ctx.enter_context(
                        tc.tile_pool(name="attn_gu", bufs=min(n_groups + 1, H)))
                    grp = {}
                    for h in range(H):
                        if hkeys[h] in grp:
                            u_sb, rs = grp[hkeys[h]]
                        else:
                            if R_sb is None or n_r > 1:
                                R_sb = cp.tile([P, 8, IC], F32, name="R_sb",
                                               bufs=2)
                                nc.sync.dma_start(R_sb[:],
                                                  R_in.ap()[min(h, n_r - 1)])
                            t_sb = up.tile([P, 8, IC], F32, name="t_sb")
                            if p0[h] > 0.0 and abs(sp1[h] - sp2[h]) < 1e-12:
                                nc.scalar.activation(t_sb[:], S_sb[:], AF.Exp,
                                                     scale=-sp1[h],
                                                     bias=eb_sb[:, h:h + 1])
                            elif p0[h] > 0.0:
                                e2 = up.tile([P, 8, IC], F32, name="e2_sb")
                                nc.scalar.activation(t_sb[:], S_sb[:], AF.Exp,
                                                     scale=-sp1[h],
                                                     bias=eb_sb[:, h:h + 1])
                                nc.scalar.activation(e2[:], S_sb[:], AF.Exp,
                                                     scale=-sp2[h],
                                                     bias=eb_sb[:, h:h + 1])
                                nc.vector.tensor_add(t_sb[:], t_sb[:], e2[:])
                            elif p0[h] == 0.0:
                                nc.any.memset(t_sb[:], 0.0)
                            else:
                                e2 = up.tile([P, 8, IC], F32, name="e2_sb")
                                nc.scalar.activation(t_sb[:], S_sb[:], AF.Exp,
                                                     scale=-sp1[h])
                                nc.scalar.activation(e2[:], S_sb[:], AF.Exp,
                                                     scale=-sp2[h])
                                nc.vector.tensor_add(t_sb[:], t_sb[:], e2[:])
                                nc.vector.tensor_scalar(t_sb[:], t_sb[:],
                                                        p0[h], None, ALU.mult)
                            nc.vector.tensor_add(t_sb[:], t_sb[:], R_sb[:])
                            u_sb = gup.tile([P, 8, IC], BF16, name="u_sb")
                            nc.scalar.activation(u_sb[:], t_sb[:], AF.Exp)
                            ps_s = pp.tile([P, 2], F32, name="ps_s")
                            for jc in range(8):
                                nc.tensor.matmul(ps_s[:], u_sb[:, jc],
                                                 ones_colb[:],
                                                 start=(jc == 0),
                                                 stop=(jc == 7))
                            rs = sp_.tile([P, 1], F32, name="rs_t",
                                          bufs=min(n_groups + 1, H))
                            nc.vector.reciprocal(rs[:], ps_s[:, 0:1])
                            grp[hkeys[h]] = (u_sb, rs)

                        ps_o = [pp.tile([P, DV], F32, name=f"ps_o{b}")
                                for b in range(B)]
                        for jc in range(8):
                            lhsT = u_sb[:, jc]
                            for b in range(B):
                                nc.tensor.matmul(
                                    ps_o[b][:], lhsT,
                                    v_sb[:, b * 8 + jc, h * DV:(h + 1) * DV],
                                    start=(jc == 0), stop=(jc == 7))
                        for b in range(B):
                            nc.vector.tensor_scalar(
                                O_sb[:, b, h * DV:(h + 1) * DV],
                                ps_o[b][:], rs[:], None, ALU.mult)

                    OT = ap_.tile([P, HD // P, ROWS], BF16, name="OT")
                    for b in range(B):
                        for hc in range(HD // P):
                            pt = pp.tile([P, P], F32, name="pt")
                            nc.tensor.transpose(
                                pt[:], O_sb[:, b, hc * P:(hc + 1) * P], ident[:])
                            nc.vector.tensor_copy(
                                OT[:, hc, b * IC:(b + 1) * IC], pt[:])
                    O_augt = O_aug[0:HD, :].rearrange("(c p) r -> p c r", p=P)
                    for hc in range(HD // P):
                        nc.sync.dma_start(O_augt[:, hc], OT[:, hc])
                    if bias_on["fc"]:
                        _write_aug_block(nc, tc, O_aug[HD:KFC, :], BF16, "O")

                # ---- stage C: fc + LN1 ----
                with _mm_pools("fc") as mp:
                    matmul_tile_kernel(tc, fcT.ap(), O_aug[:], FC[:], **mp)
                _layernorm(nc, tc, FC[:], ln1g, ln1b, Y_aug[0:D, :],
                           ones_col, "ln1")
                if bias_on["w1"]:
                    _write_aug_block(nc, tc, Y_aug[D:KW1, :], BF16, "Y")

                # ---- stage D: FFN ----
                with _mm_pools("w1") as mp:
                    matmul_tile_kernel(tc, w1T.ap(), Y_aug[:],
                                       H_aug[0:HID, :], use_relu=True, **mp)
                if bias_on["w2"]:
                    _write_aug_block(nc, tc, H_aug[HID:KW2, :], BF16, "H")
                with _mm_pools("w2") as mp:
                    matmul_tile_kernel(tc, w2T.ap(), H_aug[:], FF[:],
                                       accumulate_ap=Y_aug[0:D, :], **mp)
                _layernorm(nc, tc, FF[:], ln2g, ln2b, Z_d[:], ones_col, "ln2")

                # ---- stage E: h2o + log_softmax ----
                with contextlib.ExitStack() as ctx:
                    zp = ctx.enter_context(tc.tile_pool(name="h2o_z", bufs=1))
                    wp = ctx.enter_context(tc.tile_pool(name="h2o_w", bufs=6))
                    ep = ctx.enter_context(tc.tile_pool(name="h2o_e", bufs=3))
                    op_ = ctx.enter_context(tc.tile_pool(name="h2o_o", bufs=4))
                    pp = ctx.enter_context(
                        tc.tile_pool(name="h2o_p", bufs=4, space="PSUM"))

                    Z_sb = zp.tile([P, D // P, ROWS], BF16, name="Z_sb")
                    Z_dt = Z_d[:].rearrange("(c p) r -> p c r", p=P)
                    for dc in range(D // P):
                        nc.sync.dma_start(Z_sb[:, dc], Z_dt[:, dc])
                    L16 = zp.tile([P, B, V], F16, name="L16")          # 16 MB
                    parts = zp.tile([P, B, len(VTILES)], F32, name="parts")
                    if bias_on["h2o"]:
                        ones_row = zp.tile([1, ROWS], BF16, name="ones_row_z")
                        nc.sync.dma_start(ones_row[:], onesr.ap())

                    h2oT_t = h2oT.ap().rearrange("(c p) v -> p c v", p=P)
                    for vi, (vs, vsz) in enumerate(VTILES):
                        W_sb = wp.tile([P, D // P, 512], BF16, name="W_sb")
                        nc.sync.dma_start(W_sb[:, :, :vsz],
                                          h2oT_t[:, :, vs:vs + vsz])
                        if bias_on["h2o"]:
                            bias_sb = ep.tile([1, 512], BF16, name="bias_sb")
                            nc.sync.dma_start(bias_sb[:, :vsz],
                                              h2ob.ap()[:, vs:vs + vsz])
                        for rt in range(B):
                            ps = pp.tile([P, 512], F32, name="ps_l")
                            for dc in range(D // P):
                                nc.tensor.matmul(
                                    ps[:, :vsz],
                                    Z_sb[:, dc, rt * IC:(rt + 1) * IC],
                                    W_sb[:, dc, :vsz],
                                    start=(dc == 0),
                                    stop=(dc == D // P - 1
                                          and not bias_on["h2o"]))
                            if bias_on["h2o"]:
                                nc.tensor.matmul(
                                    ps[:, :vsz],
                                    ones_row[:, rt * IC:(rt + 1) * IC],
                                    bias_sb[:, :vsz],
                                    start=False, stop=True)
                            nc.vector.tensor_copy(L16[:, rt, vs:vs + vsz],
                                                  ps[:, :vsz])
                            esc = ep.tile([P, 512], F32, name="esc", bufs=2)
                            nc.scalar.activation(
                                esc[:, :vsz], ps[:, :vsz], AF.Exp,
                                accum_out=parts[:, rt, vi:vi + 1])

                    for rt in range(B):
                        s_t = ep.tile([P, 1], F32, name="s_t")
                        nc.vector.reduce_sum(s_t[:], parts[:, rt, :], axis=AX.X)
                        lse = ep.tile([P, 1], F32, name="lse_t")
                        nc.scalar.activation(lse[:], s_t[:], AF.Ln)
                        for (vs, vsz) in VTILES:
                            ot = op_.tile([P, 512], F32, name="ot")
                            nc.vector.tensor_scalar(
                                ot[:, :vsz], L16[:, rt, vs:vs + vsz],
                                lse[:], None, ALU.subtract)
                            nc.sync.dma_start(
                                y.ap()[rt * IC:(rt + 1) * IC, vs:vs + vsz],
                                ot[:, :vsz])

    nc.compile()
    return nc


_CACHE = {}


def _aug_pad(wT, bias, dtype=np.float32):
    """[K, M] + bias row + 127 zero rows -> [K+128, M]."""
    K, M = wT.shape
    out = np.zeros((K + P, M), dtype)
    out[:K] = wT
    out[K] = bias
    return out


def kernel(**inputs):
    f32 = np.float32
    bf16 = ml_dtypes.bfloat16
    x = np.asarray(inputs["x"], f32)
    wv = np.asarray(inputs["wv"], f32)
    bv = np.asarray(inputs["bv"], f32)
    fc_w = np.asarray(inputs["fc_w"], f32)
    fc_b = np.asarray(inputs["fc_b"], f32)
    ln1_g = np.asarray(inputs["ln1_g"], f32)
    ln1_b = np.asarray(inputs["ln1_b"], f32)
    w1 = np.asarray(inputs["w1"], f32)
    b1 = np.asarray(inputs["b1"], f32)
    w2 = np.asarray(inputs["w2"], f32)
    b2 = np.asarray(inputs["b2"], f32)
    ln2_g = np.asarray(inputs["ln2_g"], f32)
    ln2_b = np.asarray(inputs["ln2_b"], f32)
    h2o_w = np.asarray(inputs["h2o_w"], f32)
    h2o_b = np.asarray(inputs["h2o_b"], f32)
    p0 = np.asarray(inputs["p0"], np.float64)
    p1 = np.asarray(inputs["p1"], np.float64)
    p2 = np.asarray(inputs["p2"], np.float64)
    p3 = np.asarray(inputs["p3"], np.float64)
    # wk/bk deliberately unused: constant along the softmax axis.

    sp1 = np.float32(_softplus(p1)).astype(np.float64)
    sp2 = np.float32(_softplus(p2)).astype(np.float64)

    bias_on = {
        "v": bool(np.any(bv)),
        "fc": bool(np.any(fc_b)),
        "w1": bool(np.any(b1)),
        "w2": bool(np.any(b2)),
        "h2o": bool(np.any(h2o_b)),
    }

    key = (p0.tobytes(), sp1.tobytes(), sp2.tobytes(), p3.tobytes(),
           tuple(sorted(bias_on.items())))
    if key not in _CACHE:
        _CACHE[key] = _build(p0, sp1, sp2, p3, bias_on)
    nc = _CACHE[key]

    x2T = np.ascontiguousarray(x.reshape(B * L, D).T)
    if bias_on["v"]:
        ones_blk = np.zeros((P, B * L), f32)
        ones_blk[0] = 1.0
        xT_host = np.concatenate([x2T, ones_blk], axis=0)
        wvT_host = _aug_pad(wv.T, bv)
    else:
        xT_host = x2T
        wvT_host = np.ascontiguousarray(wv.T)
    shared = {
        "xT": xT_host,
        "wvT": wvT_host,
        "fcT": _aug_pad(fc_w.T, fc_b) if bias_on["fc"]
               else np.ascontiguousarray(fc_w.T),
        "onesc": np.ones((P, 2), f32),
        "ln1g": np.ascontiguousarray(ln1_g.reshape(D // P, P).T),
        "ln1b": np.ascontiguousarray(ln1_b.reshape(D // P, P).T),
        "ln2g": np.ascontiguousarray(ln2_g.reshape(D // P, P).T),
        "ln2b": np.ascontiguousarray(ln2_b.reshape(D // P, P).T),
    }
    shared = {k: np.ascontiguousarray(a, f32) for k, a in shared.items()}
    for k in ("xT", "wvT", "fcT"):
        shared[k] = np.ascontiguousarray(shared[k].astype(bf16))
    shared["onesb"] = np.ones((P, 2), bf16)
    shared["w1T"] = np.ascontiguousarray(
        _aug_pad(w1.T, b1, bf16) if bias_on["w1"] else w1.T.astype(bf16))
    shared["w2T"] = np.ascontiguousarray(
        _aug_pad(w2.T, b2, bf16) if bias_on["w2"] else w2.T.astype(bf16))
    shared["h2oT"] = np.ascontiguousarray(h2o_w.T.astype(bf16))
    if bias_on["h2o"]:
        shared["h2ob"] = np.ascontiguousarray(h2o_b[None].astype(bf16))
        shared["onesr"] = np.ones((1, ROWS), bf16)

    p3_zero = bool(np.all(p3 == 0.0))
    ebv = np.zeros(H, np.float64)
    for h in range(H):
        if p0[h] > 0.0 and abs(sp1[h] - sp2[h]) < 1e-12:
            ebv[h] = math.log(2.0 * p0[h])
        elif p0[h] > 0.0:
            ebv[h] = math.log(p0[h])
    expb_host = np.ascontiguousarray(
        np.broadcast_to(ebv.astype(f32)[None, :], (P, H)))

    j = np.arange(L)
    in_maps = []
    for c in range(NCORES):
        i_idx = c * IC + np.arange(IC)
        Sji = np.abs(j[:, None] - i_idx[None, :]).astype(f32)       # [L, IC]
        eye = (Sji == 0).astype(f32)
        if p3_zero:
            Rs = [NEG_BIG * eye]
        else:
            Aji = (i_idx[None, :] < j[:, None]).astype(f32)
            Rs = [np.float32(p3[h]) * Aji + NEG_BIG * eye for h in range(H)]

        def tile_ji(a):  # [L, IC] -> [jp, jc, IC]
            return np.ascontiguousarray(
                a.reshape(8, P, IC).transpose(1, 0, 2), f32)

        m = dict(shared)
        m["S_in"] = tile_ji(Sji)
        m["expb"] = expb_host
        m["R_in"] = np.stack([tile_ji(R) for R in Rs], axis=0)
        in_maps.append(m)

    res = run_bass_kernel_spmd(nc, in_maps, core_ids=list(range(NCORES)))

    out = np.empty((B, L, V), f32)
    for c in range(NCORES):
        yc = res.results[c]["y"]
        for b in range(B):
            out[b, c * IC:(c + 1) * IC, :] = yc[b * IC:(b + 1) * IC, :]
    return out


# revision 18
# speedup vs baseline: 1.3851x; 1.0237x over previous
"""Trainium2 Bass kernel for a single-layer "BiTRF" dense transformer block.

Math (see reference):
  posi[h,i,j] = p0*(exp(-sp1*|i-j|) + exp(-sp2*|i-j|)) + p3*(i<j)   (sp=softplus(p))
  attn[h,b,i,j] = kproj[b,i,h] + posi[h,i,j], diag masked, softmax over j.
  Because kproj[b,i,h] is constant along the softmax axis j, softmax is
  invariant to it, so the wk/bk projection drops out entirely and the
  attention weights W[h,i,:] are shared across the batch (and across heads
  with identical (p0, sp1, sp2, p3) — computed once per unique group).
  out  = LN1(attnout @ fc_w.T + fc_b)
  out2 = LN2(relu(out @ w1.T + b1) @ w2.T + b2 + out)
  y    = log_softmax(out2 @ h2o_w.T + h2o_b)

Sharding: 8 cores, core c owns query rows i in [c*128,(c+1)*128) for BOTH
batches (256 row-instances).  v = x@wv.T is computed redundantly on every
core (avoids any collective); everything else is row-sharded, h2o is
row-sharded too (each core computes its rows x full 32000 vocab, so
log_softmax is fully local).

The whole pre-h2o chain (v-proj, attention, fc, LN1, FFN, LN2) lives in
SBUF — no DRAM round-trips between stages.  Activations are feature-major
[feat, row] (LN partition reductions via ones-column matmuls); attention
output is transposed back with PE-transposes.  Biases are applied as
per-partition scalars at PSUM-eviction time (bv rides through the
attention because softmax rows sum to 1, so it is added at the transpose
eviction).  The h2o weight stream pool lives at top scope so its first
tiles prefetch during earlier phases.

dtypes: matmuls run bf16 (weights pre-cast on host, fp32 PSUM
accumulation); LayerNorm statistics and log-sum-exp run in fp32; raw
logits are staged in fp16 for the final lse subtraction.
"""

import contextlib
import math

import ml_dtypes
import numpy as np

import concourse.mybir as mybir
import concourse.tile as tile
from concourse import bacc
from concourse.bass_utils import run_bass_kernel_spmd
from concourse.masks import make_identity

B, L, D, H, DV, HID, V = 2, 1024, 1024, 16, 64, 4096, 32000
NCORES = 8
IC = L // NCORES        # 128 query rows per core
ROWS = B * IC           # 256 row-instances per core
HD = H * DV             # 1024
P = 128
DC = D // P             # 8 feature chunks
HC = HID // P           # 32 hidden chunks
EPS = 1e-5
NEG_BIG = -1.0e9

F32 = mybir.dt.float32
F32R = mybir.dt.float32r
BF16 = mybir.dt.bfloat16
F16 = mybir.dt.float16
AF = mybir.ActivationFunctionType
ALU = mybir.AluOpType
AX = mybir.AxisListType

# h2o vocab tiling: 62 tiles of 512 + 1 tile of 256
VTILES = [(i * 512, 512) for i in range(62)] + [(62 * 512, 256)]


def _r(ap):
    return ap.bitcast(F32R)


def _softplus(x):
    return np.logaddexp(0.0, x.astype(np.float64))


def _layernorm_sb(nc, tc, F_sb, g_dram, b_dram, Y_sb, ones_col, tag):
    """LN over the feature (partition) axis, fully in SBUF.
    F_sb: [P, DC, ROWS] f32r source; Y_sb: [P, DC, ROWS] dst (any dtype)."""
    with contextlib.ExitStack() as ctx:
        lp = ctx.enter_context(tc.tile_pool(name=f"ln_{tag}", bufs=2))
        cp = ctx.enter_context(tc.tile_pool(name=f"lnc_{tag}", bufs=1))
        pp = ctx.enter_context(tc.tile_pool(name=f"lnp_{tag}", bufs=2, space="PSUM"))

        SQ = lp.tile([P, DC, ROWS], F32R, name=f"SQ_{tag}")
        nc.vector.tensor_mul(SQ[:], F_sb[:], F_sb[:])

        g_sb = cp.tile([P, DC], F32, name=f"g_{tag}")
        nc.sync.dma_start(g_sb[:], g_dram.ap())
        b_sb = cp.tile([P, DC], F32, name=f"b_{tag}")
        nc.sync.dma_start(b_sb[:], b_dram.ap())

        ps_sum = pp.tile([2, ROWS], F32, name=f"pssum_{tag}")
        ps_sq = pp.tile([2, ROWS], F32, name=f"pssq_{tag}")
        for dc in range(DC):
            nc.tensor.matmul(ps_sum[:], ones_col[:], F_sb[:, dc],
                             start=(dc == 0), stop=(dc == DC - 1))
            nc.tensor.matmul(ps_sq[:], ones_col[:], SQ[:, dc],
                             start=(dc == 0), stop=(dc == DC - 1))

        mean = lp.tile([1, ROWS], F32, name=f"mean_{tag}")
        nc.vector.tensor_scalar(mean[:], ps_sum[0:1, :], 1.0 / D, None, ALU.mult)
        ex2 = lp.tile([1, ROWS], F32, name=f"ex2_{tag}")
        nc.vector.tensor_scalar(ex2[:], ps_sq[0:1, :], 1.0 / D, None, ALU.mult)
        var = lp.tile([1, ROWS], F32, name=f"var_{tag}")
        nc.vector.tensor_mul(var[:], mean[:], mean[:])
        nc.vector.tensor_sub(var[:], ex2[:], var[:])
        veps = lp.tile([1, ROWS], F32, name=f"veps_{tag}")
        nc.vector.tensor_scalar(veps[:], var[:], EPS, None, ALU.add)
        s0 = lp.tile([1, ROWS], F32, name=f"s0_{tag}")
        nc.scalar.activation(s0[:], veps[:], AF.Sqrt)
        r0 = lp.tile([1, ROWS], F32, name=f"r0_{tag}")
        nc.vector.reciprocal(r0[:], s0[:])
        s1 = lp.tile([1, ROWS], F32, name=f"s1_{tag}")
        nc.vector.tensor_mul(s1[:], veps[:], r0[:])
        nc.vector.tensor_add(s1[:], s1[:], s0[:])
        nc.vector.tensor_scalar(s1[:], s1[:], 0.5, None, ALU.mult)
        rstd = lp.tile([1, ROWS], F32, name=f"rstd_{tag}")
        nc.vector.reciprocal(rstd[:], s1[:])

        meanB = lp.tile([P, ROWS], F32, name=f"meanB_{tag}")
        nc.gpsimd.partition_broadcast(meanB[:], mean[:])
        rstdB = lp.tile([P, ROWS], F32, name=f"rstdB_{tag}")
        nc.gpsimd.partition_broadcast(rstdB[:], rstd[:])

        for dc in range(DC):
            t1 = lp.tile([P, ROWS], F32, name=f"t1_{tag}", bufs=3)
            nc.vector.tensor_sub(t1[:], F_sb[:, dc], meanB[:])
            nc.vector.tensor_mul(t1[:], t1[:], rstdB[:])
            nc.vector.tensor_scalar(Y_sb[:, dc], t1[:],
                                    g_sb[:, dc:dc + 1], b_sb[:, dc:dc + 1],
                                    ALU.mult, ALU.add)


def _build(p0, sp1, sp2, p3, bias_on):
    """Build + compile the SPMD program.  p0/sp1/sp2/p3 are [H] host floats
    baked into the NEFF as activation immediates; bias_on['h2o'] selects the
    rank-1 vocab-bias matmul (other biases are always applied, free)."""
    p3_zero = bool(np.all(p3 == 0.0))
    n_r = 1 if p3_zero else H

    nc = bacc.Bacc(None, target_bir_lowering=False, debug=False,
                   num_devices=NCORES)

    def inp(name, shape, dtype):
        return nc.dram_tensor(name, shape, dtype, kind="ExternalInput")

    xT = inp("xT", [D, B * L], BF16)
    wvT = inp("wvT", [D, HD], BF16)
    fcT = inp("fcT", [HD, D], BF16)
    w1T = inp("w1T", [D, HID], BF16)
    w2T = inp("w2T", [HID, D], BF16)
    h2oT = inp("h2oT", [D, V], BF16)
    bv2 = inp("bv2", [P, DC], F32)
    fcb2 = inp("fcb2", [P, DC], F32)
    b12 = inp("b12", [P, HC], F32)
    b22 = inp("b22", [P, DC], F32)
    if bias_on["h2o"]:
        h2ob = inp("h2ob", [1, V], BF16)
        onesr = inp("onesr", [1, ROWS], BF16)
    onesc = inp("onesc", [P, 2], F32R)
    onesb = inp("onesb", [P, 2], BF16)
    ln1g = inp("ln1g", [P, DC], F32)
    ln1b = inp("ln1b", [P, DC], F32)
    ln2g = inp("ln2g", [P, DC], F32)
    ln2b = inp("ln2b", [P, DC], F32)
    S_in = inp("S_in", [P, 8, IC], F32)          # |i-j| tiled [jp, jc, i]
    expb = inp("expb", [P, H], F32)              # per-head exp bias ln(2*p0)
    R_in = inp("R_in", [n_r, P, 8, IC], F32)     # p3*(i<j) - BIG*eye, per head
    y = nc.dram_tensor("y", [ROWS, V], F32, kind="ExternalOutput")

    with tile.TileContext(nc) as tc, contextlib.ExitStack() as top:
        c0 = top.enter_context(tc.tile_pool(name="const0", bufs=1))
        wp = top.enter_context(tc.tile_pool(name="h2o_w", bufs=6))
        zp = top.enter_context(tc.tile_pool(name="zmid", bufs=1))

        ones_col = c0.tile([P, 2], F32R, name="ones_col")
        nc.sync.dma_start(ones_col[:], onesc.ap())
        ones_colb = c0.tile([P, 2], BF16, name="ones_colb")
        nc.sync.dma_start(ones_colb[:], onesb.ap())
        Z_sb = zp.tile([P, DC, ROWS], BF16, name="Z_sb")

        # h2o weight stream: pool at top scope -> first tiles prefetch early
        h2oT_t = h2oT.ap().rearrange("(c p) v -> p c v", p=P)
        W_sbs = []
        for vi, (vs, vsz) in enumerate(VTILES):
            W_sb = wp.tile([P, DC, 512], BF16, name="W_sb")
            nc.sync.dma_start(W_sb[:, :, :vsz], h2oT_t[:, :, vs:vs + vsz])
            W_sbs.append(W_sb)

        with contextlib.ExitStack() as s1:
            OT = s1.enter_context(tc.tile_pool(name="otp", bufs=1)).tile(
                [P, DC, ROWS], BF16, name="OT")
            sab = contextlib.ExitStack()
            # ---------------- stage A: v = x @ wv.T ----------------
            vp = sab.enter_context(tc.tile_pool(name="vpool", bufs=1))
            v_sb = vp.tile([P, B * L // P, HD], BF16, name="v_sb")
            with contextlib.ExitStack() as sa:
                rp = sa.enter_context(tc.tile_pool(name="resid", bufs=1))
                pa = sa.enter_context(tc.tile_pool(name="psA", bufs=4,
                                                   space="PSUM"))
                xT_sb = rp.tile([P, DC, B * L], BF16, name="xT_sb")
                xT_t = xT.ap().rearrange("(c p) r -> p c r", p=P)
                for dc in range(DC):
                    nc.sync.dma_start(xT_sb[:, dc], xT_t[:, dc])
                wvT_sb = rp.tile([P, DC, HD], BF16, name="wvT_sb")
                nc.sync.dma_start(wvT_sb[:],
                                  wvT.ap().rearrange("(c p) f -> p c f", p=P))
                for rc in range(B * L // P):
                    for nh in range(2):
                        psv = pa.tile([P, 512], F32, name="psv")
                        for dc in range(DC):
                            nc.tensor.matmul(
                                psv[:],
                                xT_sb[:, dc, rc * P:(rc + 1) * P],
                                wvT_sb[:, dc, nh * 512:(nh + 1) * 512],
                                start=(dc == 0), stop=(dc == DC - 1))
                        nc.vector.tensor_copy(
                            v_sb[:, rc, nh * 512:(nh + 1) * 512], psv[:])

            # ---------------- stage B: attention ----------------
            # (bv is added at the transpose eviction: softmax rows sum to 1)
            with sab, contextlib.ExitStack() as sb:
                up = sb.enter_context(tc.tile_pool(name="attn_u", bufs=2))
                sp_ = sb.enter_context(tc.tile_pool(name="attn_s", bufs=3))
                cp = sb.enter_context(tc.tile_pool(name="attn_c", bufs=1))
                ab = sb.enter_context(tc.tile_pool(name="attn_b", bufs=1))
                pp = sb.enter_context(tc.tile_pool(name="attn_p", bufs=2,
                                                   space="PSUM"))

                S_sb = cp.tile([P, 8, IC], F32, name="S_sb")
                nc.sync.dma_start(S_sb[:], S_in.ap())
                eb_sb = cp.tile([P, H], F32, name="eb_sb")
                nc.sync.dma_start(eb_sb[:], expb.ap())
                bv_sb = cp.tile([P, DC], F32, name="bv_sb")
                nc.sync.dma_start(bv_sb[:], bv2.ap())
                ident = cp.tile([P, P], F32, name="ident")
                make_identity(nc, ident[:])
                R_sb = None
                O_sb = ab.tile([P, B, HD], F32, name="O_sb")

                hkeys = [(float(p0[h]), float(sp1[h]), float(sp2[h]),
                          float(p3[h])) for h in range(H)]
                n_groups = len(set(hkeys))
                gup = sb.enter_context(
                    tc.tile_pool(name="attn_gu", bufs=min(n_groups + 1, H)))
                grp = {}
                for h in range(H):
                    if hkeys[h] in grp:
                        u_sb, rs = grp[hkeys[h]]
                    else:
                        if R_sb is None or n_r > 1:
                            R_sb = cp.tile([P, 8, IC], F32, name="R_sb",
                                           bufs=2)
                            nc.sync.dma_start(R_sb[:],
                                              R_in.ap()[min(h, n_r - 1)])
                        t_sb = up.tile([P, 8, IC], F32, name="t_sb")
                        if p0[h] > 0.0 and abs(sp1[h] - sp2[h]) < 1e-12:
                            nc.scalar.activation(t_sb[:], S_sb[:], AF.Exp,
                                                 scale=-sp1[h],
                                                 bias=eb_sb[:, h:h + 1])
                        elif p0[h] > 0.0:
                            e2 = up.tile([P, 8, IC], F32, name="e2_sb")
                            nc.scalar.activation(t_sb[:], S_sb[:], AF.Exp,
                                                 scale=-sp1[h],
                                                 bias=eb_sb[:, h:h + 1])
                            nc.scalar.activation(e2[:], S_sb[:], AF.Exp,
                                                 scale=-sp2[h],
                                                 bias=eb_sb[:, h:h + 1])
                            nc.vector.tensor_add(t_sb[:], t_sb[:], e2[:])
                        elif p0[h] == 0.0:
                            nc.any.memset(t_sb[:], 0.0)
                        else:
                            e2 = up.tile([P, 8, IC], F32, name="e2_sb")
                            nc.scalar.activation(t_sb[:], S_sb[:], AF.Exp,
                                                 scale=-sp1[h])
                            nc.scalar.activation(e2[:], S_sb[:], AF.Exp,
                                                 scale=-sp2[h])
                            nc.vector.tensor_add(t_sb[:], t_sb[:], e2[:])
                            nc.vector.tensor_scalar(t_sb[:], t_sb[:], p0[h],
                                                    None, ALU.mult)
                        nc.vector.tensor_add(t_sb[:], t_sb[:], R_sb[:])
                        u_sb = gup.tile([P, 8, IC], BF16, name="u_sb")
                        nc.scalar.activation(u_sb[:], t_sb[:], AF.Exp)
                        ps_s = pp.tile([P, 2], F32, name="ps_s")
                        for jc in range(8):
                            nc.tensor.matmul(ps_s[:], u_sb[:, jc],
                                             ones_colb[:],
                                             start=(jc == 0), stop=(jc == 7))
                        rs = sp_.tile([P, 1], F32, name="rs_t",
                                      bufs=min(n_groups + 1, H))
                        nc.vector.reciprocal(rs[:], ps_s[:, 0:1])
                        grp[hkeys[h]] = (u_sb, rs)

                    ps_o = [pp.tile([P, DV], F32, name=f"ps_o{b}")
                            for b in range(B)]
                    for jc in range(8):
                        lhsT = u_sb[:, jc]
                        for b in range(B):
                            nc.tensor.matmul(
                                ps_o[b][:], lhsT,
                                v_sb[:, b * 8 + jc, h * DV:(h + 1) * DV],
                                start=(jc == 0), stop=(jc == 7))
                    for b in range(B):
                        nc.vector.tensor_scalar(
                            O_sb[:, b, h * DV:(h + 1) * DV],
                            ps_o[b][:], rs[:], None, ALU.mult)

                # transpose row-major [i, hd] -> feature-major [hd, (b,i)],
                # adding bv (exact: softmax rows sum to 1)
                for b in range(B):
                    for hc in range(DC):
                        pt = pp.tile([P, P], F32, name="pt")
                        nc.tensor.transpose(
                            pt[:], O_sb[:, b, hc * P:(hc + 1) * P], ident[:])
                        nc.vector.tensor_scalar(
                            OT[:, hc, b * IC:(b + 1) * IC], pt[:],
                            bv_sb[:, hc:hc + 1], None, ALU.add)

            # ---------------- stage C: fc + LN1 ----------------
            yp = s1.enter_context(tc.tile_pool(name="ypool", bufs=1))
            Y_sb = yp.tile([P, DC, ROWS], BF16, name="Y_sb")
            with contextlib.ExitStack() as sc:
                fp = sc.enter_context(tc.tile_pool(name="fcp", bufs=1))
                cc = sc.enter_context(tc.tile_pool(name="fcc", bufs=1))
                pc = sc.enter_context(tc.tile_pool(name="psC", bufs=4,
                                                   space="PSUM"))
                fcT_sb = cc.tile([P, DC, D], BF16, name="fcT_sb")
                nc.sync.dma_start(fcT_sb[:],
                                  fcT.ap().rearrange("(c p) f -> p c f", p=P))
                fcb_sb = cc.tile([P, DC], F32, name="fcb_sb")
                nc.sync.dma_start(fcb_sb[:], fcb2.ap())
                F1 = fp.tile([P, DC, ROWS], F32R, name="F1")
                for do in range(DC):
                    psf = pc.tile([P, ROWS], F32, name="psf")
                    for hc in range(DC):
                        nc.tensor.matmul(
                            psf[:],
                            fcT_sb[:, hc, do * P:(do + 1) * P],
                            OT[:, hc, :],
                            start=(hc == 0), stop=(hc == DC - 1))
                    nc.vector.tensor_scalar(F1[:, do], psf[:],
                                            fcb_sb[:, do:do + 1], None,
                                            ALU.add)
                _layernorm_sb(nc, tc, F1, ln1g, ln1b, Y_sb, ones_col, "ln1")

            # ---------------- stage D: FFN ----------------
            with contextlib.ExitStack() as sd:
                hp = sd.enter_context(tc.tile_pool(name="hpool", bufs=1))
                wsp = sd.enter_context(tc.tile_pool(name="wstr", bufs=2))
                w2p = sd.enter_context(tc.tile_pool(name="w2str", bufs=5))
                cd = sd.enter_context(tc.tile_pool(name="cD", bufs=1))
                pd = sd.enter_context(tc.tile_pool(name="psD", bufs=2,
                                                   space="PSUM"))
                H_sb = hp.tile([P, HC, ROWS], BF16, name="H_sb")
                b1_sb = cd.tile([P, HC], F32, name="b1_sb")
                nc.sync.dma_start(b1_sb[:], b12.ap())
                b2_sb = cd.tile([P, DC], F32, name="b2_sb")
                nc.sync.dma_start(b2_sb[:], b22.ap())

                w1T_t = w1T.ap().rearrange("(c p) m -> p c m", p=P)
                for hs in range(8):           # 512-wide hid slices
                    W1t = wsp.tile([P, DC, 512], BF16, name="W1t")
                    nc.sync.dma_start(W1t[:],
                                      w1T_t[:, :, hs * 512:(hs + 1) * 512])
                    for m2 in range(4):       # 128-wide subchunks
                        psh = pd.tile([P, ROWS], F32, name="psh")
                        for dc in range(DC):
                            nc.tensor.matmul(
                                psh[:],
                                W1t[:, dc, m2 * P:(m2 + 1) * P],
                                Y_sb[:, dc, :],
                                start=(dc == 0), stop=(dc == DC - 1))
                        hcix = hs * 4 + m2
                        nc.scalar.activation(H_sb[:, hcix], psh[:], AF.Relu,
                                             bias=b1_sb[:, hcix:hcix + 1])

                FF = hp.tile([P, DC, ROWS], F32R, name="FF")
                w2T_t = w2T.ap().rearrange("(c p) m -> p c m", p=P)
                for ds2 in range(2):          # 512-wide d slices
                    W2ts = []
                    for g in range(4):
                        W2t = w2p.tile([P, 8, 512], BF16, name="W2t")
                        nc.sync.dma_start(
                            W2t[:],
                            w2T_t[:, g * 8:(g + 1) * 8,
                                  ds2 * 512:(ds2 + 1) * 512])
                        W2ts.append(W2t)
                    for m2 in range(4):
                        do = ds2 * 4 + m2
                        psw = pd.tile([P, ROWS], F32, name="psw")
                        for hc in range(HC):
                            nc.tensor.matmul(
                                psw[:],
                                W2ts[hc // 8][:, hc % 8, m2 * P:(m2 + 1) * P],
                                H_sb[:, hc, :],
                                start=(hc == 0), stop=(hc == HC - 1))
                        nc.vector.tensor_scalar(psw[:], psw[:],
                                                b2_sb[:, do:do + 1], None,
                                                ALU.add)
                        nc.vector.tensor_add(FF[:, do], psw[:], Y_sb[:, do])
                _layernorm_sb(nc, tc, FF, ln2g, ln2b, Z_sb, ones_col, "ln2")

        # ---------------- stage E: h2o + log_softmax ----------------
        with contextlib.ExitStack() as se:
            ep = se.enter_context(tc.tile_pool(name="h2o_e", bufs=3))
            op_ = se.enter_context(tc.tile_pool(name="h2o_o", bufs=4))
            lp_ = se.enter_context(tc.tile_pool(name="h2o_l", bufs=1))
            pp = se.enter_context(tc.tile_pool(name="h2o_p", bufs=4,
                                               space="PSUM"))

            L16 = lp_.tile([P, B, V], F16, name="L16")          # 16 MB
            parts = lp_.tile([P, B, len(VTILES)], F32, name="parts")
            if bias_on["h2o"]:
                ones_row = lp_.tile([1, ROWS], BF16, name="ones_row_z")
                nc.sync.dma_start(ones_row[:], onesr.ap())

            for vi, (vs, vsz) in enumerate(VTILES):
                W_sb = W_sbs[vi]
                if bias_on["h2o"]:
                    bias_sb = ep.tile([1, 512], BF16, name="bias_sb")
                    nc.sync.dma_start(bias_sb[:, :vsz],
                                      h2ob.ap()[:, vs:vs + vsz])
                for rt in range(B):
                    ps = pp.tile([P, 512], F32, name="ps_l")
                    for dc in range(DC):
                        nc.tensor.matmul(
                            ps[:, :vsz],
                            Z_sb[:, dc, rt * IC:(rt + 1) * IC],
                            W_sb[:, dc, :vsz],
                            start=(dc == 0),
                            stop=(dc == DC - 1 and not bias_on["h2o"]))
                    if bias_on["h2o"]:
                        nc.tensor.matmul(
                            ps[:, :vsz],
                            ones_row[:, rt * IC:(rt + 1) * IC],
                            bias_sb[:, :vsz],
                            start=False, stop=True)
                    nc.vector.tensor_copy(L16[:, rt, vs:vs + vsz],
                                          ps[:, :vsz])
                    esc = ep.tile([P, 512], F32, name="esc", bufs=2)
                    nc.scalar.activation(
                        esc[:, :vsz], ps[:, :vsz], AF.Exp,
                        accum_out=parts[:, rt, vi:vi + 1])

            for rt in range(B):
                s_t = ep.tile([P, 1], F32, name="s_t")
                nc.vector.reduce_sum(s_t[:], parts[:, rt, :], axis=AX.X)
                lse = ep.tile([P, 1], F32, name="lse_t")
                nc.scalar.activation(lse[:], s_t[:], AF.Ln)
                for ti, (vs, vsz) in enumerate(VTILES):
                    ot = op_.tile([P, 512], F32, name="ot")
                    eng = nc.vector if ti % 2 == 0 else nc.gpsimd
                    eng.tensor_scalar(ot[:, :vsz], L16[:, rt, vs:vs + vsz],
                                      lse[:], None, ALU.subtract)
                    nc.sync.dma_start(
                        y.ap()[rt * IC:(rt + 1) * IC, vs:vs + vsz],
                        ot[:, :vsz])

    nc.compile()
    return nc


_CACHE = {}


def _ppart(vec, chunks):
    """[chunks*P] -> [P, chunks] per-partition layout."""
    return np.ascontiguousarray(vec.reshape(chunks, P).T, np.float32)


def kernel(**inputs):
    f32 = np.float32
    bf16 = ml_dtypes.bfloat16
    x = np.asarray(inputs["x"], f32)
    wv = np.asarray(inputs["wv"], f32)
    bv = np.asarray(inputs["bv"], f32)
    fc_w = np.asarray(inputs["fc_w"], f32)
    fc_b = np.asarray(inputs["fc_b"], f32)
    ln1_g = np.asarray(inputs["ln1_g"], f32)
    ln1_b = np.asarray(inputs["ln1_b"], f32)
    w1 = np.asarray(inputs["w1"], f32)
    b1 = np.asarray(inputs["b1"], f32)
    w2 = np.asarray(inputs["w2"], f32)
    b2 = np.asarray(inputs["b2"], f32)
    ln2_g = np.asarray(inputs["ln2_g"], f32)
    ln2_b = np.asarray(inputs["ln2_b"], f32)
    h2o_w = np.asarray(inputs["h2o_w"], f32)
    h2o_b = np.asarray(inputs["h2o_b"], f32)
    p0 = np.asarray(inputs["p0"], np.float64)
    p1 = np.asarray(inputs["p1"], np.float64)
    p2 = np.asarray(inputs["p2"], np.float64)
    p3 = np.asarray(inputs["p3"], np.float64)
    # wk/bk deliberately unused: constant along the softmax axis.

    sp1 = np.float32(_softplus(p1)).astype(np.float64)
    sp2 = np.float32(_softplus(p2)).astype(np.float64)

    bias_on = {"h2o": bool(np.any(h2o_b))}

    key = (p0.tobytes(), sp1.tobytes(), sp2.tobytes(), p3.tobytes(),
           bias_on["h2o"])
    if key not in _CACHE:
        _CACHE[key] = _build(p0, sp1, sp2, p3, bias_on)
    nc = _CACHE[key]

    x2T = np.ascontiguousarray(x.reshape(B * L, D).T)
    shared = {
        "xT": np.ascontiguousarray(x2T.astype(bf16)),
        "wvT": np.ascontiguousarray(wv.T.astype(bf16)),
        "fcT": np.ascontiguousarray(fc_w.T.astype(bf16)),
        "w1T": np.ascontiguousarray(w1.T.astype(bf16)),
        "w2T": np.ascontiguousarray(w2.T.astype(bf16)),
        "h2oT": np.ascontiguousarray(h2o_w.T.astype(bf16)),
        "bv2": _ppart(bv, DC),
        "fcb2": _ppart(fc_b, DC),
        "b12": _ppart(b1, HC),
        "b22": _ppart(b2, DC),
        "onesc": np.ones((P, 2), f32),
        "onesb": np.ones((P, 2), bf16),
        "ln1g": _ppart(ln1_g, DC),
        "ln1b": _ppart(ln1_b, DC),
        "ln2g": _ppart(ln2_g, DC),
        "ln2b": _ppart(ln2_b, DC),
    }
    if bias_on["h2o"]:
        shared["h2ob"] = np.ascontiguousarray(h2o_b[None].astype(bf16))
        shared["onesr"] = np.ones((1, ROWS), bf16)

    p3_zero = bool(np.all(p3 == 0.0))
    ebv = np.zeros(H, np.float64)
    for h in range(H):
        if p0[h] > 0.0 and abs(sp1[h] - sp2[h]) < 1e-12:
            ebv[h] = math.log(2.0 * p0[h])
        elif p0[h] > 0.0:
            ebv[h] = math.log(p0[h])
    expb_host = np.ascontiguousarray(
        np.broadcast_to(ebv.astype(f32)[None, :], (P, H)))

    j = np.arange(L)
    in_maps = []
    for c in range(NCORES):
        i_idx = c * IC + np.arange(IC)
        Sji = np.abs(j[:, None] - i_idx[None, :]).astype(f32)       # [L, IC]
        eye = (Sji == 0).astype(f32)
        if p3_zero:
            Rs = [NEG_BIG * eye]
        else:
            Aji = (i_idx[None, :] < j[:, None]).astype(f32)
            Rs = [np.float32(p3[h]) * Aji + NEG_BIG * eye for h in range(H)]

        def tile_ji(a):  # [L, IC] -> [jp, jc, IC]
            return np.ascontiguousarray(
                a.reshape(8, P, IC).transpose(1, 0, 2), f32)

        m = dict(shared)
        m["S_in"] = tile_ji(Sji)
        m["expb"] = expb_host
        m["R_in"] = np.stack([tile_ji(R) for R in Rs], axis=0)
        in_maps.append(m)

    res = run_bass_kernel_spmd(nc, in_maps, core_ids=list(range(NCORES)))

    out = np.empty((B, L, V), f32)
    for c in range(NCORES):
        yc = res.results[c]["y"]
        for b in range(B):
            out[b, c * IC:(c + 1) * IC, :] = yc[b * IC:(b + 1) * IC, :]
    return out


# revision 20
# speedup vs baseline: 1.4630x; 1.0562x over previous
"""Trainium2 Bass kernel for a single-layer "BiTRF" dense transformer block.

Math (see reference):
  posi[h,i,j] = p0*(exp(-sp1*|i-j|) + exp(-sp2*|i-j|)) + p3*(i<j)   (sp=softplus(p))
  attn[h,b,i,j] = kproj[b,i,h] + posi[h,i,j], diag masked, softmax over j.
  Because kproj[b,i,h] is constant along the softmax axis j, softmax is
  invariant to it, so the wk/bk projection drops out entirely and the
  attention weights W[h,i,:] are shared across the batch (and across heads
  with identical (p0, sp1, sp2, p3) — computed once per unique group).
  out  = LN1(attnout @ fc_w.T + fc_b)
  out2 = LN2(relu(out @ w1.T + b1) @ w2.T + b2 + out)
  y    = log_softmax(out2 @ h2o_w.T + h2o_b)

Sharding: 8 cores, core c owns query rows i in [c*128,(c+1)*128) for BOTH
batches (256 row-instances).  v = x@wv.T is computed redundantly on every
core (avoids any collective); everything else is row-sharded, h2o is
row-sharded too (each core computes its rows x full 32000 vocab, so
log_softmax is fully local).

The whole pre-h2o chain (v-proj, attention, fc, LN1, FFN, LN2) lives in
SBUF — no DRAM round-trips between stages.  Activations are feature-major
[feat, row] (LN partition reductions via ones-column matmuls); attention
output is transposed back with PE-transposes.  Biases are applied as
per-partition scalars at PSUM-eviction time (bv rides through the
attention because softmax rows sum to 1, so it is added at the transpose
eviction).  The h2o weight stream pool lives at top scope so its first
tiles prefetch during earlier phases.

dtypes: matmuls run bf16 (weights pre-cast on host, fp32 PSUM
accumulation); LayerNorm statistics and log-sum-exp run in fp32; raw
logits are staged in fp16 for the final lse subtraction.
"""

import contextlib
import math

import ml_dtypes
import numpy as np

import concourse.mybir as mybir
import concourse.tile as tile
from concourse import bacc
from concourse.bass_utils import run_bass_kernel_spmd
from concourse.masks import make_identity

B, L, D, H, DV, HID, V = 2, 1024, 1024, 16, 64, 4096, 32000
NCORES = 8
IC = L // NCORES        # 128 query rows per core
ROWS = B * IC           # 256 row-instances per core
HD = H * DV             # 1024
P = 128
DC = D // P             # 8 feature chunks
HC = HID // P           # 32 hidden chunks
EPS = 1e-5
NEG_BIG = -1.0e9

F32 = mybir.dt.float32
F32R = mybir.dt.float32r
BF16 = mybir.dt.bfloat16
F16 = mybir.dt.float16
AF = mybir.ActivationFunctionType
ALU = mybir.AluOpType
AX = mybir.AxisListType

# h2o vocab tiling: 62 tiles of 512 + 1 tile of 256
VTILES = [(i * 512, 512) for i in range(62)] + [(62 * 512, 256)]


def _r(ap):
    return ap.bitcast(F32R)


def _softplus(x):
    return np.logaddexp(0.0, x.astype(np.float64))


def _layernorm_sb(nc, tc, F_sb, g_dram, b_dram, Y_sb, ones_col, tag):
    """LN over the feature (partition) axis, fully in SBUF.
    F_sb: [P, DC, ROWS] f32r source; Y_sb: [P, DC, ROWS] dst (any dtype)."""
    with contextlib.ExitStack() as ctx:
        lp = ctx.enter_context(tc.tile_pool(name=f"ln_{tag}", bufs=2))
        cp = ctx.enter_context(tc.tile_pool(name=f"lnc_{tag}", bufs=1))
        pp = ctx.enter_context(tc.tile_pool(name=f"lnp_{tag}", bufs=2, space="PSUM"))

        SQ = lp.tile([P, DC, ROWS], F32R, name=f"SQ_{tag}")
        nc.vector.tensor_mul(SQ[:], F_sb[:], F_sb[:])

        g_sb = cp.tile([P, DC], F32, name=f"g_{tag}")
        nc.sync.dma_start(g_sb[:], g_dram.ap())
        b_sb = cp.tile([P, DC], F32, name=f"b_{tag}")
        nc.sync.dma_start(b_sb[:], b_dram.ap())

        ps_sum = pp.tile([2, ROWS], F32, name=f"pssum_{tag}")
        ps_sq = pp.tile([2, ROWS], F32, name=f"pssq_{tag}")
        for dc in range(DC):
            nc.tensor.matmul(ps_sum[:], ones_col[:], F_sb[:, dc],
                             start=(dc == 0), stop=(dc == DC - 1))
            nc.tensor.matmul(ps_sq[:], ones_col[:], SQ[:, dc],
                             start=(dc == 0), stop=(dc == DC - 1))

        mean = lp.tile([1, ROWS], F32, name=f"mean_{tag}")
        nc.vector.tensor_scalar(mean[:], ps_sum[0:1, :], 1.0 / D, None, ALU.mult)
        ex2 = lp.tile([1, ROWS], F32, name=f"ex2_{tag}")
        nc.vector.tensor_scalar(ex2[:], ps_sq[0:1, :], 1.0 / D, None, ALU.mult)
        var = lp.tile([1, ROWS], F32, name=f"var_{tag}")
        nc.vector.tensor_mul(var[:], mean[:], mean[:])
        nc.vector.tensor_sub(var[:], ex2[:], var[:])
        veps = lp.tile([1, ROWS], F32, name=f"veps_{tag}")
        nc.vector.tensor_scalar(veps[:], var[:], EPS, None, ALU.add)
        s0 = lp.tile([1, ROWS], F32, name=f"s0_{tag}")
        nc.scalar.activation(s0[:], veps[:], AF.Sqrt)
        r0 = lp.tile([1, ROWS], F32, name=f"r0_{tag}")
        nc.vector.reciprocal(r0[:], s0[:])
        s1 = lp.tile([1, ROWS], F32, name=f"s1_{tag}")
        nc.vector.tensor_mul(s1[:], veps[:], r0[:])
        nc.vector.tensor_add(s1[:], s1[:], s0[:])
        nc.vector.tensor_scalar(s1[:], s1[:], 0.5, None, ALU.mult)
        rstd = lp.tile([1, ROWS], F32, name=f"rstd_{tag}")
        nc.vector.reciprocal(rstd[:], s1[:])

        meanB = lp.tile([P, ROWS], F32, name=f"meanB_{tag}")
        nc.gpsimd.partition_broadcast(meanB[:], mean[:])
        rstdB = lp.tile([P, ROWS], F32, name=f"rstdB_{tag}")
        nc.gpsimd.partition_broadcast(rstdB[:], rstd[:])

        for dc in range(DC):
            t1 = lp.tile([P, ROWS], F32, name=f"t1_{tag}", bufs=3)
            nc.vector.tensor_sub(t1[:], F_sb[:, dc], meanB[:])
            nc.vector.tensor_mul(t1[:], t1[:], rstdB[:])
            nc.vector.tensor_scalar(Y_sb[:, dc], t1[:],
                                    g_sb[:, dc:dc + 1], b_sb[:, dc:dc + 1],
                                    ALU.mult, ALU.add)


def _build(p0, sp1, sp2, p3, bias_on):
    """Build + compile the SPMD program.  p0/sp1/sp2/p3 are [H] host floats
    baked into the NEFF as activation immediates; bias_on['h2o'] selects the
    rank-1 vocab-bias matmul (other biases are always applied, free)."""
    p3_zero = bool(np.all(p3 == 0.0))
    n_r = 1 if p3_zero else H

    nc = bacc.Bacc(None, target_bir_lowering=False, debug=False,
                   num_devices=NCORES)

    def inp(name, shape, dtype):
        return nc.dram_tensor(name, shape, dtype, kind="ExternalInput")

    xT = inp("xT", [D, B * L], BF16)
    wvT = inp("wvT", [D, HD], BF16)
    fcT = inp("fcT", [HD, D], BF16)
    w1T = inp("w1T", [D, HID], BF16)
    w2T = inp("w2T", [HID, D], BF16)
    h2oT = inp("h2oT", [D, V], BF16)
    bv2 = inp("bv2", [P, DC], F32)
    fcb2 = inp("fcb2", [P, DC], F32)
    b12 = inp("b12", [P, HC], F32)
    b22 = inp("b22", [P, DC], F32)
    if bias_on["h2o"]:
        h2ob = inp("h2ob", [1, V], BF16)
        onesr = inp("onesr", [1, ROWS], BF16)
    onesc = inp("onesc", [P, 2], F32R)
    onesb = inp("onesb", [P, 2], BF16)
    ln1g = inp("ln1g", [P, DC], F32)
    ln1b = inp("ln1b", [P, DC], F32)
    ln2g = inp("ln2g", [P, DC], F32)
    ln2b = inp("ln2b", [P, DC], F32)
    S_in = inp("S_in", [P, 8, IC], F32)          # |i-j| tiled [jp, jc, i]
    expb = inp("expb", [P, H], F32)              # per-head exp bias ln(2*p0)
    R_in = inp("R_in", [n_r, P, 8, IC], F32)     # p3*(i<j) - BIG*eye, per head
    y = nc.dram_tensor("y", [ROWS, V], F32, kind="ExternalOutput")

    with tile.TileContext(nc) as tc, contextlib.ExitStack() as top:
        c0 = top.enter_context(tc.tile_pool(name="const0", bufs=1))
        wp = top.enter_context(tc.tile_pool(name="h2o_w", bufs=6))
        zp = top.enter_context(tc.tile_pool(name="zmid", bufs=1))

        ones_col = c0.tile([P, 2], F32R, name="ones_col")
        nc.sync.dma_start(ones_col[:], onesc.ap())
        ones_colb = c0.tile([P, 2], BF16, name="ones_colb")
        nc.sync.dma_start(ones_colb[:], onesb.ap())
        Z_sb = zp.tile([P, DC, ROWS], BF16, name="Z_sb")

        with contextlib.ExitStack() as s1:
            OT = s1.enter_context(tc.tile_pool(name="otp", bufs=1)).tile(
                [P, DC, ROWS], BF16, name="OT")
            # fc weights pool created before the A/B scope so pool
            # stack order holds; DMA traced here too (no deps, prefetches)
            fcp0 = s1.enter_context(tc.tile_pool(name="fcc", bufs=1))
            fcT_sb = fcp0.tile([P, DC, D], BF16, name="fcT_sb")
            nc.sync.dma_start(fcT_sb[:],
                              fcT.ap().rearrange("(c p) f -> p c f", p=P))
            fcb_sb = fcp0.tile([P, DC], F32, name="fcb_sb")
            nc.sync.dma_start(fcb_sb[:], fcb2.ap())

            sab = contextlib.ExitStack()
            # ---------------- stage A: v = x @ wv.T ----------------
            vp = sab.enter_context(tc.tile_pool(name="vpool", bufs=1))
            v_sb = vp.tile([P, B * L // P, HD], BF16, name="v_sb")
            with contextlib.ExitStack() as sa:
                rp = sa.enter_context(tc.tile_pool(name="resid", bufs=1))
                pa = sa.enter_context(tc.tile_pool(name="psA", bufs=4,
                                                   space="PSUM"))
                xT_sb = rp.tile([P, DC, B * L], BF16, name="xT_sb")
                xT_t = xT.ap().rearrange("(c p) r -> p c r", p=P)
                wvT_sb = rp.tile([P, DC, HD], BF16, name="wvT_sb")
                wvT_t = wvT.ap().rearrange("(c p) f -> p c f", p=P)
                for dc in range(DC):
                    nc.sync.dma_start(wvT_sb[:, dc], wvT_t[:, dc])
                for rcg in range(4):
                    for dc in range(DC):
                        nc.sync.dma_start(
                            xT_sb[:, dc, rcg * 512:(rcg + 1) * 512],
                            xT_t[:, dc, rcg * 512:(rcg + 1) * 512])
                for rc in range(B * L // P):
                    for nh in range(2):
                        psv = pa.tile([P, 512], F32, name="psv")
                        for dc in range(DC):
                            nc.tensor.matmul(
                                psv[:],
                                xT_sb[:, dc, rc * P:(rc + 1) * P],
                                wvT_sb[:, dc, nh * 512:(nh + 1) * 512],
                                start=(dc == 0), stop=(dc == DC - 1))
                        nc.vector.tensor_copy(
                            v_sb[:, rc, nh * 512:(nh + 1) * 512], psv[:])

            # ---------------- stage B: attention ----------------
            # (bv is added at the transpose eviction: softmax rows sum to 1)
            with sab, contextlib.ExitStack() as sb:
                up = sb.enter_context(tc.tile_pool(name="attn_u", bufs=2))
                sp_ = sb.enter_context(tc.tile_pool(name="attn_s", bufs=3))
                cp = sb.enter_context(tc.tile_pool(name="attn_c", bufs=1))
                ab = sb.enter_context(tc.tile_pool(name="attn_b", bufs=1))
                pp = sb.enter_context(tc.tile_pool(name="attn_p", bufs=2,
                                                   space="PSUM"))

                S_sb = cp.tile([P, 8, IC], F32, name="S_sb")
                nc.sync.dma_start(S_sb[:], S_in.ap())
                eb_sb = cp.tile([P, H], F32, name="eb_sb")
                nc.sync.dma_start(eb_sb[:], expb.ap())
                bv_sb = cp.tile([P, DC], F32, name="bv_sb")
                nc.sync.dma_start(bv_sb[:], bv2.ap())
                ident = cp.tile([P, P], F32, name="ident")
                make_identity(nc, ident[:])
                R_sb = None
                O_sb = ab.tile([P, B, HD], F32, name="O_sb")

                hkeys = [(float(p0[h]), float(sp1[h]), float(sp2[h]),
                          float(p3[h])) for h in range(H)]
                n_groups = len(set(hkeys))
                gup = sb.enter_context(
                    tc.tile_pool(name="attn_gu", bufs=min(n_groups + 1, H)))
                grp = {}
                for h in range(H):
                    if hkeys[h] in grp:
                        u_sb, rs = grp[hkeys[h]]
                    else:
                        if R_sb is None or n_r > 1:
                            R_sb = cp.tile([P, 8, IC], F32, name="R_sb",
                                           bufs=2)
                            nc.sync.dma_start(R_sb[:],
                                              R_in.ap()[min(h, n_r - 1)])
                        t_sb = up.tile([P, 8, IC], F32, name="t_sb")
                        if p0[h] > 0.0 and abs(sp1[h] - sp2[h]) < 1e-12:
                            nc.scalar.activation(t_sb[:], S_sb[:], AF.Exp,
                                                 scale=-sp1[h],
                                                 bias=eb_sb[:, h:h + 1])
                        elif p0[h] > 0.0:
                            e2 = up.tile([P, 8, IC], F32, name="e2_sb")
                            nc.scalar.activation(t_sb[:], S_sb[:], AF.Exp,
                                                 scale=-sp1[h],
                                                 bias=eb_sb[:, h:h + 1])
                            nc.scalar.activation(e2[:], S_sb[:], AF.Exp,
                                                 scale=-sp2[h],
                                                 bias=eb_sb[:, h:h + 1])
                            nc.vector.tensor_add(t_sb[:], t_sb[:], e2[:])
                        elif p0[h] == 0.0:
                            nc.any.memset(t_sb[:], 0.0)
                        else:
                            e2 = up.tile([P, 8, IC], F32, name="e2_sb")
                            nc.scalar.activation(t_sb[:], S_sb[:], AF.Exp,
                                                 scale=-sp1[h])
                            nc.scalar.activation(e2[:], S_sb[:], AF.Exp,
                                                 scale=-sp2[h])
                            nc.vector.tensor_add(t_sb[:], t_sb[:], e2[:])
                            nc.vector.tensor_scalar(t_sb[:], t_sb[:], p0[h],
                                                    None, ALU.mult)
                        nc.vector.tensor_add(t_sb[:], t_sb[:], R_sb[:])
                        u_sb = gup.tile([P, 8, IC], BF16, name="u_sb")
                        nc.scalar.activation(u_sb[:], t_sb[:], AF.Exp)
                        ps_s = pp.tile([P, 2], F32, name="ps_s")
                        for jc in range(8):
                            nc.tensor.matmul(ps_s[:], u_sb[:, jc],
                                             ones_colb[:],
                                             start=(jc == 0), stop=(jc == 7))
                        rs = sp_.tile([P, 1], F32, name="rs_t",
                                      bufs=min(n_groups + 1, H))
                        nc.vector.reciprocal(rs[:], ps_s[:, 0:1])
                        grp[hkeys[h]] = (u_sb, rs)

                    ps_o = [pp.tile([P, DV], F32, name=f"ps_o{b}")
                            for b in range(B)]
                    for jc in range(8):
                        lhsT = u_sb[:, jc]
                        for b in range(B):
                            nc.tensor.matmul(
                                ps_o[b][:], lhsT,
                                v_sb[:, b * 8 + jc, h * DV:(h + 1) * DV],
                                start=(jc == 0), stop=(jc == 7))
                    for b in range(B):
                        nc.vector.tensor_scalar(
                            O_sb[:, b, h * DV:(h + 1) * DV],
                            ps_o[b][:], rs[:], None, ALU.mult)

                # transpose row-major [i, hd] -> feature-major [hd, (b,i)],
                # adding bv (exact: softmax rows sum to 1)
                for b in range(B):
                    for hc in range(DC):
                        pt = pp.tile([P, P], F32, name="pt")
                        nc.tensor.transpose(
                            pt[:], O_sb[:, b, hc * P:(hc + 1) * P], ident[:])
                        nc.vector.tensor_scalar(
                            OT[:, hc, b * IC:(b + 1) * IC], pt[:],
                            bv_sb[:, hc:hc + 1], None, ALU.add)

            # h2o weight stream (traced here so it doesn't outprioritize
            # the stage-A input loads; still prefetches during fc/FFN)
            h2oT_t = h2oT.ap().rearrange("(c p) v -> p c v", p=P)
            W_sbs = []
            for vi, (vs, vsz) in enumerate(VTILES):
                W_sb = wp.tile([P, DC, 512], BF16, name="W_sb")
                nc.sync.dma_start(W_sb[:, :, :vsz], h2oT_t[:, :, vs:vs + vsz])
                W_sbs.append(W_sb)

            # ---------------- stage C: fc + LN1 ----------------
            yp = s1.enter_context(tc.tile_pool(name="ypool", bufs=1))
            Y_sb = yp.tile([P, DC, ROWS], BF16, name="Y_sb")
            with contextlib.ExitStack() as sc:
                fp = sc.enter_context(tc.tile_pool(name="fcp", bufs=1))
                pc = sc.enter_context(tc.tile_pool(name="psC", bufs=4,
                                                   space="PSUM"))
                F1 = fp.tile([P, DC, ROWS], F32R, name="F1")
                for do in range(DC):
                    psf = pc.tile([P, ROWS], F32, name="psf")
                    for hc in range(DC):
                        nc.tensor.matmul(
                            psf[:],
                            fcT_sb[:, hc, do * P:(do + 1) * P],
                            OT[:, hc, :],
                            start=(hc == 0), stop=(hc == DC - 1))
                    nc.vector.tensor_scalar(F1[:, do], psf[:],
                                            fcb_sb[:, do:do + 1], None,
                                            ALU.add)
                _layernorm_sb(nc, tc, F1, ln1g, ln1b, Y_sb, ones_col, "ln1")

            # ---------------- stage D: FFN ----------------
            with contextlib.ExitStack() as sd:
                hp = sd.enter_context(tc.tile_pool(name="hpool", bufs=1))
                wsp = sd.enter_context(tc.tile_pool(name="wstr", bufs=2))
                w2p = sd.enter_context(tc.tile_pool(name="w2str", bufs=5))
                cd = sd.enter_context(tc.tile_pool(name="cD", bufs=1))
                pd = sd.enter_context(tc.tile_pool(name="psD", bufs=2,
                                                   space="PSUM"))
                H_sb = hp.tile([P, HC, ROWS], BF16, name="H_sb")
                b1_sb = cd.tile([P, HC], F32, name="b1_sb")
                nc.sync.dma_start(b1_sb[:], b12.ap())
                b2_sb = cd.tile([P, DC], F32, name="b2_sb")
                nc.sync.dma_start(b2_sb[:], b22.ap())

                w1T_t = w1T.ap().rearrange("(c p) m -> p c m", p=P)
                for hs in range(8):           # 512-wide hid slices
                    W1t = wsp.tile([P, DC, 512], BF16, name="W1t")
                    nc.sync.dma_start(W1t[:],
                                      w1T_t[:, :, hs * 512:(hs + 1) * 512])
                    for m2 in range(4):       # 128-wide subchunks
                        psh = pd.tile([P, ROWS], F32, name="psh")
                        for dc in range(DC):
                            nc.tensor.matmul(
                                psh[:],
                                W1t[:, dc, m2 * P:(m2 + 1) * P],
                                Y_sb[:, dc, :],
                                start=(dc == 0), stop=(dc == DC - 1))
                        hcix = hs * 4 + m2
                        nc.scalar.activation(H_sb[:, hcix], psh[:], AF.Relu,
                                             bias=b1_sb[:, hcix:hcix + 1])

                FF = hp.tile([P, DC, ROWS], F32R, name="FF")
                w2T_t = w2T.ap().rearrange("(c p) m -> p c m", p=P)
                for ds2 in range(2):          # 512-wide d slices
                    W2ts = []
                    for g in range(4):
                        W2t = w2p.tile([P, 8, 512], BF16, name="W2t")
                        nc.sync.dma_start(
                            W2t[:],
                            w2T_t[:, g * 8:(g + 1) * 8,
                                  ds2 * 512:(ds2 + 1) * 512])
                        W2ts.append(W2t)
                    for m2 in range(4):
                        do = ds2 * 4 + m2
                        psw = pd.tile([P, ROWS], F32, name="psw")
                        for hc in range(HC):
                            nc.tensor.matmul(
                                psw[:],
                                W2ts[hc // 8][:, hc % 8, m2 * P:(m2 + 1) * P],
                                H_sb[:, hc, :],
                                start=(hc == 0), stop=(hc == HC - 1))
                        nc.vector.tensor_scalar(psw[:], psw[:],
                                                b2_sb[:, do:do + 1], None,
                                                ALU.add)
                        nc.vector.tensor_add(FF[:, do], psw[:], Y_sb[:, do])
                _layernorm_sb(nc, tc, FF, ln2g, ln2b, Z_sb, ones_col, "ln2")

        # ---------------- stage E: h2o + log_softmax ----------------
        with contextlib.ExitStack() as se:
            ep = se.enter_context(tc.tile_pool(name="h2o_e", bufs=3))
            op_ = se.enter_context(tc.tile_pool(name="h2o_o", bufs=4))
            lp_ = se.enter_context(tc.tile_pool(name="h2o_l", bufs=1))
            pp = se.enter_context(tc.tile_pool(name="h2o_p", bufs=4,
                                               space="PSUM"))

            L16 = lp_.tile([P, B, V], F16, name="L16")          # 16 MB
            parts = lp_.tile([P, B, len(VTILES)], F32, name="parts")
            if bias_on["h2o"]:
                ones_row = lp_.tile([1, ROWS], BF16, name="ones_row_z")
                nc.sync.dma_start(ones_row[:], onesr.ap())

            for vi, (vs, vsz) in enumerate(VTILES):
                W_sb = W_sbs[vi]
                if bias_on["h2o"]:
                    bias_sb = ep.tile([1, 512], BF16, name="bias_sb")
                    nc.sync.dma_start(bias_sb[:, :vsz],
                                      h2ob.ap()[:, vs:vs + vsz])
                for rt in range(B):
                    ps = pp.tile([P, 512], F32, name="ps_l")
                    for dc in range(DC):
                        nc.tensor.matmul(
                            ps[:, :vsz],
                            Z_sb[:, dc, rt * IC:(rt + 1) * IC],
                            W_sb[:, dc, :vsz],
                            start=(dc == 0),
                            stop=(dc == DC - 1 and not bias_on["h2o"]))
                    if bias_on["h2o"]:
                        nc.tensor.matmul(
                            ps[:, :vsz],
                            ones_row[:, rt * IC:(rt + 1) * IC],
                            bias_sb[:, :vsz],
                            start=False, stop=True)
                    nc.vector.tensor_copy(L16[:, rt, vs:vs + vsz],
                                          ps[:, :vsz])
                    esc = ep.tile([P, 512], F32, name="esc", bufs=2)
                    nc.scalar.activation(
                        esc[:, :vsz], ps[:, :vsz], AF.Exp,
                        accum_out=parts[:, rt, vi:vi + 1])

            for rt in range(B):
                s_t = ep.tile([P, 1], F32, name="s_t")
                nc.vector.reduce_sum(s_t[:], parts[:, rt, :], axis=AX.X)
                lse = ep.tile([P, 1], F32, name="lse_t")
                nc.scalar.activation(lse[:], s_t[:], AF.Ln)
                for ti, (vs, vsz) in enumerate(VTILES):
                    ot = op_.tile([P, 512], F32, name="ot")
                    eng = nc.vector if ti % 2 == 0 else nc.gpsimd
                    eng.tensor_scalar(ot[:, :vsz], L16[:, rt, vs:vs + vsz],
                                      lse[:], None, ALU.subtract)
                    nc.sync.dma_start(
                        y.ap()[rt * IC:(rt + 1) * IC, vs:vs + vsz],
                        ot[:, :vsz])

    nc.compile()
    return nc


_CACHE = {}


def _ppart(vec, chunks):
    """[chunks*P] -> [P, chunks] per-partition layout."""
    return np.ascontiguousarray(vec.reshape(chunks, P).T, np.float32)


def kernel(**inputs):
    f32 = np.float32
    bf16 = ml_dtypes.bfloat16
    x = np.asarray(inputs["x"], f32)
    wv = np.asarray(inputs["wv"], f32)
    bv = np.asarray(inputs["bv"], f32)
    fc_w = np.asarray(inputs["fc_w"], f32)
    fc_b = np.asarray(inputs["fc_b"], f32)
    ln1_g = np.asarray(inputs["ln1_g"], f32)
    ln1_b = np.asarray(inputs["ln1_b"], f32)
    w1 = np.asarray(inputs["w1"], f32)
    b1 = np.asarray(inputs["b1"], f32)
    w2 = np.asarray(inputs["w2"], f32)
    b2 = np.asarray(inputs["b2"], f32)
    ln2_g = np.asarray(inputs["ln2_g"], f32)
    ln2_b = np.asarray(inputs["ln2_b"], f32)
    h2o_w = np.asarray(inputs["h2o_w"], f32)
    h2o_b = np.asarray(inputs["h2o_b"], f32)
    p0 = np.asarray(inputs["p0"], np.float64)
    p1 = np.asarray(inputs["p1"], np.float64)
    p2 = np.asarray(inputs["p2"], np.float64)
    p3 = np.asarray(inputs["p3"], np.float64)
    # wk/bk deliberately unused: constant along the softmax axis.

    sp1 = np.float32(_softplus(p1)).astype(np.float64)
    sp2 = np.float32(_softplus(p2)).astype(np.float64)

    bias_on = {"h2o": bool(np.any(h2o_b))}

    key = (p0.tobytes(), sp1.tobytes(), sp2.tobytes(), p3.tobytes(),
           bias_on["h2o"])
    if key not in _CACHE:
        _CACHE[key] = _build(p0, sp1, sp2, p3, bias_on)
    nc = _CACHE[key]

    x2T = np.ascontiguousarray(x.reshape(B * L, D).T)
    shared = {
        "xT": np.ascontiguousarray(x2T.astype(bf16)),
        "wvT": np.ascontiguousarray(wv.T.astype(bf16)),
        "fcT": np.ascontiguousarray(fc_w.T.astype(bf16)),
        "w1T": np.ascontiguousarray(w1.T.astype(bf16)),
        "w2T": np.ascontiguousarray(w2.T.astype(bf16)),
        "h2oT": np.ascontiguousarray(h2o_w.T.astype(bf16)),
        "bv2": _ppart(bv, DC),
        "fcb2": _ppart(fc_b, DC),
        "b12": _ppart(b1, HC),
        "b22": _ppart(b2, DC),
        "onesc": np.ones((P, 2), f32),
        "onesb": np.ones((P, 2), bf16),
        "ln1g": _ppart(ln1_g, DC),
        "ln1b": _ppart(ln1_b, DC),
        "ln2g": _ppart(ln2_g, DC),
        "ln2b": _ppart(ln2_b, DC),
    }
    if bias_on["h2o"]:
        shared["h2ob"] = np.ascontiguousarray(h2o_b[None].astype(bf16))
        shared["onesr"] = np.ones((1, ROWS), bf16)

    p3_zero = bool(np.all(p3 == 0.0))
    ebv = np.zeros(H, np.float64)
    for h in range(H):
        if p0[h] > 0.0 and abs(sp1[h] - sp2[h]) < 1e-12:
            ebv[h] = math.log(2.0 * p0[h])
        elif p0[h] > 0.0:
            ebv[h] = math.log(p0[h])
    expb_host = np.ascontiguousarray(
        np.broadcast_to(ebv.astype(f32)[None, :], (P, H)))

    j = np.arange(L)
    in_maps = []
    for c in range(NCORES):
        i_idx = c * IC + np.arange(IC)
        Sji = np.abs(j[:, None] - i_idx[None, :]).astype(f32)       # [L, IC]
        eye = (Sji == 0).astype(f32)
        if p3_zero:
            Rs = [NEG_BIG * eye]
        else:
            Aji = (i_idx[None, :] < j[:, None]).astype(f32)
            Rs = [np.float32(p3[h]) * Aji + NEG_BIG * eye for h in range(H)]

        def tile_ji(a):  # [L, IC] -> [jp, jc, IC]
            return np.ascontiguousarray(
                a.reshape(8, P, IC).transpose(1, 0, 2), f32)

        m = dict(shared)
        m["S_in"] = tile_ji(Sji)
        m["expb"] = expb_host
        m["R_in"] = np.stack([tile_ji(R) for R in Rs], axis=0)
        in_maps.append(m)

    res = run_bass_kernel_spmd(nc, in_maps, core_ids=list(range(NCORES)))

    out = np.empty((B, L, V), f32)
    for c in range(NCORES):
        yc = res.results[c]["y"]
        for b in range(B):
            out[b, c * IC:(c + 1) * IC, :] = yc[b * IC:(b + 1) * IC, :]
    return out


# revision 21
# speedup vs baseline: 1.4690x; 1.0041x over previous
"""Trainium2 Bass kernel for a single-layer "BiTRF" dense transformer block.

Math (see reference):
  posi[h,i,j] = p0*(exp(-sp1*|i-j|) + exp(-sp2*|i-j|)) + p3*(i<j)   (sp=softplus(p))
  attn[h,b,i,j] = kproj[b,i,h] + posi[h,i,j], diag masked, softmax over j.
  Because kproj[b,i,h] is constant along the softmax axis j, softmax is
  invariant to it, so the wk/bk projection drops out entirely and the
  attention weights W[h,i,:] are shared across the batch (and across heads
  with identical (p0, sp1, sp2, p3) — computed once per unique group).
  out  = LN1(attnout @ fc_w.T + fc_b)
  out2 = LN2(relu(out @ w1.T + b1) @ w2.T + b2 + out)
  y    = log_softmax(out2 @ h2o_w.T + h2o_b)

Sharding: 8 cores, core c owns query rows i in [c*128,(c+1)*128) for BOTH
batches (256 row-instances).  v = x@wv.T is computed redundantly on every
core (avoids any collective); everything else is row-sharded, h2o is
row-sharded too (each core computes its rows x full 32000 vocab, so
log_softmax is fully local).

The whole pre-h2o chain (v-proj, attention, fc, LN1, FFN, LN2) lives in
SBUF — no DRAM round-trips between stages.  Activations are feature-major
[feat, row] (LN partition reductions via ones-column matmuls); attention
output is transposed back with PE-transposes.  Biases are applied as
per-partition scalars at PSUM-eviction time (bv rides through the
attention because softmax rows sum to 1, so it is added at the transpose
eviction).  The h2o weight stream pool lives at top scope so its first
tiles prefetch during earlier phases.

dtypes: matmuls run bf16 (weights pre-cast on host, fp32 PSUM
accumulation); LayerNorm statistics and log-sum-exp run in fp32; raw
logits are staged in fp16 for the final lse subtraction.
"""

import contextlib
import math

import ml_dtypes
import numpy as np

import concourse.mybir as mybir
import concourse.tile as tile
from concourse import bacc
from concourse.bass_utils import run_bass_kernel_spmd
from concourse.masks import make_identity

B, L, D, H, DV, HID, V = 2, 1024, 1024, 16, 64, 4096, 32000
NCORES = 8
IC = L // NCORES        # 128 query rows per core
ROWS = B * IC           # 256 row-instances per core
HD = H * DV             # 1024
P = 128
DC = D // P             # 8 feature chunks
HC = HID // P           # 32 hidden chunks
EPS = 1e-5
NEG_BIG = -1.0e9

F32 = mybir.dt.float32
F32R = mybir.dt.float32r
BF16 = mybir.dt.bfloat16
F16 = mybir.dt.float16
AF = mybir.ActivationFunctionType
ALU = mybir.AluOpType
AX = mybir.AxisListType

# h2o vocab tiling: 62 tiles of 512 + 1 tile of 256
VTILES = [(i * 512, 512) for i in range(62)] + [(62 * 512, 256)]


def _r(ap):
    return ap.bitcast(F32R)


def _softplus(x):
    return np.logaddexp(0.0, x.astype(np.float64))


def _layernorm_sb(nc, tc, F_sb, g_dram, b_dram, Y_sb, ones_col, tag):
    """LN over the feature (partition) axis, fully in SBUF.
    F_sb: [P, DC, ROWS] f32r source; Y_sb: [P, DC, ROWS] dst (any dtype)."""
    with contextlib.ExitStack() as ctx:
        lp = ctx.enter_context(tc.tile_pool(name=f"ln_{tag}", bufs=2))
        cp = ctx.enter_context(tc.tile_pool(name=f"lnc_{tag}", bufs=1))
        pp = ctx.enter_context(tc.tile_pool(name=f"lnp_{tag}", bufs=2, space="PSUM"))

        SQ = lp.tile([P, DC, ROWS], F32R, name=f"SQ_{tag}")
        nc.vector.tensor_mul(SQ[:], F_sb[:], F_sb[:])

        g_sb = cp.tile([P, DC], F32, name=f"g_{tag}")
        nc.sync.dma_start(g_sb[:], g_dram.ap())
        b_sb = cp.tile([P, DC], F32, name=f"b_{tag}")
        nc.sync.dma_start(b_sb[:], b_dram.ap())

        ps_sum = pp.tile([2, ROWS], F32, name=f"pssum_{tag}")
        ps_sq = pp.tile([2, ROWS], F32, name=f"pssq_{tag}")
        for dc in range(DC):
            nc.tensor.matmul(ps_sum[:], ones_col[:], F_sb[:, dc],
                             start=(dc == 0), stop=(dc == DC - 1))
            nc.tensor.matmul(ps_sq[:], ones_col[:], SQ[:, dc],
                             start=(dc == 0), stop=(dc == DC - 1))

        mean = lp.tile([1, ROWS], F32, name=f"mean_{tag}")
        nc.vector.tensor_scalar(mean[:], ps_sum[0:1, :], 1.0 / D, None, ALU.mult)
        ex2 = lp.tile([1, ROWS], F32, name=f"ex2_{tag}")
        nc.vector.tensor_scalar(ex2[:], ps_sq[0:1, :], 1.0 / D, None, ALU.mult)
        var = lp.tile([1, ROWS], F32, name=f"var_{tag}")
        nc.vector.tensor_mul(var[:], mean[:], mean[:])
        nc.vector.tensor_sub(var[:], ex2[:], var[:])
        veps = lp.tile([1, ROWS], F32, name=f"veps_{tag}")
        nc.vector.tensor_scalar(veps[:], var[:], EPS, None, ALU.add)
        s0 = lp.tile([1, ROWS], F32, name=f"s0_{tag}")
        nc.scalar.activation(s0[:], veps[:], AF.Sqrt)
        r0 = lp.tile([1, ROWS], F32, name=f"r0_{tag}")
        nc.vector.reciprocal(r0[:], s0[:])
        s1 = lp.tile([1, ROWS], F32, name=f"s1_{tag}")
        nc.vector.tensor_mul(s1[:], veps[:], r0[:])
        nc.vector.tensor_add(s1[:], s1[:], s0[:])
        nc.vector.tensor_scalar(s1[:], s1[:], 0.5, None, ALU.mult)
        rstd = lp.tile([1, ROWS], F32, name=f"rstd_{tag}")
        nc.vector.reciprocal(rstd[:], s1[:])

        meanB = lp.tile([P, ROWS], F32, name=f"meanB_{tag}")
        nc.gpsimd.partition_broadcast(meanB[:], mean[:])
        rstdB = lp.tile([P, ROWS], F32, name=f"rstdB_{tag}")
        nc.gpsimd.partition_broadcast(rstdB[:], rstd[:])

        for dc in range(DC):
            t1 = lp.tile([P, ROWS], F32, name=f"t1_{tag}", bufs=3)
            nc.vector.tensor_sub(t1[:], F_sb[:, dc], meanB[:])
            nc.vector.tensor_mul(t1[:], t1[:], rstdB[:])
            nc.vector.tensor_scalar(Y_sb[:, dc], t1[:],
                                    g_sb[:, dc:dc + 1], b_sb[:, dc:dc + 1],
                                    ALU.mult, ALU.add)


def _build(p0, sp1, sp2, p3, bias_on):
    """Build + compile the SPMD program.  p0/sp1/sp2/p3 are [H] host floats
    baked into the NEFF as activation immediates; bias_on['h2o'] selects the
    rank-1 vocab-bias matmul (other biases are always applied, free)."""
    p3_zero = bool(np.all(p3 == 0.0))
    n_r = 1 if p3_zero else H

    nc = bacc.Bacc(None, target_bir_lowering=False, debug=False,
                   num_devices=NCORES)

    def inp(name, shape, dtype):
        return nc.dram_tensor(name, shape, dtype, kind="ExternalInput")

    xT = inp("xT", [D, B * L], BF16)
    wvT = inp("wvT", [D, HD], BF16)
    fcT = inp("fcT", [HD, D], BF16)
    w1T = inp("w1T", [D, HID], BF16)
    w2T = inp("w2T", [HID, D], BF16)
    h2oT = inp("h2oT", [D, V], BF16)
    bv2 = inp("bv2", [P, DC], F32)
    fcb2 = inp("fcb2", [P, DC], F32)
    b12 = inp("b12", [P, HC], F32)
    b22 = inp("b22", [P, DC], F32)
    if bias_on["h2o"]:
        h2ob = inp("h2ob", [1, V], BF16)
        onesr = inp("onesr", [1, ROWS], BF16)
    onesc = inp("onesc", [P, 2], F32R)
    onesb = inp("onesb", [P, 2], BF16)
    ln1g = inp("ln1g", [P, DC], F32)
    ln1b = inp("ln1b", [P, DC], F32)
    ln2g = inp("ln2g", [P, DC], F32)
    ln2b = inp("ln2b", [P, DC], F32)
    S_in = inp("S_in", [P, 8, IC], F32)          # |i-j| tiled [jp, jc, i]
    expb = inp("expb", [P, H], F32)              # per-head exp bias ln(2*p0)
    R_in = inp("R_in", [n_r, P, 8, IC], F32)     # p3*(i<j) - BIG*eye, per head
    y = nc.dram_tensor("y", [ROWS, V], F32, kind="ExternalOutput")

    with tile.TileContext(nc) as tc, contextlib.ExitStack() as top:
        c0 = top.enter_context(tc.tile_pool(name="const0", bufs=1))
        wp = top.enter_context(tc.tile_pool(name="h2o_w", bufs=6))
        zp = top.enter_context(tc.tile_pool(name="zmid", bufs=1))

        ones_col = c0.tile([P, 2], F32R, name="ones_col")
        nc.sync.dma_start(ones_col[:], onesc.ap())
        ones_colb = c0.tile([P, 2], BF16, name="ones_colb")
        nc.sync.dma_start(ones_colb[:], onesb.ap())
        Z_sb = zp.tile([P, DC, ROWS], BF16, name="Z_sb")

        with contextlib.ExitStack() as s1:
            OT = s1.enter_context(tc.tile_pool(name="otp", bufs=1)).tile(
                [P, DC, ROWS], BF16, name="OT")
            # fc weights pool created before the A/B scope so pool
            # stack order holds; DMA traced here too (no deps, prefetches)
            fcp0 = s1.enter_context(tc.tile_pool(name="fcc", bufs=1))
            fcT_sb = fcp0.tile([P, DC, D], BF16, name="fcT_sb")
            nc.sync.dma_start(fcT_sb[:],
                              fcT.ap().rearrange("(c p) f -> p c f", p=P))
            fcb_sb = fcp0.tile([P, DC], F32, name="fcb_sb")
            nc.sync.dma_start(fcb_sb[:], fcb2.ap())

            sab = contextlib.ExitStack()
            # ---------------- stage A: v = x @ wv.T ----------------
            vp = sab.enter_context(tc.tile_pool(name="vpool", bufs=1))
            v_sb = vp.tile([P, B * L // P, HD], BF16, name="v_sb")
            with contextlib.ExitStack() as sa:
                rp = sa.enter_context(tc.tile_pool(name="resid", bufs=1))
                pa = sa.enter_context(tc.tile_pool(name="psA", bufs=4,
                                                   space="PSUM"))
                xT_sb = rp.tile([P, DC, B * L], BF16, name="xT_sb")
                xT_t = xT.ap().rearrange("(c p) r -> p c r", p=P)
                wvT_sb = rp.tile([P, DC, HD], BF16, name="wvT_sb")
                wvT_t = wvT.ap().rearrange("(c p) f -> p c f", p=P)
                for dc in range(DC):
                    nc.sync.dma_start(wvT_sb[:, dc], wvT_t[:, dc])
                for rcg in range(4):
                    for dc in range(DC):
                        nc.sync.dma_start(
                            xT_sb[:, dc, rcg * 512:(rcg + 1) * 512],
                            xT_t[:, dc, rcg * 512:(rcg + 1) * 512])
                for rc in range(B * L // P):
                    for nh in range(2):
                        psv = pa.tile([P, 512], F32, name="psv")
                        for dc in range(DC):
                            nc.tensor.matmul(
                                psv[:],
                                xT_sb[:, dc, rc * P:(rc + 1) * P],
                                wvT_sb[:, dc, nh * 512:(nh + 1) * 512],
                                start=(dc == 0), stop=(dc == DC - 1))
                        nc.vector.tensor_copy(
                            v_sb[:, rc, nh * 512:(nh + 1) * 512], psv[:])

            # ---------------- stage B: attention ----------------
            # (bv is added at the transpose eviction: softmax rows sum to 1)
            with sab, contextlib.ExitStack() as sb:
                up = sb.enter_context(tc.tile_pool(name="attn_u", bufs=2))
                sp_ = sb.enter_context(tc.tile_pool(name="attn_s", bufs=3))
                cp = sb.enter_context(tc.tile_pool(name="attn_c", bufs=1))
                ab = sb.enter_context(tc.tile_pool(name="attn_b", bufs=1))
                pp = sb.enter_context(tc.tile_pool(name="attn_p", bufs=2,
                                                   space="PSUM"))

                S_sb = cp.tile([P, 8, IC], F32, name="S_sb")
                nc.sync.dma_start(S_sb[:], S_in.ap())
                eb_sb = cp.tile([P, H], F32, name="eb_sb")
                nc.sync.dma_start(eb_sb[:], expb.ap())
                bv_sb = cp.tile([P, DC], F32, name="bv_sb")
                nc.sync.dma_start(bv_sb[:], bv2.ap())
                ident = cp.tile([P, P], F32, name="ident")
                make_identity(nc, ident[:])
                R_sb = None
                O_sb = ab.tile([P, B, HD], F32, name="O_sb")

                hkeys = [(float(p0[h]), float(sp1[h]), float(sp2[h]),
                          float(p3[h])) for h in range(H)]
                n_groups = len(set(hkeys))
                gup = sb.enter_context(
                    tc.tile_pool(name="attn_gu", bufs=min(n_groups + 1, H)))
                grp = {}
                for h in range(H):
                    if hkeys[h] in grp:
                        u_sb, rs = grp[hkeys[h]]
                    else:
                        if R_sb is None or n_r > 1:
                            R_sb = cp.tile([P, 8, IC], F32, name="R_sb",
                                           bufs=2)
                            nc.sync.dma_start(R_sb[:],
                                              R_in.ap()[min(h, n_r - 1)])
                        t_sb = up.tile([P, 8, IC], F32, name="t_sb")
                        if p0[h] > 0.0 and abs(sp1[h] - sp2[h]) < 1e-12:
                            nc.scalar.activation(t_sb[:], S_sb[:], AF.Exp,
                                                 scale=-sp1[h],
                                                 bias=eb_sb[:, h:h + 1])
                        elif p0[h] > 0.0:
                            e2 = up.tile([P, 8, IC], F32, name="e2_sb")
                            nc.scalar.activation(t_sb[:], S_sb[:], AF.Exp,
                                                 scale=-sp1[h],
                                                 bias=eb_sb[:, h:h + 1])
                            nc.scalar.activation(e2[:], S_sb[:], AF.Exp,
                                                 scale=-sp2[h],
                                                 bias=eb_sb[:, h:h + 1])
                            nc.vector.tensor_add(t_sb[:], t_sb[:], e2[:])
                        elif p0[h] == 0.0:
                            nc.any.memset(t_sb[:], 0.0)
                        else:
                            e2 = up.tile([P, 8, IC], F32, name="e2_sb")
                            nc.scalar.activation(t_sb[:], S_sb[:], AF.Exp,
                                                 scale=-sp1[h])
                            nc.scalar.activation(e2[:], S_sb[:], AF.Exp,
                                                 scale=-sp2[h])
                            nc.vector.tensor_add(t_sb[:], t_sb[:], e2[:])
                            nc.vector.tensor_scalar(t_sb[:], t_sb[:], p0[h],
                                                    None, ALU.mult)
                        nc.vector.tensor_add(t_sb[:], t_sb[:], R_sb[:])
                        u_sb = gup.tile([P, 8, IC], BF16, name="u_sb")
                        nc.scalar.activation(u_sb[:], t_sb[:], AF.Exp)
                        ps_s = pp.tile([P, 2], F32, name="ps_s")
                        for jc in range(8):
                            nc.tensor.matmul(ps_s[:], u_sb[:, jc],
                                             ones_colb[:],
                                             start=(jc == 0), stop=(jc == 7))
                        rs = sp_.tile([P, 1], F32, name="rs_t",
                                      bufs=min(n_groups + 1, H))
                        nc.vector.reciprocal(rs[:], ps_s[:, 0:1])
                        grp[hkeys[h]] = (u_sb, rs)

                    ps_o = [pp.tile([P, DV], F32, name=f"ps_o{b}")
                            for b in range(B)]
                    for jc in range(8):
                        lhsT = u_sb[:, jc]
                        for b in range(B):
                            nc.tensor.matmul(
                                ps_o[b][:], lhsT,
                                v_sb[:, b * 8 + jc, h * DV:(h + 1) * DV],
                                start=(jc == 0), stop=(jc == 7))
                    for b in range(B):
                        nc.vector.tensor_scalar(
                            O_sb[:, b, h * DV:(h + 1) * DV],
                            ps_o[b][:], rs[:], None, ALU.mult)

                    # once both heads of a 128-col chunk are done,
                    # transpose it to feature-major (adding bv; exact since
                    # softmax rows sum to 1)
                    if h % 2 == 1:
                        hc = h // 2
                        for b in range(B):
                            pt = pp.tile([P, P], F32, name="pt")
                            nc.tensor.transpose(
                                pt[:], O_sb[:, b, hc * P:(hc + 1) * P],
                                ident[:])
                            nc.vector.tensor_scalar(
                                OT[:, hc, b * IC:(b + 1) * IC], pt[:],
                                bv_sb[:, hc:hc + 1], None, ALU.add)


            # h2o weight stream (traced here so it doesn't outprioritize
            # the stage-A input loads; still prefetches during fc/FFN)
            h2oT_t = h2oT.ap().rearrange("(c p) v -> p c v", p=P)
            W_sbs = []
            for vi, (vs, vsz) in enumerate(VTILES):
                W_sb = wp.tile([P, DC, 512], BF16, name="W_sb")
                nc.sync.dma_start(W_sb[:, :, :vsz], h2oT_t[:, :, vs:vs + vsz])
                W_sbs.append(W_sb)

            # ---------------- stage C: fc + LN1 ----------------
            yp = s1.enter_context(tc.tile_pool(name="ypool", bufs=1))
            Y_sb = yp.tile([P, DC, ROWS], BF16, name="Y_sb")
            with contextlib.ExitStack() as sc:
                fp = sc.enter_context(tc.tile_pool(name="fcp", bufs=1))
                pc = sc.enter_context(tc.tile_pool(name="psC", bufs=4,
                                                   space="PSUM"))
                F1 = fp.tile([P, DC, ROWS], F32R, name="F1")
                for do in range(DC):
                    psf = pc.tile([P, ROWS], F32, name="psf")
                    for hc in range(DC):
                        nc.tensor.matmul(
                            psf[:],
                            fcT_sb[:, hc, do * P:(do + 1) * P],
                            OT[:, hc, :],
                            start=(hc == 0), stop=(hc == DC - 1))
                    nc.vector.tensor_scalar(F1[:, do], psf[:],
                                            fcb_sb[:, do:do + 1], None,
                                            ALU.add)
                _layernorm_sb(nc, tc, F1, ln1g, ln1b, Y_sb, ones_col, "ln1")

            # ---------------- stage D: FFN ----------------
            with contextlib.ExitStack() as sd:
                hp = sd.enter_context(tc.tile_pool(name="hpool", bufs=1))
                wsp = sd.enter_context(tc.tile_pool(name="wstr", bufs=2))
                w2p = sd.enter_context(tc.tile_pool(name="w2str", bufs=5))
                cd = sd.enter_context(tc.tile_pool(name="cD", bufs=1))
                pd = sd.enter_context(tc.tile_pool(name="psD", bufs=2,
                                                   space="PSUM"))
                H_sb = hp.tile([P, HC, ROWS], BF16, name="H_sb")
                b1_sb = cd.tile([P, HC], F32, name="b1_sb")
                nc.sync.dma_start(b1_sb[:], b12.ap())
                b2_sb = cd.tile([P, DC], F32, name="b2_sb")
                nc.sync.dma_start(b2_sb[:], b22.ap())

                w1T_t = w1T.ap().rearrange("(c p) m -> p c m", p=P)
                for hs in range(8):           # 512-wide hid slices
                    W1t = wsp.tile([P, DC, 512], BF16, name="W1t")
                    nc.sync.dma_start(W1t[:],
                                      w1T_t[:, :, hs * 512:(hs + 1) * 512])
                    for m2 in range(4):       # 128-wide subchunks
                        psh = pd.tile([P, ROWS], F32, name="psh")
                        for dc in range(DC):
                            nc.tensor.matmul(
                                psh[:],
                                W1t[:, dc, m2 * P:(m2 + 1) * P],
                                Y_sb[:, dc, :],
                                start=(dc == 0), stop=(dc == DC - 1))
                        hcix = hs * 4 + m2
                        nc.scalar.activation(H_sb[:, hcix], psh[:], AF.Relu,
                                             bias=b1_sb[:, hcix:hcix + 1])

                FF = hp.tile([P, DC, ROWS], F32R, name="FF")
                w2T_t = w2T.ap().rearrange("(c p) m -> p c m", p=P)
                for ds2 in range(2):          # 512-wide d slices
                    W2ts = []
                    for g in range(4):
                        W2t = w2p.tile([P, 8, 512], BF16, name="W2t")
                        nc.sync.dma_start(
                            W2t[:],
                            w2T_t[:, g * 8:(g + 1) * 8,
                                  ds2 * 512:(ds2 + 1) * 512])
                        W2ts.append(W2t)
                    for m2 in range(4):
                        do = ds2 * 4 + m2
                        psw = pd.tile([P, ROWS], F32, name="psw")
                        for hc in range(HC):
                            nc.tensor.matmul(
                                psw[:],
                                W2ts[hc // 8][:, hc % 8, m2 * P:(m2 + 1) * P],
                                H_sb[:, hc, :],
                                start=(hc == 0), stop=(hc == HC - 1))
                        nc.vector.tensor_scalar(psw[:], psw[:],
                                                b2_sb[:, do:do + 1], None,
                                                ALU.add)
                        nc.vector.tensor_add(FF[:, do], psw[:], Y_sb[:, do])
                _layernorm_sb(nc, tc, FF, ln2g, ln2b, Z_sb, ones_col, "ln2")

        # ---------------- stage E: h2o + log_softmax ----------------
        with contextlib.ExitStack() as se:
            ep = se.enter_context(tc.tile_pool(name="h2o_e", bufs=3))
            op_ = se.enter_context(tc.tile_pool(name="h2o_o", bufs=4))
            lp_ = se.enter_context(tc.tile_pool(name="h2o_l", bufs=1))
            pp = se.enter_context(tc.tile_pool(name="h2o_p", bufs=4,
                                               space="PSUM"))

            L16 = lp_.tile([P, B, V], F16, name="L16")          # 16 MB
            parts = lp_.tile([P, B, len(VTILES)], F32, name="parts")
            if bias_on["h2o"]:
                ones_row = lp_.tile([1, ROWS], BF16, name="ones_row_z")
                nc.sync.dma_start(ones_row[:], onesr.ap())

            for vi, (vs, vsz) in enumerate(VTILES):
                W_sb = W_sbs[vi]
                if bias_on["h2o"]:
                    bias_sb = ep.tile([1, 512], BF16, name="bias_sb")
                    nc.sync.dma_start(bias_sb[:, :vsz],
                                      h2ob.ap()[:, vs:vs + vsz])
                for rt in range(B):
                    ps = pp.tile([P, 512], F32, name="ps_l")
                    for dc in range(DC):
                        nc.tensor.matmul(
                            ps[:, :vsz],
                            Z_sb[:, dc, rt * IC:(rt + 1) * IC],
                            W_sb[:, dc, :vsz],
                            start=(dc == 0),
                            stop=(dc == DC - 1 and not bias_on["h2o"]))
                    if bias_on["h2o"]:
                        nc.tensor.matmul(
                            ps[:, :vsz],
                            ones_row[:, rt * IC:(rt + 1) * IC],
                            bias_sb[:, :vsz],
                            start=False, stop=True)
                    nc.vector.tensor_copy(L16[:, rt, vs:vs + vsz],
                                          ps[:, :vsz])
                    esc = ep.tile([P, 512], F32, name="esc", bufs=2)
                    nc.scalar.activation(
                        esc[:, :vsz], ps[:, :vsz], AF.Exp,
                        accum_out=parts[:, rt, vi:vi + 1])

            for rt in range(B):
                s_t = ep.tile([P, 1], F32, name="s_t")
                nc.vector.reduce_sum(s_t[:], parts[:, rt, :], axis=AX.X)
                lse = ep.tile([P, 1], F32, name="lse_t")
                nc.scalar.activation(lse[:], s_t[:], AF.Ln)
                for ti, (vs, vsz) in enumerate(VTILES):
                    ot = op_.tile([P, 512], F32, name="ot")
                    eng = nc.vector if ti % 2 == 0 else nc.gpsimd
                    eng.tensor_scalar(ot[:, :vsz], L16[:, rt, vs:vs + vsz],
                                      lse[:], None, ALU.subtract)
                    nc.sync.dma_start(
                        y.ap()[rt * IC:(rt + 1) * IC, vs:vs + vsz],
                        ot[:, :vsz])

    nc.compile()
    return nc


_CACHE = {}


def _ppart(vec, chunks):
    """[chunks*P] -> [P, chunks] per-partition layout."""
    return np.ascontiguousarray(vec.reshape(chunks, P).T, np.float32)


def kernel(**inputs):
    f32 = np.float32
    bf16 = ml_dtypes.bfloat16
    x = np.asarray(inputs["x"], f32)
    wv = np.asarray(inputs["wv"], f32)
    bv = np.asarray(inputs["bv"], f32)
    fc_w = np.asarray(inputs["fc_w"], f32)
    fc_b = np.asarray(inputs["fc_b"], f32)
    ln1_g = np.asarray(inputs["ln1_g"], f32)
    ln1_b = np.asarray(inputs["ln1_b"], f32)
    w1 = np.asarray(inputs["w1"], f32)
    b1 = np.asarray(inputs["b1"], f32)
    w2 = np.asarray(inputs["w2"], f32)
    b2 = np.asarray(inputs["b2"], f32)
    ln2_g = np.asarray(inputs["ln2_g"], f32)
    ln2_b = np.asarray(inputs["ln2_b"], f32)
    h2o_w = np.asarray(inputs["h2o_w"], f32)
    h2o_b = np.asarray(inputs["h2o_b"], f32)
    p0 = np.asarray(inputs["p0"], np.float64)
    p1 = np.asarray(inputs["p1"], np.float64)
    p2 = np.asarray(inputs["p2"], np.float64)
    p3 = np.asarray(inputs["p3"], np.float64)
    # wk/bk deliberately unused: constant along the softmax axis.

    sp1 = np.float32(_softplus(p1)).astype(np.float64)
    sp2 = np.float32(_softplus(p2)).astype(np.float64)

    bias_on = {"h2o": bool(np.any(h2o_b))}

    key = (p0.tobytes(), sp1.tobytes(), sp2.tobytes(), p3.tobytes(),
           bias_on["h2o"])
    if key not in _CACHE:
        _CACHE[key] = _build(p0, sp1, sp2, p3, bias_on)
    nc = _CACHE[key]

    x2T = np.ascontiguousarray(x.reshape(B * L, D).T)
    shared = {
        "xT": np.ascontiguousarray(x2T.astype(bf16)),
        "wvT": np.ascontiguousarray(wv.T.astype(bf16)),
        "fcT": np.ascontiguousarray(fc_w.T.astype(bf16)),
        "w1T": np.ascontiguousarray(w1.T.astype(bf16)),
        "w2T": np.ascontiguousarray(w2.T.astype(bf16)),
        "h2oT": np.ascontiguousarray(h2o_w.T.astype(bf16)),
        "bv2": _ppart(bv, DC),
        "fcb2": _ppart(fc_b, DC),
        "b12": _ppart(b1, HC),
        "b22": _ppart(b2, DC),
        "onesc": np.ones((P, 2), f32),
        "onesb": np.ones((P, 2), bf16),
        "ln1g": _ppart(ln1_g, DC),
        "ln1b": _ppart(ln1_b, DC),
        "ln2g": _ppart(ln2_g, DC),
        "ln2b": _ppart(ln2_b, DC),
    }
    if bias_on["h2o"]:
        shared["h2ob"] = np.ascontiguousarray(h2o_b[None].astype(bf16))
        shared["onesr"] = np.ones((1, ROWS), bf16)

    p3_zero = bool(np.all(p3 == 0.0))
    ebv = np.zeros(H, np.float64)
    for h in range(H):
        if p0[h] > 0.0 and abs(sp1[h] - sp2[h]) < 1e-12:
            ebv[h] = math.log(2.0 * p0[h])
        elif p0[h] > 0.0:
            ebv[h] = math.log(p0[h])
    expb_host = np.ascontiguousarray(
        np.broadcast_to(ebv.astype(f32)[None, :], (P, H)))

    j = np.arange(L)
    in_maps = []
    for c in range(NCORES):
        i_idx = c * IC + np.arange(IC)
        Sji = np.abs(j[:, None] - i_idx[None, :]).astype(f32)       # [L, IC]
        eye = (Sji == 0).astype(f32)
        if p3_zero:
            Rs = [NEG_BIG * eye]
        else:
            Aji = (i_idx[None, :] < j[:, None]).astype(f32)
            Rs = [np.float32(p3[h]) * Aji + NEG_BIG * eye for h in range(H)]

        def tile_ji(a):  # [L, IC] -> [jp, jc, IC]
            return np.ascontiguousarray(
                a.reshape(8, P, IC).transpose(1, 0, 2), f32)

        m = dict(shared)
        m["S_in"] = tile_ji(Sji)
        m["expb"] = expb_host
        m["R_in"] = np.stack([tile_ji(R) for R in Rs], axis=0)
        in_maps.append(m)

    res = run_bass_kernel_spmd(nc, in_maps, core_ids=list(range(NCORES)))

    out = np.empty((B, L, V), f32)
    for c in range(NCORES):
        yc = res.results[c]["y"]
        for b in range(B):
            out[b, c * IC:(c + 1) * IC, :] = yc[b * IC:(b + 1) * IC, :]
    return out


# revision 23
# speedup vs baseline: 1.4907x; 1.0148x over previous
"""Trainium2 Bass kernel for a single-layer "BiTRF" dense transformer block.

Math (see reference):
  posi[h,i,j] = p0*(exp(-sp1*|i-j|) + exp(-sp2*|i-j|)) + p3*(i<j)   (sp=softplus(p))
  attn[h,b,i,j] = kproj[b,i,h] + posi[h,i,j], diag masked, softmax over j.
  Because kproj[b,i,h] is constant along the softmax axis j, softmax is
  invariant to it, so the wk/bk projection drops out entirely and the
  attention weights W[h,i,:] are shared across the batch (and across heads
  with identical (p0, sp1, sp2, p3) — computed once per unique group).
  out  = LN1(attnout @ fc_w.T + fc_b)
  out2 = LN2(relu(out @ w1.T + b1) @ w2.T + b2 + out)
  y    = log_softmax(out2 @ h2o_w.T + h2o_b)

Sharding: 8 cores, core c owns query rows i in [c*128,(c+1)*128) for BOTH
batches (256 row-instances).  v = x@wv.T is computed redundantly on every
core (avoids any collective); everything else is row-sharded, h2o is
row-sharded too (each core computes its rows x full 32000 vocab, so
log_softmax is fully local).

The whole pre-h2o chain (v-proj, attention, fc, LN1, FFN, LN2) lives in
SBUF — no DRAM round-trips between stages.  Activations are feature-major
[feat, row] (LN partition reductions via ones-column matmuls); attention
output is transposed back with PE-transposes.  Biases are applied as
per-partition scalars at PSUM-eviction time (bv rides through the
attention because softmax rows sum to 1, so it is added at the transpose
eviction).  The h2o weight stream pool lives at top scope so its first
tiles prefetch during earlier phases.

dtypes: matmuls run bf16 (weights pre-cast on host, fp32 PSUM
accumulation); LayerNorm statistics and log-sum-exp run in fp32; raw
logits are staged in fp16 for the final lse subtraction.
"""

import contextlib
import math

import ml_dtypes
import numpy as np

import concourse.mybir as mybir
import concourse.tile as tile
from concourse import bacc
from concourse.bass_utils import run_bass_kernel_spmd
from concourse.masks import make_identity

B, L, D, H, DV, HID, V = 2, 1024, 1024, 16, 64, 4096, 32000
NCORES = 8
IC = L // NCORES        # 128 query rows per core
ROWS = B * IC           # 256 row-instances per core
HD = H * DV             # 1024
P = 128
DC = D // P             # 8 feature chunks
HC = HID // P           # 32 hidden chunks
EPS = 1e-5
NEG_BIG = -1.0e9

F32 = mybir.dt.float32
F32R = mybir.dt.float32r
BF16 = mybir.dt.bfloat16
F16 = mybir.dt.float16
AF = mybir.ActivationFunctionType
ALU = mybir.AluOpType
AX = mybir.AxisListType

# h2o vocab tiling: 62 tiles of 512 + 1 tile of 256
VTILES = [(i * 512, 512) for i in range(62)] + [(62 * 512, 256)]


def _r(ap):
    return ap.bitcast(F32R)


def _softplus(x):
    return np.logaddexp(0.0, x.astype(np.float64))


def _layernorm_sb(nc, tc, F_sb, g_dram, b_dram, Y_sb, ones_col, tag):
    """LN over the feature (partition) axis, fully in SBUF.
    F_sb: [P, DC, ROWS] f32r source; Y_sb: [P, DC, ROWS] dst (any dtype)."""
    with contextlib.ExitStack() as ctx:
        lp = ctx.enter_context(tc.tile_pool(name=f"ln_{tag}", bufs=2))
        cp = ctx.enter_context(tc.tile_pool(name=f"lnc_{tag}", bufs=1))
        pp = ctx.enter_context(tc.tile_pool(name=f"lnp_{tag}", bufs=2, space="PSUM"))

        SQ = lp.tile([P, DC, ROWS], F32R, name=f"SQ_{tag}")
        nc.vector.tensor_mul(SQ[:], F_sb[:], F_sb[:])

        g_sb = cp.tile([P, DC], F32, name=f"g_{tag}")
        nc.sync.dma_start(g_sb[:], g_dram.ap())
        b_sb = cp.tile([P, DC], F32, name=f"b_{tag}")
        nc.sync.dma_start(b_sb[:], b_dram.ap())

        ps_sum = pp.tile([2, ROWS], F32, name=f"pssum_{tag}")
        ps_sq = pp.tile([2, ROWS], F32, name=f"pssq_{tag}")
        for dc in range(DC):
            nc.tensor.matmul(ps_sum[:], ones_col[:], F_sb[:, dc],
                             start=(dc == 0), stop=(dc == DC - 1))
            nc.tensor.matmul(ps_sq[:], ones_col[:], SQ[:, dc],
                             start=(dc == 0), stop=(dc == DC - 1))

        mean = lp.tile([1, ROWS], F32, name=f"mean_{tag}")
        nc.vector.tensor_scalar(mean[:], ps_sum[0:1, :], 1.0 / D, None, ALU.mult)
        ex2 = lp.tile([1, ROWS], F32, name=f"ex2_{tag}")
        nc.vector.tensor_scalar(ex2[:], ps_sq[0:1, :], 1.0 / D, None, ALU.mult)
        var = lp.tile([1, ROWS], F32, name=f"var_{tag}")
        nc.vector.tensor_mul(var[:], mean[:], mean[:])
        nc.vector.tensor_sub(var[:], ex2[:], var[:])
        veps = lp.tile([1, ROWS], F32, name=f"veps_{tag}")
        nc.vector.tensor_scalar(veps[:], var[:], EPS, None, ALU.add)
        s0 = lp.tile([1, ROWS], F32, name=f"s0_{tag}")
        nc.scalar.activation(s0[:], veps[:], AF.Sqrt)
        r0 = lp.tile([1, ROWS], F32, name=f"r0_{tag}")
        nc.vector.reciprocal(r0[:], s0[:])
        s1 = lp.tile([1, ROWS], F32, name=f"s1_{tag}")
        nc.vector.tensor_mul(s1[:], veps[:], r0[:])
        nc.vector.tensor_add(s1[:], s1[:], s0[:])
        nc.vector.tensor_scalar(s1[:], s1[:], 0.5, None, ALU.mult)
        rstd = lp.tile([1, ROWS], F32, name=f"rstd_{tag}")
        nc.vector.reciprocal(rstd[:], s1[:])

        meanB = lp.tile([P, ROWS], F32, name=f"meanB_{tag}")
        nc.gpsimd.partition_broadcast(meanB[:], mean[:])
        rstdB = lp.tile([P, ROWS], F32, name=f"rstdB_{tag}")
        nc.gpsimd.partition_broadcast(rstdB[:], rstd[:])

        for dc in range(DC):
            t1 = lp.tile([P, ROWS], F32, name=f"t1_{tag}", bufs=3)
            nc.vector.tensor_sub(t1[:], F_sb[:, dc], meanB[:])
            nc.vector.tensor_mul(t1[:], t1[:], rstdB[:])
            nc.vector.tensor_scalar(Y_sb[:, dc], t1[:],
                                    g_sb[:, dc:dc + 1], b_sb[:, dc:dc + 1],
                                    ALU.mult, ALU.add)


def _build(p0, sp1, sp2, p3, bias_on):
    """Build + compile the SPMD program.  p0/sp1/sp2/p3 are [H] host floats
    baked into the NEFF as activation immediates; bias_on['h2o'] selects the
    rank-1 vocab-bias matmul (other biases are always applied, free)."""
    p3_zero = bool(np.all(p3 == 0.0))
    n_r = 1 if p3_zero else H

    nc = bacc.Bacc(None, target_bir_lowering=False, debug=False,
                   num_devices=NCORES)

    def inp(name, shape, dtype):
        return nc.dram_tensor(name, shape, dtype, kind="ExternalInput")

    xT = inp("xT", [D, B * L], BF16)
    wvT = inp("wvT", [D, HD], BF16)
    fcT = inp("fcT", [HD, D], BF16)
    w1T = inp("w1T", [D, HID], BF16)
    w2T = inp("w2T", [HID, D], BF16)
    h2oT = inp("h2oT", [D, V], BF16)
    bv2 = inp("bv2", [P, DC], F32)
    fcb2 = inp("fcb2", [P, DC], F32)
    b12 = inp("b12", [P, HC], F32)
    b22 = inp("b22", [P, DC], F32)
    if bias_on["h2o"]:
        h2ob = inp("h2ob", [1, V], BF16)
        onesr = inp("onesr", [1, ROWS], BF16)
    onesc = inp("onesc", [P, 2], F32R)
    onesb = inp("onesb", [P, 2], BF16)
    ln1g = inp("ln1g", [P, DC], F32)
    ln1b = inp("ln1b", [P, DC], F32)
    ln2g = inp("ln2g", [P, DC], F32)
    ln2b = inp("ln2b", [P, DC], F32)
    S_in = inp("S_in", [P, 8, IC], F32)          # |i-j| tiled [jp, jc, i]
    expb = inp("expb", [P, H], F32)              # per-head exp bias ln(2*p0)
    R_in = inp("R_in", [n_r, P, 8, IC], F32)     # p3*(i<j) - BIG*eye, per head
    y = nc.dram_tensor("y", [ROWS, V], F32, kind="ExternalOutput")

    with tile.TileContext(nc) as tc, contextlib.ExitStack() as top:
        c0 = top.enter_context(tc.tile_pool(name="const0", bufs=1))
        wp = top.enter_context(tc.tile_pool(name="h2o_w", bufs=6))
        zp = top.enter_context(tc.tile_pool(name="zmid", bufs=1))

        ones_col = c0.tile([P, 2], F32R, name="ones_col")
        nc.sync.dma_start(ones_col[:], onesc.ap())
        ones_colb = c0.tile([P, 2], BF16, name="ones_colb")
        nc.sync.dma_start(ones_colb[:], onesb.ap())
        Z_sb = zp.tile([P, DC, ROWS], BF16, name="Z_sb")

        with contextlib.ExitStack() as s1:
            OT = s1.enter_context(tc.tile_pool(name="otp", bufs=1)).tile(
                [P, DC, ROWS], BF16, name="OT")
            # fc weights pool created before the A/B scope so pool
            # stack order holds; DMA traced here too (no deps, prefetches)
            fcp0 = s1.enter_context(tc.tile_pool(name="fcc", bufs=1))
            fcT_sb = fcp0.tile([P, DC, D], BF16, name="fcT_sb")
            fcb_sb = fcp0.tile([P, DC], F32, name="fcb_sb")

            sab = contextlib.ExitStack()
            # ---------------- stage A: v = x @ wv.T ----------------
            vp = sab.enter_context(tc.tile_pool(name="vpool", bufs=1))
            v_sb = vp.tile([P, B * L // P, HD], BF16, name="v_sb")
            with contextlib.ExitStack() as sa:
                rp = sa.enter_context(tc.tile_pool(name="resid", bufs=1))
                pa = sa.enter_context(tc.tile_pool(name="psA", bufs=4,
                                                   space="PSUM"))
                xT_sb = rp.tile([P, DC, B * L], BF16, name="xT_sb")
                xT_t = xT.ap().rearrange("(c p) r -> p c r", p=P)
                wvT_sb = rp.tile([P, DC, HD], BF16, name="wvT_sb")
                wvT_t = wvT.ap().rearrange("(c p) f -> p c f", p=P)
                for dc in range(DC):
                    nc.sync.dma_start(wvT_sb[:, dc], wvT_t[:, dc])
                for rcg in range(4):
                    for dc in range(DC):
                        nc.sync.dma_start(
                            xT_sb[:, dc, rcg * 512:(rcg + 1) * 512],
                            xT_t[:, dc, rcg * 512:(rcg + 1) * 512])
                for rc in range(B * L // P):
                    for nh in range(2):
                        psv = pa.tile([P, 512], F32, name="psv")
                        for dc in range(DC):
                            nc.tensor.matmul(
                                psv[:],
                                xT_sb[:, dc, rc * P:(rc + 1) * P],
                                wvT_sb[:, dc, nh * 512:(nh + 1) * 512],
                                start=(dc == 0), stop=(dc == DC - 1))
                        nc.vector.tensor_copy(
                            v_sb[:, rc, nh * 512:(nh + 1) * 512], psv[:])

            # fc weight prefetch: traced after stage A's input loads so it
            # doesn't delay them; lands during attention
            nc.sync.dma_start(fcT_sb[:],
                              fcT.ap().rearrange("(c p) f -> p c f", p=P))
            nc.sync.dma_start(fcb_sb[:], fcb2.ap())

            # ---------------- stage B: attention ----------------
            # (bv is added at the transpose eviction: softmax rows sum to 1)
            with sab, contextlib.ExitStack() as sb:
                up = sb.enter_context(tc.tile_pool(name="attn_u", bufs=2))
                sp_ = sb.enter_context(tc.tile_pool(name="attn_s", bufs=3))
                cp = sb.enter_context(tc.tile_pool(name="attn_c", bufs=1))
                ab = sb.enter_context(tc.tile_pool(name="attn_b", bufs=1))
                pp = sb.enter_context(tc.tile_pool(name="attn_p", bufs=2,
                                                   space="PSUM"))

                S_sb = cp.tile([P, 8, IC], F32, name="S_sb")
                nc.sync.dma_start(S_sb[:], S_in.ap())
                eb_sb = cp.tile([P, H], F32, name="eb_sb")
                nc.sync.dma_start(eb_sb[:], expb.ap())
                bv_sb = cp.tile([P, DC], F32, name="bv_sb")
                nc.sync.dma_start(bv_sb[:], bv2.ap())
                ident = cp.tile([P, P], F32, name="ident")
                make_identity(nc, ident[:])
                R_sb = None
                O_sb = ab.tile([P, B, HD], F32, name="O_sb")

                hkeys = [(float(p0[h]), float(sp1[h]), float(sp2[h]),
                          float(p3[h])) for h in range(H)]
                n_groups = len(set(hkeys))
                gup = sb.enter_context(
                    tc.tile_pool(name="attn_gu", bufs=min(n_groups + 1, H)))
                grp = {}
                for h in range(H):
                    if hkeys[h] in grp:
                        u_sb, rs = grp[hkeys[h]]
                    else:
                        if R_sb is None or n_r > 1:
                            R_sb = cp.tile([P, 8, IC], F32, name="R_sb",
                                           bufs=2)
                            nc.sync.dma_start(R_sb[:],
                                              R_in.ap()[min(h, n_r - 1)])
                        t_sb = up.tile([P, 8, IC], F32, name="t_sb")
                        if p0[h] > 0.0 and abs(sp1[h] - sp2[h]) < 1e-12:
                            nc.scalar.activation(t_sb[:], S_sb[:], AF.Exp,
                                                 scale=-sp1[h],
                                                 bias=eb_sb[:, h:h + 1])
                        elif p0[h] > 0.0:
                            e2 = up.tile([P, 8, IC], F32, name="e2_sb")
                            nc.scalar.activation(t_sb[:], S_sb[:], AF.Exp,
                                                 scale=-sp1[h],
                                                 bias=eb_sb[:, h:h + 1])
                            nc.scalar.activation(e2[:], S_sb[:], AF.Exp,
                                                 scale=-sp2[h],
                                                 bias=eb_sb[:, h:h + 1])
                            nc.vector.tensor_add(t_sb[:], t_sb[:], e2[:])
                        elif p0[h] == 0.0:
                            nc.any.memset(t_sb[:], 0.0)
                        else:
                            e2 = up.tile([P, 8, IC], F32, name="e2_sb")
                            nc.scalar.activation(t_sb[:], S_sb[:], AF.Exp,
                                                 scale=-sp1[h])
                            nc.scalar.activation(e2[:], S_sb[:], AF.Exp,
                                                 scale=-sp2[h])
                            nc.vector.tensor_add(t_sb[:], t_sb[:], e2[:])
                            nc.vector.tensor_scalar(t_sb[:], t_sb[:], p0[h],
                                                    None, ALU.mult)
                        nc.vector.tensor_add(t_sb[:], t_sb[:], R_sb[:])
                        u_sb = gup.tile([P, 8, IC], BF16, name="u_sb")
                        nc.scalar.activation(u_sb[:], t_sb[:], AF.Exp)
                        ps_s = pp.tile([P, 2], F32, name="ps_s")
                        for jc in range(8):
                            nc.tensor.matmul(ps_s[:], u_sb[:, jc],
                                             ones_colb[:],
                                             start=(jc == 0), stop=(jc == 7))
                        rs = sp_.tile([P, 1], F32, name="rs_t",
                                      bufs=min(n_groups + 1, H))
                        nc.vector.reciprocal(rs[:], ps_s[:, 0:1])
                        grp[hkeys[h]] = (u_sb, rs)

                    ps_o = [pp.tile([P, DV], F32, name=f"ps_o{b}")
                            for b in range(B)]
                    for jc in range(8):
                        lhsT = u_sb[:, jc]
                        for b in range(B):
                            nc.tensor.matmul(
                                ps_o[b][:], lhsT,
                                v_sb[:, b * 8 + jc, h * DV:(h + 1) * DV],
                                start=(jc == 0), stop=(jc == 7))
                    for b in range(B):
                        nc.vector.tensor_scalar(
                            O_sb[:, b, h * DV:(h + 1) * DV],
                            ps_o[b][:], rs[:], None, ALU.mult)

                    # once both heads of a 128-col chunk are done,
                    # transpose it to feature-major (adding bv; exact since
                    # softmax rows sum to 1)
                    if h % 2 == 1:
                        hc = h // 2
                        for b in range(B):
                            pt = pp.tile([P, P], F32, name="pt")
                            nc.tensor.transpose(
                                pt[:], O_sb[:, b, hc * P:(hc + 1) * P],
                                ident[:])
                            nc.vector.tensor_scalar(
                                OT[:, hc, b * IC:(b + 1) * IC], pt[:],
                                bv_sb[:, hc:hc + 1], None, ALU.add)


            # h2o weight stream (traced here so it doesn't outprioritize
            # the stage-A input loads; still prefetches during fc/FFN)
            h2oT_t = h2oT.ap().rearrange("(c p) v -> p c v", p=P)
            W_sbs = []
            for vi, (vs, vsz) in enumerate(VTILES):
                W_sb = wp.tile([P, DC, 512], BF16, name="W_sb")
                nc.sync.dma_start(W_sb[:, :, :vsz], h2oT_t[:, :, vs:vs + vsz])
                W_sbs.append(W_sb)

            # ---------------- stage C: fc + LN1 ----------------
            yp = s1.enter_context(tc.tile_pool(name="ypool", bufs=1))
            Y_sb = yp.tile([P, DC, ROWS], BF16, name="Y_sb")
            with contextlib.ExitStack() as sc:
                fp = sc.enter_context(tc.tile_pool(name="fcp", bufs=1))
                pc = sc.enter_context(tc.tile_pool(name="psC", bufs=4,
                                                   space="PSUM"))
                F1 = fp.tile([P, DC, ROWS], F32R, name="F1")
                for half in range(2):
                    psfs = [pc.tile([P, ROWS], F32, name=f"psf{q}", bufs=1)
                            for q in range(4)]
                    for hc in range(DC):
                        for q in range(4):
                            do = half * 4 + q
                            nc.tensor.matmul(
                                psfs[q][:],
                                fcT_sb[:, hc, do * P:(do + 1) * P],
                                OT[:, hc, :],
                                start=(hc == 0), stop=(hc == DC - 1))
                    for q in range(4):
                        do = half * 4 + q
                        nc.vector.tensor_scalar(F1[:, do], psfs[q][:],
                                                fcb_sb[:, do:do + 1], None,
                                                ALU.add)
                _layernorm_sb(nc, tc, F1, ln1g, ln1b, Y_sb, ones_col, "ln1")

            # ---------------- stage D: FFN ----------------
            with contextlib.ExitStack() as sd:
                hp = sd.enter_context(tc.tile_pool(name="hpool", bufs=1))
                wsp = sd.enter_context(tc.tile_pool(name="wstr", bufs=2))
                w2p = sd.enter_context(tc.tile_pool(name="w2str", bufs=5))
                cd = sd.enter_context(tc.tile_pool(name="cD", bufs=1))
                pd = sd.enter_context(tc.tile_pool(name="psD", bufs=2,
                                                   space="PSUM"))
                H_sb = hp.tile([P, HC, ROWS], BF16, name="H_sb")
                b1_sb = cd.tile([P, HC], F32, name="b1_sb")
                nc.sync.dma_start(b1_sb[:], b12.ap())
                b2_sb = cd.tile([P, DC], F32, name="b2_sb")
                nc.sync.dma_start(b2_sb[:], b22.ap())

                w1T_t = w1T.ap().rearrange("(c p) m -> p c m", p=P)
                for hs in range(8):           # 512-wide hid slices
                    W1t = wsp.tile([P, DC, 512], BF16, name="W1t")
                    nc.sync.dma_start(W1t[:],
                                      w1T_t[:, :, hs * 512:(hs + 1) * 512])
                    for m2 in range(4):       # 128-wide subchunks
                        psh = pd.tile([P, ROWS], F32, name="psh")
                        for dc in range(DC):
                            nc.tensor.matmul(
                                psh[:],
                                W1t[:, dc, m2 * P:(m2 + 1) * P],
                                Y_sb[:, dc, :],
                                start=(dc == 0), stop=(dc == DC - 1))
                        hcix = hs * 4 + m2
                        nc.scalar.activation(H_sb[:, hcix], psh[:], AF.Relu,
                                             bias=b1_sb[:, hcix:hcix + 1])

                FF = hp.tile([P, DC, ROWS], F32R, name="FF")
                w2T_t = w2T.ap().rearrange("(c p) m -> p c m", p=P)
                for ds2 in range(2):          # 512-wide d slices
                    W2ts = []
                    for g in range(4):
                        W2t = w2p.tile([P, 8, 512], BF16, name="W2t")
                        nc.sync.dma_start(
                            W2t[:],
                            w2T_t[:, g * 8:(g + 1) * 8,
                                  ds2 * 512:(ds2 + 1) * 512])
                        W2ts.append(W2t)
                    for m2 in range(4):
                        do = ds2 * 4 + m2
                        psw = pd.tile([P, ROWS], F32, name="psw")
                        for hc in range(HC):
                            nc.tensor.matmul(
                                psw[:],
                                W2ts[hc // 8][:, hc % 8, m2 * P:(m2 + 1) * P],
                                H_sb[:, hc, :],
                                start=(hc == 0), stop=(hc == HC - 1))
                        nc.vector.tensor_scalar(psw[:], psw[:],
                                                b2_sb[:, do:do + 1], None,
                                                ALU.add)
                        nc.vector.tensor_add(FF[:, do], psw[:], Y_sb[:, do])
                _layernorm_sb(nc, tc, FF, ln2g, ln2b, Z_sb, ones_col, "ln2")

        # ---------------- stage E: h2o + log_softmax ----------------
        with contextlib.ExitStack() as se:
            ep = se.enter_context(tc.tile_pool(name="h2o_e", bufs=3))
            op_ = se.enter_context(tc.tile_pool(name="h2o_o", bufs=4))
            lp_ = se.enter_context(tc.tile_pool(name="h2o_l", bufs=1))
            pp = se.enter_context(tc.tile_pool(name="h2o_p", bufs=4,
                                               space="PSUM"))

            L16 = lp_.tile([P, B, V], F16, name="L16")          # 16 MB
            parts = lp_.tile([P, B, len(VTILES)], F32, name="parts")
            if bias_on["h2o"]:
                ones_row = lp_.tile([1, ROWS], BF16, name="ones_row_z")
                nc.sync.dma_start(ones_row[:], onesr.ap())

            for vi, (vs, vsz) in enumerate(VTILES):
                W_sb = W_sbs[vi]
                if bias_on["h2o"]:
                    bias_sb = ep.tile([1, 512], BF16, name="bias_sb")
                    nc.sync.dma_start(bias_sb[:, :vsz],
                                      h2ob.ap()[:, vs:vs + vsz])
                for rt in range(B):
                    ps = pp.tile([P, 512], F32, name="ps_l")
                    for dc in range(DC):
                        nc.tensor.matmul(
                            ps[:, :vsz],
                            Z_sb[:, dc, rt * IC:(rt + 1) * IC],
                            W_sb[:, dc, :vsz],
                            start=(dc == 0),
                            stop=(dc == DC - 1 and not bias_on["h2o"]))
                    if bias_on["h2o"]:
                        nc.tensor.matmul(
                            ps[:, :vsz],
                            ones_row[:, rt * IC:(rt + 1) * IC],
                            bias_sb[:, :vsz],
                            start=False, stop=True)
                    nc.vector.tensor_copy(L16[:, rt, vs:vs + vsz],
                                          ps[:, :vsz])
                    esc = ep.tile([P, 512], F32, name="esc", bufs=2)
                    nc.scalar.activation(
                        esc[:, :vsz], ps[:, :vsz], AF.Exp,
                        accum_out=parts[:, rt, vi:vi + 1])

            for rt in range(B):
                s_t = ep.tile([P, 1], F32, name="s_t")
                nc.vector.reduce_sum(s_t[:], parts[:, rt, :], axis=AX.X)
                lse = ep.tile([P, 1], F32, name="lse_t")
                nc.scalar.activation(lse[:], s_t[:], AF.Ln)
                for ti, (vs, vsz) in enumerate(VTILES):
                    ot = op_.tile([P, 512], F32, name="ot")
                    eng = nc.vector if ti % 2 == 0 else nc.gpsimd
                    eng.tensor_scalar(ot[:, :vsz], L16[:, rt, vs:vs + vsz],
                                      lse[:], None, ALU.subtract)
                    nc.sync.dma_start(
                        y.ap()[rt * IC:(rt + 1) * IC, vs:vs + vsz],
                        ot[:, :vsz])

    nc.compile()
    return nc


_CACHE = {}


def _ppart(vec, chunks):
    """[chunks*P] -> [P, chunks] per-partition layout."""
    return np.ascontiguousarray(vec.reshape(chunks, P).T, np.float32)


def kernel(**inputs):
    f32 = np.float32
    bf16 = ml_dtypes.bfloat16
    x = np.asarray(inputs["x"], f32)
    wv = np.asarray(inputs["wv"], f32)
    bv = np.asarray(inputs["bv"], f32)
    fc_w = np.asarray(inputs["fc_w"], f32)
    fc_b = np.asarray(inputs["fc_b"], f32)
    ln1_g = np.asarray(inputs["ln1_g"], f32)
    ln1_b = np.asarray(inputs["ln1_b"], f32)
    w1 = np.asarray(inputs["w1"], f32)
    b1 = np.asarray(inputs["b1"], f32)
    w2 = np.asarray(inputs["w2"], f32)
    b2 = np.asarray(inputs["b2"], f32)
    ln2_g = np.asarray(inputs["ln2_g"], f32)
    ln2_b = np.asarray(inputs["ln2_b"], f32)
    h2o_w = np.asarray(inputs["h2o_w"], f32)
    h2o_b = np.asarray(inputs["h2o_b"], f32)
    p0 = np.asarray(inputs["p0"], np.float64)
    p1 = np.asarray(inputs["p1"], np.float64)
    p2 = np.asarray(inputs["p2"], np.float64)
    p3 = np.asarray(inputs["p3"], np.float64)
    # wk/bk deliberately unused: constant along the softmax axis.

    sp1 = np.float32(_softplus(p1)).astype(np.float64)
    sp2 = np.float32(_softplus(p2)).astype(np.float64)

    bias_on = {"h2o": bool(np.any(h2o_b))}

    key = (p0.tobytes(), sp1.tobytes(), sp2.tobytes(), p3.tobytes(),
           bias_on["h2o"])
    if key not in _CACHE:
        _CACHE[key] = _build(p0, sp1, sp2, p3, bias_on)
    nc = _CACHE[key]

    x2T = np.ascontiguousarray(x.reshape(B * L, D).T)
    shared = {
        "xT": np.ascontiguousarray(x2T.astype(bf16)),
        "wvT": np.ascontiguousarray(wv.T.astype(bf16)),
        "fcT": np.ascontiguousarray(fc_w.T.astype(bf16)),
        "w1T": np.ascontiguousarray(w1.T.astype(bf16)),
        "w2T": np.ascontiguousarray(w2.T.astype(bf16)),
        "h2oT": np.ascontiguousarray(h2o_w.T.astype(bf16)),
        "bv2": _ppart(bv, DC),
        "fcb2": _ppart(fc_b, DC),
        "b12": _ppart(b1, HC),
        "b22": _ppart(b2, DC),
        "onesc": np.ones((P, 2), f32),
        "onesb": np.ones((P, 2), bf16),
        "ln1g": _ppart(ln1_g, DC),
        "ln1b": _ppart(ln1_b, DC),
        "ln2g": _ppart(ln2_g, DC),
        "ln2b": _ppart(ln2_b, DC),
    }
    if bias_on["h2o"]:
        shared["h2ob"] = np.ascontiguousarray(h2o_b[None].astype(bf16))
        shared["onesr"] = np.ones((1, ROWS), bf16)

    p3_zero = bool(np.all(p3 == 0.0))
    ebv = np.zeros(H, np.float64)
    for h in range(H):
        if p0[h] > 0.0 and abs(sp1[h] - sp2[h]) < 1e-12:
            ebv[h] = math.log(2.0 * p0[h])
        elif p0[h] > 0.0:
            ebv[h] = math.log(p0[h])
    expb_host = np.ascontiguousarray(
        np.broadcast_to(ebv.astype(f32)[None, :], (P, H)))

    j = np.arange(L)
    in_maps = []
    for c in range(NCORES):
        i_idx = c * IC + np.arange(IC)
        Sji = np.abs(j[:, None] - i_idx[None, :]).astype(f32)       # [L, IC]
        eye = (Sji == 0).astype(f32)
        if p3_zero:
            Rs = [NEG_BIG * eye]
        else:
            Aji = (i_idx[None, :] < j[:, None]).astype(f32)
            Rs = [np.float32(p3[h]) * Aji + NEG_BIG * eye for h in range(H)]

        def tile_ji(a):  # [L, IC] -> [jp, jc, IC]
            return np.ascontiguousarray(
                a.reshape(8, P, IC).transpose(1, 0, 2), f32)

        m = dict(shared)
        m["S_in"] = tile_ji(Sji)
        m["expb"] = expb_host
        m["R_in"] = np.stack([tile_ji(R) for R in Rs], axis=0)
        in_maps.append(m)

    res = run_bass_kernel_spmd(nc, in_maps, core_ids=list(range(NCORES)))

    out = np.empty((B, L, V), f32)
    for c in range(NCORES):
        yc = res.results[c]["y"]
        for b in range(B):
            out[b, c * IC:(c + 1) * IC, :] = yc[b * IC:(b + 1) * IC, :]
    return out


# revision 24
# speedup vs baseline: 1.5231x; 1.0218x over previous
"""Trainium2 Bass kernel for a single-layer "BiTRF" dense transformer block.

Math (see reference):
  posi[h,i,j] = p0*(exp(-sp1*|i-j|) + exp(-sp2*|i-j|)) + p3*(i<j)   (sp=softplus(p))
  attn[h,b,i,j] = kproj[b,i,h] + posi[h,i,j], diag masked, softmax over j.
  Because kproj[b,i,h] is constant along the softmax axis j, softmax is
  invariant to it, so the wk/bk projection drops out entirely and the
  attention weights W[h,i,:] are shared across the batch (and across heads
  with identical (p0, sp1, sp2, p3) — computed once per unique group).
  out  = LN1(attnout @ fc_w.T + fc_b)
  out2 = LN2(relu(out @ w1.T + b1) @ w2.T + b2 + out)
  y    = log_softmax(out2 @ h2o_w.T + h2o_b)

Sharding: 8 cores, core c owns query rows i in [c*128,(c+1)*128) for BOTH
batches (256 row-instances).  v = x@wv.T is computed redundantly on every
core (avoids any collective); everything else is row-sharded, h2o is
row-sharded too (each core computes its rows x full 32000 vocab, so
log_softmax is fully local).

The whole pre-h2o chain (v-proj, attention, fc, LN1, FFN, LN2) lives in
SBUF — no DRAM round-trips between stages.  Activations are feature-major
[feat, row] (LN partition reductions via ones-column matmuls); attention
output is transposed back with PE-transposes.  Biases are applied as
per-partition scalars at PSUM-eviction time (bv rides through the
attention because softmax rows sum to 1, so it is added at the transpose
eviction).  The h2o weight stream pool lives at top scope so its first
tiles prefetch during earlier phases.

dtypes: matmuls run bf16 (weights pre-cast on host, fp32 PSUM
accumulation); LayerNorm statistics and log-sum-exp run in fp32; raw
logits are staged in fp16 for the final lse subtraction.
"""

import contextlib
import math

import ml_dtypes
import numpy as np

import concourse.mybir as mybir
import concourse.tile as tile
from concourse import bacc
from concourse.bass_utils import run_bass_kernel_spmd
from concourse.masks import make_identity

B, L, D, H, DV, HID, V = 2, 1024, 1024, 16, 64, 4096, 32000
NCORES = 8
IC = L // NCORES        # 128 query rows per core
ROWS = B * IC           # 256 row-instances per core
HD = H * DV             # 1024
P = 128
DC = D // P             # 8 feature chunks
HC = HID // P           # 32 hidden chunks
EPS = 1e-5
NEG_BIG = -1.0e9

F32 = mybir.dt.float32
F32R = mybir.dt.float32r
BF16 = mybir.dt.bfloat16
F16 = mybir.dt.float16
AF = mybir.ActivationFunctionType
ALU = mybir.AluOpType
AX = mybir.AxisListType

# h2o vocab tiling: 62 tiles of 512 + 1 tile of 256
VTILES = [(i * 512, 512) for i in range(62)] + [(62 * 512, 256)]


def _r(ap):
    return ap.bitcast(F32R)


def _softplus(x):
    return np.logaddexp(0.0, x.astype(np.float64))


def _layernorm_sb(nc, tc, F_sb, g_dram, b_dram, Y_sb, ones_col, tag):
    """LN over the feature (partition) axis, fully in SBUF.
    F_sb: [P, DC, ROWS] f32r source; Y_sb: [P, DC, ROWS] dst (any dtype)."""
    with contextlib.ExitStack() as ctx:
        lp = ctx.enter_context(tc.tile_pool(name=f"ln_{tag}", bufs=2))
        cp = ctx.enter_context(tc.tile_pool(name=f"lnc_{tag}", bufs=1))
        pp = ctx.enter_context(tc.tile_pool(name=f"lnp_{tag}", bufs=2, space="PSUM"))

        SQ = lp.tile([P, DC, ROWS], F32R, name=f"SQ_{tag}")
        nc.vector.tensor_mul(SQ[:], F_sb[:], F_sb[:])

        g_sb = cp.tile([P, DC], F32, name=f"g_{tag}")
        nc.sync.dma_start(g_sb[:], g_dram.ap())
        b_sb = cp.tile([P, DC], F32, name=f"b_{tag}")
        nc.sync.dma_start(b_sb[:], b_dram.ap())

        ps_sum = pp.tile([2, ROWS], F32, name=f"pssum_{tag}")
        ps_sq = pp.tile([2, ROWS], F32, name=f"pssq_{tag}")
        for dc in range(DC):
            nc.tensor.matmul(ps_sum[:], ones_col[:], F_sb[:, dc],
                             start=(dc == 0), stop=(dc == DC - 1))
            nc.tensor.matmul(ps_sq[:], ones_col[:], SQ[:, dc],
                             start=(dc == 0), stop=(dc == DC - 1))

        mean = lp.tile([1, ROWS], F32, name=f"mean_{tag}")
        nc.vector.tensor_scalar(mean[:], ps_sum[0:1, :], 1.0 / D, None, ALU.mult)
        ex2 = lp.tile([1, ROWS], F32, name=f"ex2_{tag}")
        nc.vector.tensor_scalar(ex2[:], ps_sq[0:1, :], 1.0 / D, None, ALU.mult)
        var = lp.tile([1, ROWS], F32, name=f"var_{tag}")
        nc.vector.tensor_mul(var[:], mean[:], mean[:])
        nc.vector.tensor_sub(var[:], ex2[:], var[:])
        veps = lp.tile([1, ROWS], F32, name=f"veps_{tag}")
        nc.vector.tensor_scalar(veps[:], var[:], EPS, None, ALU.add)
        s0 = lp.tile([1, ROWS], F32, name=f"s0_{tag}")
        nc.scalar.activation(s0[:], veps[:], AF.Sqrt)
        r0 = lp.tile([1, ROWS], F32, name=f"r0_{tag}")
        nc.vector.reciprocal(r0[:], s0[:])
        s1 = lp.tile([1, ROWS], F32, name=f"s1_{tag}")
        nc.vector.tensor_mul(s1[:], veps[:], r0[:])
        nc.vector.tensor_add(s1[:], s1[:], s0[:])
        nc.vector.tensor_scalar(s1[:], s1[:], 0.5, None, ALU.mult)
        rstd = lp.tile([1, ROWS], F32, name=f"rstd_{tag}")
        nc.vector.reciprocal(rstd[:], s1[:])

        meanB = lp.tile([P, ROWS], F32, name=f"meanB_{tag}")
        nc.gpsimd.partition_broadcast(meanB[:], mean[:])
        rstdB = lp.tile([P, ROWS], F32, name=f"rstdB_{tag}")
        nc.gpsimd.partition_broadcast(rstdB[:], rstd[:])

        for dc in range(DC):
            t1 = lp.tile([P, ROWS], F32, name=f"t1_{tag}", bufs=3)
            nc.vector.tensor_sub(t1[:], F_sb[:, dc], meanB[:])
            nc.vector.tensor_mul(t1[:], t1[:], rstdB[:])
            nc.vector.tensor_scalar(Y_sb[:, dc], t1[:],
                                    g_sb[:, dc:dc + 1], b_sb[:, dc:dc + 1],
                                    ALU.mult, ALU.add)


def _build(p0, sp1, sp2, p3, bias_on):
    """Build + compile the SPMD program.  p0/sp1/sp2/p3 are [H] host floats
    baked into the NEFF as activation immediates; bias_on['h2o'] selects the
    rank-1 vocab-bias matmul (other biases are always applied, free)."""
    p3_zero = bool(np.all(p3 == 0.0))
    n_r = 1 if p3_zero else H

    nc = bacc.Bacc(None, target_bir_lowering=False, debug=False,
                   num_devices=NCORES)

    def inp(name, shape, dtype):
        return nc.dram_tensor(name, shape, dtype, kind="ExternalInput")

    xT = inp("xT", [D, B * L], BF16)
    wvT = inp("wvT", [D, HD], BF16)
    fcT = inp("fcT", [HD, D], BF16)
    w1T = inp("w1T", [D, HID], BF16)
    w2T = inp("w2T", [HID, D], BF16)
    h2oT = inp("h2oT", [D, V], BF16)
    bv2 = inp("bv2", [P, DC], F32)
    fcb2 = inp("fcb2", [P, DC], F32)
    b12 = inp("b12", [P, HC], F32)
    b22 = inp("b22", [P, DC], F32)
    if bias_on["h2o"]:
        h2ob = inp("h2ob", [1, V], BF16)
        onesr = inp("onesr", [1, ROWS], BF16)
    onesc = inp("onesc", [P, 2], F32R)
    onesb = inp("onesb", [P, 2], BF16)
    ln1g = inp("ln1g", [P, DC], F32)
    ln1b = inp("ln1b", [P, DC], F32)
    ln2g = inp("ln2g", [P, DC], F32)
    ln2b = inp("ln2b", [P, DC], F32)
    S_in = inp("S_in", [P, 8, IC], F32)          # |i-j| tiled [jp, jc, i]
    expb = inp("expb", [P, H], F32)              # per-head exp bias ln(2*p0)
    R_in = inp("R_in", [n_r, P, 8, IC], F32)     # p3*(i<j) - BIG*eye, per head
    # fp16 device output: logits are already fp16-staged; the extra
    # rounding is ~2^-11 * |out| (~8e-4 rel), and it halves the 32MB
    # output write that bounds the kernel tail.  Host casts back to f32.
    y = nc.dram_tensor("y", [ROWS, V], F16, kind="ExternalOutput")

    with tile.TileContext(nc) as tc, contextlib.ExitStack() as top:
        c0 = top.enter_context(tc.tile_pool(name="const0", bufs=1))
        wp = top.enter_context(tc.tile_pool(name="h2o_w", bufs=6))
        zp = top.enter_context(tc.tile_pool(name="zmid", bufs=1))

        ones_col = c0.tile([P, 2], F32R, name="ones_col")
        nc.sync.dma_start(ones_col[:], onesc.ap())
        ones_colb = c0.tile([P, 2], BF16, name="ones_colb")
        nc.sync.dma_start(ones_colb[:], onesb.ap())
        Z_sb = zp.tile([P, DC, ROWS], BF16, name="Z_sb")

        with contextlib.ExitStack() as s1:
            OT = s1.enter_context(tc.tile_pool(name="otp", bufs=1)).tile(
                [P, DC, ROWS], BF16, name="OT")
            # fc weights pool created before the A/B scope so pool
            # stack order holds; DMA traced here too (no deps, prefetches)
            fcp0 = s1.enter_context(tc.tile_pool(name="fcc", bufs=1))
            fcT_sb = fcp0.tile([P, DC, D], BF16, name="fcT_sb")
            fcb_sb = fcp0.tile([P, DC], F32, name="fcb_sb")

            sab = contextlib.ExitStack()
            # ---------------- stage A: v = x @ wv.T ----------------
            vp = sab.enter_context(tc.tile_pool(name="vpool", bufs=1))
            v_sb = vp.tile([P, B * L // P, HD], BF16, name="v_sb")
            with contextlib.ExitStack() as sa:
                rp = sa.enter_context(tc.tile_pool(name="resid", bufs=1))
                pa = sa.enter_context(tc.tile_pool(name="psA", bufs=4,
                                                   space="PSUM"))
                xT_sb = rp.tile([P, DC, B * L], BF16, name="xT_sb")
                xT_t = xT.ap().rearrange("(c p) r -> p c r", p=P)
                wvT_sb = rp.tile([P, DC, HD], BF16, name="wvT_sb")
                wvT_t = wvT.ap().rearrange("(c p) f -> p c f", p=P)
                for dc in range(DC):
                    nc.sync.dma_start(wvT_sb[:, dc], wvT_t[:, dc])
                for rcg in range(4):
                    for dc in range(DC):
                        nc.sync.dma_start(
                            xT_sb[:, dc, rcg * 512:(rcg + 1) * 512],
                            xT_t[:, dc, rcg * 512:(rcg + 1) * 512])
                for rc in range(B * L // P):
                    for nh in range(2):
                        psv = pa.tile([P, 512], F32, name="psv")
                        for dc in range(DC):
                            nc.tensor.matmul(
                                psv[:],
                                xT_sb[:, dc, rc * P:(rc + 1) * P],
                                wvT_sb[:, dc, nh * 512:(nh + 1) * 512],
                                start=(dc == 0), stop=(dc == DC - 1))
                        nc.vector.tensor_copy(
                            v_sb[:, rc, nh * 512:(nh + 1) * 512], psv[:])

            # fc weight prefetch: traced after stage A's input loads so it
            # doesn't delay them; lands during attention
            nc.sync.dma_start(fcT_sb[:],
                              fcT.ap().rearrange("(c p) f -> p c f", p=P))
            nc.sync.dma_start(fcb_sb[:], fcb2.ap())

            # ---------------- stage B: attention ----------------
            # (bv is added at the transpose eviction: softmax rows sum to 1)
            with sab, contextlib.ExitStack() as sb:
                up = sb.enter_context(tc.tile_pool(name="attn_u", bufs=2))
                sp_ = sb.enter_context(tc.tile_pool(name="attn_s", bufs=3))
                cp = sb.enter_context(tc.tile_pool(name="attn_c", bufs=1))
                ab = sb.enter_context(tc.tile_pool(name="attn_b", bufs=1))
                pp = sb.enter_context(tc.tile_pool(name="attn_p", bufs=2,
                                                   space="PSUM"))

                S_sb = cp.tile([P, 8, IC], F32, name="S_sb")
                nc.sync.dma_start(S_sb[:], S_in.ap())
                eb_sb = cp.tile([P, H], F32, name="eb_sb")
                nc.sync.dma_start(eb_sb[:], expb.ap())
                bv_sb = cp.tile([P, DC], F32, name="bv_sb")
                nc.sync.dma_start(bv_sb[:], bv2.ap())
                ident = cp.tile([P, P], F32, name="ident")
                make_identity(nc, ident[:])
                R_sb = None
                O_sb = ab.tile([P, B, HD], F32, name="O_sb")

                hkeys = [(float(p0[h]), float(sp1[h]), float(sp2[h]),
                          float(p3[h])) for h in range(H)]
                n_groups = len(set(hkeys))
                gup = sb.enter_context(
                    tc.tile_pool(name="attn_gu", bufs=min(n_groups + 1, H)))
                grp = {}
                for h in range(H):
                    if hkeys[h] in grp:
                        u_sb, rs = grp[hkeys[h]]
                    else:
                        if R_sb is None or n_r > 1:
                            R_sb = cp.tile([P, 8, IC], F32, name="R_sb",
                                           bufs=2)
                            nc.sync.dma_start(R_sb[:],
                                              R_in.ap()[min(h, n_r - 1)])
                        t_sb = up.tile([P, 8, IC], F32, name="t_sb")
                        if p0[h] > 0.0 and abs(sp1[h] - sp2[h]) < 1e-12:
                            nc.scalar.activation(t_sb[:], S_sb[:], AF.Exp,
                                                 scale=-sp1[h],
                                                 bias=eb_sb[:, h:h + 1])
                        elif p0[h] > 0.0:
                            e2 = up.tile([P, 8, IC], F32, name="e2_sb")
                            nc.scalar.activation(t_sb[:], S_sb[:], AF.Exp,
                                                 scale=-sp1[h],
                                                 bias=eb_sb[:, h:h + 1])
                            nc.scalar.activation(e2[:], S_sb[:], AF.Exp,
                                                 scale=-sp2[h],
                                                 bias=eb_sb[:, h:h + 1])
                            nc.vector.tensor_add(t_sb[:], t_sb[:], e2[:])
                        elif p0[h] == 0.0:
                            nc.any.memset(t_sb[:], 0.0)
                        else:
                            e2 = up.tile([P, 8, IC], F32, name="e2_sb")
                            nc.scalar.activation(t_sb[:], S_sb[:], AF.Exp,
                                                 scale=-sp1[h])
                            nc.scalar.activation(e2[:], S_sb[:], AF.Exp,
                                                 scale=-sp2[h])
                            nc.vector.tensor_add(t_sb[:], t_sb[:], e2[:])
                            nc.vector.tensor_scalar(t_sb[:], t_sb[:], p0[h],
                                                    None, ALU.mult)
                        nc.vector.tensor_add(t_sb[:], t_sb[:], R_sb[:])
                        u_sb = gup.tile([P, 8, IC], BF16, name="u_sb")
                        nc.scalar.activation(u_sb[:], t_sb[:], AF.Exp)
                        ps_s = pp.tile([P, 2], F32, name="ps_s")
                        for jc in range(8):
                            nc.tensor.matmul(ps_s[:], u_sb[:, jc],
                                             ones_colb[:],
                                             start=(jc == 0), stop=(jc == 7))
                        rs = sp_.tile([P, 1], F32, name="rs_t",
                                      bufs=min(n_groups + 1, H))
                        nc.vector.reciprocal(rs[:], ps_s[:, 0:1])
                        grp[hkeys[h]] = (u_sb, rs)

                    ps_o = [pp.tile([P, DV], F32, name=f"ps_o{b}")
                            for b in range(B)]
                    for jc in range(8):
                        lhsT = u_sb[:, jc]
                        for b in range(B):
                            nc.tensor.matmul(
                                ps_o[b][:], lhsT,
                                v_sb[:, b * 8 + jc, h * DV:(h + 1) * DV],
                                start=(jc == 0), stop=(jc == 7))
                    for b in range(B):
                        nc.vector.tensor_scalar(
                            O_sb[:, b, h * DV:(h + 1) * DV],
                            ps_o[b][:], rs[:], None, ALU.mult)

                    # once both heads of a 128-col chunk are done,
                    # transpose it to feature-major (adding bv; exact since
                    # softmax rows sum to 1)
                    if h % 2 == 1:
                        hc = h // 2
                        for b in range(B):
                            pt = pp.tile([P, P], F32, name="pt")
                            nc.tensor.transpose(
                                pt[:], O_sb[:, b, hc * P:(hc + 1) * P],
                                ident[:])
                            nc.vector.tensor_scalar(
                                OT[:, hc, b * IC:(b + 1) * IC], pt[:],
                                bv_sb[:, hc:hc + 1], None, ALU.add)


            # h2o weight stream (traced here so it doesn't outprioritize
            # the stage-A input loads; still prefetches during fc/FFN)
            h2oT_t = h2oT.ap().rearrange("(c p) v -> p c v", p=P)
            W_sbs = []
            for vi, (vs, vsz) in enumerate(VTILES):
                W_sb = wp.tile([P, DC, 512], BF16, name="W_sb")
                nc.sync.dma_start(W_sb[:, :, :vsz], h2oT_t[:, :, vs:vs + vsz])
                W_sbs.append(W_sb)

            # ---------------- stage C: fc + LN1 ----------------
            yp = s1.enter_context(tc.tile_pool(name="ypool", bufs=1))
            Y_sb = yp.tile([P, DC, ROWS], BF16, name="Y_sb")
            with contextlib.ExitStack() as sc:
                fp = sc.enter_context(tc.tile_pool(name="fcp", bufs=1))
                pc = sc.enter_context(tc.tile_pool(name="psC", bufs=4,
                                                   space="PSUM"))
                F1 = fp.tile([P, DC, ROWS], F32R, name="F1")
                for half in range(2):
                    psfs = [pc.tile([P, ROWS], F32, name=f"psf{q}", bufs=1)
                            for q in range(4)]
                    for hc in range(DC):
                        for q in range(4):
                            do = half * 4 + q
                            nc.tensor.matmul(
                                psfs[q][:],
                                fcT_sb[:, hc, do * P:(do + 1) * P],
                                OT[:, hc, :],
                                start=(hc == 0), stop=(hc == DC - 1))
                    for q in range(4):
                        do = half * 4 + q
                        nc.vector.tensor_scalar(F1[:, do], psfs[q][:],
                                                fcb_sb[:, do:do + 1], None,
                                                ALU.add)
                _layernorm_sb(nc, tc, F1, ln1g, ln1b, Y_sb, ones_col, "ln1")

            # ---------------- stage D: FFN ----------------
            with contextlib.ExitStack() as sd:
                hp = sd.enter_context(tc.tile_pool(name="hpool", bufs=1))
                wsp = sd.enter_context(tc.tile_pool(name="wstr", bufs=2))
                w2p = sd.enter_context(tc.tile_pool(name="w2str", bufs=5))
                cd = sd.enter_context(tc.tile_pool(name="cD", bufs=1))
                pd = sd.enter_context(tc.tile_pool(name="psD", bufs=2,
                                                   space="PSUM"))
                H_sb = hp.tile([P, HC, ROWS], BF16, name="H_sb")
                b1_sb = cd.tile([P, HC], F32, name="b1_sb")
                nc.sync.dma_start(b1_sb[:], b12.ap())
                b2_sb = cd.tile([P, DC], F32, name="b2_sb")
                nc.sync.dma_start(b2_sb[:], b22.ap())

                w1T_t = w1T.ap().rearrange("(c p) m -> p c m", p=P)
                for hs in range(8):           # 512-wide hid slices
                    W1t = wsp.tile([P, DC, 512], BF16, name="W1t")
                    nc.sync.dma_start(W1t[:],
                                      w1T_t[:, :, hs * 512:(hs + 1) * 512])
                    for m2 in range(4):       # 128-wide subchunks
                        psh = pd.tile([P, ROWS], F32, name="psh")
                        for dc in range(DC):
                            nc.tensor.matmul(
                                psh[:],
                                W1t[:, dc, m2 * P:(m2 + 1) * P],
                                Y_sb[:, dc, :],
                                start=(dc == 0), stop=(dc == DC - 1))
                        hcix = hs * 4 + m2
                        nc.scalar.activation(H_sb[:, hcix], psh[:], AF.Relu,
                                             bias=b1_sb[:, hcix:hcix + 1])

                FF = hp.tile([P, DC, ROWS], F32R, name="FF")
                w2T_t = w2T.ap().rearrange("(c p) m -> p c m", p=P)
                for ds2 in range(2):          # 512-wide d slices
                    W2ts = []
                    for g in range(4):
                        W2t = w2p.tile([P, 8, 512], BF16, name="W2t")
                        nc.sync.dma_start(
                            W2t[:],
                            w2T_t[:, g * 8:(g + 1) * 8,
                                  ds2 * 512:(ds2 + 1) * 512])
                        W2ts.append(W2t)
                    for m2 in range(4):
                        do = ds2 * 4 + m2
                        psw = pd.tile([P, ROWS], F32, name="psw")
                        for hc in range(HC):
                            nc.tensor.matmul(
                                psw[:],
                                W2ts[hc // 8][:, hc % 8, m2 * P:(m2 + 1) * P],
                                H_sb[:, hc, :],
                                start=(hc == 0), stop=(hc == HC - 1))
                        nc.vector.tensor_scalar(psw[:], psw[:],
                                                b2_sb[:, do:do + 1], None,
                                                ALU.add)
                        nc.vector.tensor_add(FF[:, do], psw[:], Y_sb[:, do])
                _layernorm_sb(nc, tc, FF, ln2g, ln2b, Z_sb, ones_col, "ln2")

        # ---------------- stage E: h2o + log_softmax ----------------
        with contextlib.ExitStack() as se:
            ep = se.enter_context(tc.tile_pool(name="h2o_e", bufs=3))
            op_ = se.enter_context(tc.tile_pool(name="h2o_o", bufs=4))
            lp_ = se.enter_context(tc.tile_pool(name="h2o_l", bufs=1))
            pp = se.enter_context(tc.tile_pool(name="h2o_p", bufs=4,
                                               space="PSUM"))

            L16 = lp_.tile([P, B, V], F16, name="L16")          # 16 MB
            parts = lp_.tile([P, B, len(VTILES)], F32, name="parts")
            if bias_on["h2o"]:
                ones_row = lp_.tile([1, ROWS], BF16, name="ones_row_z")
                nc.sync.dma_start(ones_row[:], onesr.ap())

            for vi, (vs, vsz) in enumerate(VTILES):
                W_sb = W_sbs[vi]
                if bias_on["h2o"]:
                    bias_sb = ep.tile([1, 512], BF16, name="bias_sb")
                    nc.sync.dma_start(bias_sb[:, :vsz],
                                      h2ob.ap()[:, vs:vs + vsz])
                for rt in range(B):
                    ps = pp.tile([P, 512], F32, name="ps_l")
                    for dc in range(DC):
                        nc.tensor.matmul(
                            ps[:, :vsz],
                            Z_sb[:, dc, rt * IC:(rt + 1) * IC],
                            W_sb[:, dc, :vsz],
                            start=(dc == 0),
                            stop=(dc == DC - 1 and not bias_on["h2o"]))
                    if bias_on["h2o"]:
                        nc.tensor.matmul(
                            ps[:, :vsz],
                            ones_row[:, rt * IC:(rt + 1) * IC],
                            bias_sb[:, :vsz],
                            start=False, stop=True)
                    nc.vector.tensor_copy(L16[:, rt, vs:vs + vsz],
                                          ps[:, :vsz])
                    esc = ep.tile([P, 512], F32, name="esc", bufs=2)
                    nc.scalar.activation(
                        esc[:, :vsz], ps[:, :vsz], AF.Exp,
                        accum_out=parts[:, rt, vi:vi + 1])

            for rt in range(B):
                s_t = ep.tile([P, 1], F32, name="s_t")
                nc.vector.reduce_sum(s_t[:], parts[:, rt, :], axis=AX.X)
                lse = ep.tile([P, 1], F32, name="lse_t")
                nc.scalar.activation(lse[:], s_t[:], AF.Ln)
                for ti, (vs, vsz) in enumerate(VTILES):
                    ot = op_.tile([P, 512], F16, name="ot")
                    eng = nc.vector if ti % 2 == 0 else nc.gpsimd
                    eng.tensor_scalar(ot[:, :vsz], L16[:, rt, vs:vs + vsz],
                                      lse[:], None, ALU.subtract)
                    nc.sync.dma_start(
                        y.ap()[rt * IC:(rt + 1) * IC, vs:vs + vsz],
                        ot[:, :vsz])

    nc.compile()
    return nc


_CACHE = {}


def _ppart(vec, chunks):
    """[chunks*P] -> [P, chunks] per-partition layout."""
    return np.ascontiguousarray(vec.reshape(chunks, P).T, np.float32)


def kernel(**inputs):
    f32 = np.float32
    bf16 = ml_dtypes.bfloat16
    x = np.asarray(inputs["x"], f32)
    wv = np.asarray(inputs["wv"], f32)
    bv = np.asarray(inputs["bv"], f32)
    fc_w = np.asarray(inputs["fc_w"], f32)
    fc_b = np.asarray(inputs["fc_b"], f32)
    ln1_g = np.asarray(inputs["ln1_g"], f32)
    ln1_b = np.asarray(inputs["ln1_b"], f32)
    w1 = np.asarray(inputs["w1"], f32)
    b1 = np.asarray(inputs["b1"], f32)
    w2 = np.asarray(inputs["w2"], f32)
    b2 = np.asarray(inputs["b2"], f32)
    ln2_g = np.asarray(inputs["ln2_g"], f32)
    ln2_b = np.asarray(inputs["ln2_b"], f32)
    h2o_w = np.asarray(inputs["h2o_w"], f32)
    h2o_b = np.asarray(inputs["h2o_b"], f32)
    p0 = np.asarray(inputs["p0"], np.float64)
    p1 = np.asarray(inputs["p1"], np.float64)
    p2 = np.asarray(inputs["p2"], np.float64)
    p3 = np.asarray(inputs["p3"], np.float64)
    # wk/bk deliberately unused: constant along the softmax axis.

    sp1 = np.float32(_softplus(p1)).astype(np.float64)
    sp2 = np.float32(_softplus(p2)).astype(np.float64)

    bias_on = {"h2o": bool(np.any(h2o_b))}

    key = (p0.tobytes(), sp1.tobytes(), sp2.tobytes(), p3.tobytes(),
           bias_on["h2o"])
    if key not in _CACHE:
        _CACHE[key] = _build(p0, sp1, sp2, p3, bias_on)
    nc = _CACHE[key]

    x2T = np.ascontiguousarray(x.reshape(B * L, D).T)
    shared = {
        "xT": np.ascontiguousarray(x2T.astype(bf16)),
        "wvT": np.ascontiguousarray(wv.T.astype(bf16)),
        "fcT": np.ascontiguousarray(fc_w.T.astype(bf16)),
        "w1T": np.ascontiguousarray(w1.T.astype(bf16)),
        "w2T": np.ascontiguousarray(w2.T.astype(bf16)),
        "h2oT": np.ascontiguousarray(h2o_w.T.astype(bf16)),
        "bv2": _ppart(bv, DC),
        "fcb2": _ppart(fc_b, DC),
        "b12": _ppart(b1, HC),
        "b22": _ppart(b2, DC),
        "onesc": np.ones((P, 2), f32),
        "onesb": np.ones((P, 2), bf16),
        "ln1g": _ppart(ln1_g, DC),
        "ln1b": _ppart(ln1_b, DC),
        "ln2g": _ppart(ln2_g, DC),
        "ln2b": _ppart(ln2_b, DC),
    }
    if bias_on["h2o"]:
        shared["h2ob"] = np.ascontiguousarray(h2o_b[None].astype(bf16))
        shared["onesr"] = np.ones((1, ROWS), bf16)

    p3_zero = bool(np.all(p3 == 0.0))
    ebv = np.zeros(H, np.float64)
    for h in range(H):
        if p0[h] > 0.0 and abs(sp1[h] - sp2[h]) < 1e-12:
            ebv[h] = math.log(2.0 * p0[h])
        elif p0[h] > 0.0:
            ebv[h] = math.log(p0[h])
    expb_host = np.ascontiguousarray(
        np.broadcast_to(ebv.astype(f32)[None, :], (P, H)))

    j = np.arange(L)
    in_maps = []
    for c in range(NCORES):
        i_idx = c * IC + np.arange(IC)
        Sji = np.abs(j[:, None] - i_idx[None, :]).astype(f32)       # [L, IC]
        eye = (Sji == 0).astype(f32)
        if p3_zero:
            Rs = [NEG_BIG * eye]
        else:
            Aji = (i_idx[None, :] < j[:, None]).astype(f32)
            Rs = [np.float32(p3[h]) * Aji + NEG_BIG * eye for h in range(H)]

        def tile_ji(a):  # [L, IC] -> [jp, jc, IC]
            return np.ascontiguousarray(
                a.reshape(8, P, IC).transpose(1, 0, 2), f32)

        m = dict(shared)
        m["S_in"] = tile_ji(Sji)
        m["expb"] = expb_host
        m["R_in"] = np.stack([tile_ji(R) for R in Rs], axis=0)
        in_maps.append(m)

    res = run_bass_kernel_spmd(nc, in_maps, core_ids=list(range(NCORES)))

    out = np.empty((B, L, V), f32)
    for c in range(NCORES):
        yc = res.results[c]["y"]
        for b in range(B):
            out[b, c * IC:(c + 1) * IC, :] = yc[b * IC:(b + 1) * IC, :]
    return out


# revision 25
# speedup vs baseline: 1.5325x; 1.0061x over previous
"""Trainium2 Bass kernel for a single-layer "BiTRF" dense transformer block.

Math (see reference):
  posi[h,i,j] = p0*(exp(-sp1*|i-j|) + exp(-sp2*|i-j|)) + p3*(i<j)   (sp=softplus(p))
  attn[h,b,i,j] = kproj[b,i,h] + posi[h,i,j], diag masked, softmax over j.
  Because kproj[b,i,h] is constant along the softmax axis j, softmax is
  invariant to it, so the wk/bk projection drops out entirely and the
  attention weights W[h,i,:] are shared across the batch (and across heads
  with identical (p0, sp1, sp2, p3) — computed once per unique group).
  out  = LN1(attnout @ fc_w.T + fc_b)
  out2 = LN2(relu(out @ w1.T + b1) @ w2.T + b2 + out)
  y    = log_softmax(out2 @ h2o_w.T + h2o_b)

Sharding: 8 cores, core c owns query rows i in [c*128,(c+1)*128) for BOTH
batches (256 row-instances).  v = x@wv.T is computed redundantly on every
core (avoids any collective); everything else is row-sharded, h2o is
row-sharded too (each core computes its rows x full 32000 vocab, so
log_softmax is fully local).

The whole pre-h2o chain (v-proj, attention, fc, LN1, FFN, LN2) lives in
SBUF — no DRAM round-trips between stages.  Activations are feature-major
[feat, row] (LN partition reductions via ones-column matmuls); attention
output is transposed back with PE-transposes.  Biases are applied as
per-partition scalars at PSUM-eviction time (bv rides through the
attention because softmax rows sum to 1, so it is added at the transpose
eviction).  The h2o weight stream pool lives at top scope so its first
tiles prefetch during earlier phases.

dtypes: matmuls run bf16 (weights pre-cast on host, fp32 PSUM
accumulation); LayerNorm statistics and log-sum-exp run in fp32; raw
logits are staged in fp16 for the final lse subtraction.
"""

import contextlib
import math

import ml_dtypes
import numpy as np

import concourse.mybir as mybir
import concourse.tile as tile
from concourse import bacc
from concourse.bass_utils import run_bass_kernel_spmd
from concourse.masks import make_identity

B, L, D, H, DV, HID, V = 2, 1024, 1024, 16, 64, 4096, 32000
NCORES = 8
IC = L // NCORES        # 128 query rows per core
ROWS = B * IC           # 256 row-instances per core
HD = H * DV             # 1024
P = 128
DC = D // P             # 8 feature chunks
HC = HID // P           # 32 hidden chunks
EPS = 1e-5
NEG_BIG = -1.0e9

F32 = mybir.dt.float32
F32R = mybir.dt.float32r
BF16 = mybir.dt.bfloat16
F16 = mybir.dt.float16
AF = mybir.ActivationFunctionType
ALU = mybir.AluOpType
AX = mybir.AxisListType

# h2o vocab tiling: 62 tiles of 512 + 1 tile of 256
VTILES = [(i * 512, 512) for i in range(62)] + [(62 * 512, 256)]


def _r(ap):
    return ap.bitcast(F32R)


def _softplus(x):
    return np.logaddexp(0.0, x.astype(np.float64))


def _layernorm_sb(nc, tc, F_sb, g_dram, b_dram, Y_sb, ones_col, tag):
    """LN over the feature (partition) axis, fully in SBUF.
    F_sb: [P, DC, ROWS] f32r source; Y_sb: [P, DC, ROWS] dst (any dtype)."""
    with contextlib.ExitStack() as ctx:
        lp = ctx.enter_context(tc.tile_pool(name=f"ln_{tag}", bufs=2))
        cp = ctx.enter_context(tc.tile_pool(name=f"lnc_{tag}", bufs=1))
        pp = ctx.enter_context(tc.tile_pool(name=f"lnp_{tag}", bufs=2, space="PSUM"))

        SQ = lp.tile([P, DC, ROWS], F32R, name=f"SQ_{tag}")
        nc.vector.tensor_mul(SQ[:], F_sb[:], F_sb[:])

        g_sb = cp.tile([P, DC], F32, name=f"g_{tag}")
        nc.sync.dma_start(g_sb[:], g_dram.ap())
        b_sb = cp.tile([P, DC], F32, name=f"b_{tag}")
        nc.sync.dma_start(b_sb[:], b_dram.ap())

        ps_sum = pp.tile([2, ROWS], F32, name=f"pssum_{tag}")
        ps_sq = pp.tile([2, ROWS], F32, name=f"pssq_{tag}")
        for dc in range(DC):
            nc.tensor.matmul(ps_sum[:], ones_col[:], F_sb[:, dc],
                             start=(dc == 0), stop=(dc == DC - 1))
            nc.tensor.matmul(ps_sq[:], ones_col[:], SQ[:, dc],
                             start=(dc == 0), stop=(dc == DC - 1))

        mean = lp.tile([1, ROWS], F32, name=f"mean_{tag}")
        nc.vector.tensor_scalar(mean[:], ps_sum[0:1, :], 1.0 / D, None, ALU.mult)
        ex2 = lp.tile([1, ROWS], F32, name=f"ex2_{tag}")
        nc.vector.tensor_scalar(ex2[:], ps_sq[0:1, :], 1.0 / D, None, ALU.mult)
        var = lp.tile([1, ROWS], F32, name=f"var_{tag}")
        nc.vector.tensor_mul(var[:], mean[:], mean[:])
        nc.vector.tensor_sub(var[:], ex2[:], var[:])
        veps = lp.tile([1, ROWS], F32, name=f"veps_{tag}")
        nc.vector.tensor_scalar(veps[:], var[:], EPS, None, ALU.add)
        s0 = lp.tile([1, ROWS], F32, name=f"s0_{tag}")
        nc.scalar.activation(s0[:], veps[:], AF.Sqrt)
        r0 = lp.tile([1, ROWS], F32, name=f"r0_{tag}")
        nc.vector.reciprocal(r0[:], s0[:])
        s1 = lp.tile([1, ROWS], F32, name=f"s1_{tag}")
        nc.vector.tensor_mul(s1[:], veps[:], r0[:])
        nc.vector.tensor_add(s1[:], s1[:], s0[:])
        nc.vector.tensor_scalar(s1[:], s1[:], 0.5, None, ALU.mult)
        rstd = lp.tile([1, ROWS], F32, name=f"rstd_{tag}")
        nc.vector.reciprocal(rstd[:], s1[:])

        meanB = lp.tile([P, ROWS], F32, name=f"meanB_{tag}")
        nc.gpsimd.partition_broadcast(meanB[:], mean[:])
        rstdB = lp.tile([P, ROWS], F32, name=f"rstdB_{tag}")
        nc.gpsimd.partition_broadcast(rstdB[:], rstd[:])

        for dc in range(DC):
            t1 = lp.tile([P, ROWS], F32, name=f"t1_{tag}", bufs=3)
            nc.vector.tensor_sub(t1[:], F_sb[:, dc], meanB[:])
            nc.vector.tensor_mul(t1[:], t1[:], rstdB[:])
            nc.vector.tensor_scalar(Y_sb[:, dc], t1[:],
                                    g_sb[:, dc:dc + 1], b_sb[:, dc:dc + 1],
                                    ALU.mult, ALU.add)


def _build(p0, sp1, sp2, p3, bias_on):
    """Build + compile the SPMD program.  p0/sp1/sp2/p3 are [H] host floats
    baked into the NEFF as activation immediates; bias_on['h2o'] selects the
    rank-1 vocab-bias matmul (other biases are always applied, free)."""
    p3_zero = bool(np.all(p3 == 0.0))
    n_r = 1 if p3_zero else H

    nc = bacc.Bacc(None, target_bir_lowering=False, debug=False,
                   num_devices=NCORES)

    def inp(name, shape, dtype):
        return nc.dram_tensor(name, shape, dtype, kind="ExternalInput")

    xT = inp("xT", [D, B * L], BF16)
    wvT = inp("wvT", [D, HD], BF16)
    fcT = inp("fcT", [HD, D], BF16)
    w1T = inp("w1T", [D, HID], BF16)
    w2T = inp("w2T", [HID, D], BF16)
    h2oT = inp("h2oT", [D, V], BF16)
    bv2 = inp("bv2", [P, DC], F32)
    fcb2 = inp("fcb2", [P, DC], F32)
    b12 = inp("b12", [P, HC], F32)
    b22 = inp("b22", [P, DC], F32)
    if bias_on["h2o"]:
        h2ob = inp("h2ob", [1, V], BF16)
        onesr = inp("onesr", [1, ROWS], BF16)
    onesc = inp("onesc", [P, 2], F32R)
    onesb = inp("onesb", [P, 2], BF16)
    ln1g = inp("ln1g", [P, DC], F32)
    ln1b = inp("ln1b", [P, DC], F32)
    ln2g = inp("ln2g", [P, DC], F32)
    ln2b = inp("ln2b", [P, DC], F32)
    S_in = inp("S_in", [P, 8, IC], F32)          # |i-j| tiled [jp, jc, i]
    expb = inp("expb", [P, H], F32)              # per-head exp bias ln(2*p0)
    R_in = inp("R_in", [n_r, P, 8, IC], F32)     # p3*(i<j) - BIG*eye, per head
    # fp16 device output: logits are already fp16-staged; the extra
    # rounding is ~2^-11 * |out| (~8e-4 rel), and it halves the 32MB
    # output write that bounds the kernel tail.  Host casts back to f32.
    y = nc.dram_tensor("y", [ROWS, V], F16, kind="ExternalOutput")

    with tile.TileContext(nc) as tc, contextlib.ExitStack() as top:
        c0 = top.enter_context(tc.tile_pool(name="const0", bufs=1))
        wp = top.enter_context(tc.tile_pool(name="h2o_w", bufs=6))
        zp = top.enter_context(tc.tile_pool(name="zmid", bufs=1))

        ones_col = c0.tile([P, 2], F32R, name="ones_col")
        nc.sync.dma_start(ones_col[:], onesc.ap())
        ones_colb = c0.tile([P, 2], BF16, name="ones_colb")
        nc.sync.dma_start(ones_colb[:], onesb.ap())
        Z_sb = zp.tile([P, DC, ROWS], BF16, name="Z_sb")

        with contextlib.ExitStack() as s1:
            OT = s1.enter_context(tc.tile_pool(name="otp", bufs=1)).tile(
                [P, DC, ROWS], BF16, name="OT")
            # fc weights pool created before the A/B scope so pool
            # stack order holds; DMA traced here too (no deps, prefetches)
            fcp0 = s1.enter_context(tc.tile_pool(name="fcc", bufs=1))
            fcT_sb = fcp0.tile([P, DC, D], BF16, name="fcT_sb")
            fcb_sb = fcp0.tile([P, DC], F32, name="fcb_sb")

            sab = contextlib.ExitStack()
            # ---------------- stage A: v = x @ wv.T ----------------
            vp = sab.enter_context(tc.tile_pool(name="vpool", bufs=1))
            v_sb = vp.tile([P, B * L // P, HD], BF16, name="v_sb")
            with contextlib.ExitStack() as sa:
                rp = sa.enter_context(tc.tile_pool(name="resid", bufs=1))
                pa = sa.enter_context(tc.tile_pool(name="psA", bufs=4,
                                                   space="PSUM"))
                xT_sb = rp.tile([P, DC, B * L], BF16, name="xT_sb")
                xT_t = xT.ap().rearrange("(c p) r -> p c r", p=P)
                wvT_sb = rp.tile([P, DC, HD], BF16, name="wvT_sb")
                wvT_t = wvT.ap().rearrange("(c p) f -> p c f", p=P)
                for dc in range(DC):
                    nc.sync.dma_start(wvT_sb[:, dc], wvT_t[:, dc])
                for rcg in range(4):
                    for dc in range(DC):
                        nc.sync.dma_start(
                            xT_sb[:, dc, rcg * 512:(rcg + 1) * 512],
                            xT_t[:, dc, rcg * 512:(rcg + 1) * 512])
                for rc in range(B * L // P):
                    for nh in range(2):
                        psv = pa.tile([P, 512], F32, name="psv")
                        for dc in range(DC):
                            nc.tensor.matmul(
                                psv[:],
                                xT_sb[:, dc, rc * P:(rc + 1) * P],
                                wvT_sb[:, dc, nh * 512:(nh + 1) * 512],
                                start=(dc == 0), stop=(dc == DC - 1))
                        nc.vector.tensor_copy(
                            v_sb[:, rc, nh * 512:(nh + 1) * 512], psv[:])

            # fc weight prefetch: traced after stage A's input loads so it
            # doesn't delay them; lands during attention
            nc.sync.dma_start(fcT_sb[:],
                              fcT.ap().rearrange("(c p) f -> p c f", p=P))
            nc.sync.dma_start(fcb_sb[:], fcb2.ap())

            # ---------------- stage B: attention ----------------
            # (bv is added at the transpose eviction: softmax rows sum to 1)
            with sab, contextlib.ExitStack() as sb:
                up = sb.enter_context(tc.tile_pool(name="attn_u", bufs=2))
                sp_ = sb.enter_context(tc.tile_pool(name="attn_s", bufs=3))
                cp = sb.enter_context(tc.tile_pool(name="attn_c", bufs=1))
                ab = sb.enter_context(tc.tile_pool(name="attn_b", bufs=1))
                pp = sb.enter_context(tc.tile_pool(name="attn_p", bufs=2,
                                                   space="PSUM"))

                S_sb = cp.tile([P, 8, IC], F32, name="S_sb")
                nc.sync.dma_start(S_sb[:], S_in.ap())
                eb_sb = cp.tile([P, H], F32, name="eb_sb")
                nc.sync.dma_start(eb_sb[:], expb.ap())
                bv_sb = cp.tile([P, DC], F32, name="bv_sb")
                nc.sync.dma_start(bv_sb[:], bv2.ap())
                ident = cp.tile([P, P], F32, name="ident")
                make_identity(nc, ident[:])
                R_sb = None
                O_sb = ab.tile([P, B, HD], F32, name="O_sb")

                hkeys = [(float(p0[h]), float(sp1[h]), float(sp2[h]),
                          float(p3[h])) for h in range(H)]
                n_groups = len(set(hkeys))
                gup = sb.enter_context(
                    tc.tile_pool(name="attn_gu", bufs=min(n_groups + 1, H)))
                grp = {}
                for h in range(H):
                    if hkeys[h] in grp:
                        u_sb, rs = grp[hkeys[h]]
                    else:
                        if R_sb is None or n_r > 1:
                            R_sb = cp.tile([P, 8, IC], F32, name="R_sb",
                                           bufs=2)
                            nc.sync.dma_start(R_sb[:],
                                              R_in.ap()[min(h, n_r - 1)])
                        t_sb = up.tile([P, 8, IC], F32, name="t_sb")
                        if p0[h] > 0.0 and abs(sp1[h] - sp2[h]) < 1e-12:
                            nc.scalar.activation(t_sb[:], S_sb[:], AF.Exp,
                                                 scale=-sp1[h],
                                                 bias=eb_sb[:, h:h + 1])
                        elif p0[h] > 0.0:
                            e2 = up.tile([P, 8, IC], F32, name="e2_sb")
                            nc.scalar.activation(t_sb[:], S_sb[:], AF.Exp,
                                                 scale=-sp1[h],
                                                 bias=eb_sb[:, h:h + 1])
                            nc.scalar.activation(e2[:], S_sb[:], AF.Exp,
                                                 scale=-sp2[h],
                                                 bias=eb_sb[:, h:h + 1])
                            nc.vector.tensor_add(t_sb[:], t_sb[:], e2[:])
                        elif p0[h] == 0.0:
                            nc.any.memset(t_sb[:], 0.0)
                        else:
                            e2 = up.tile([P, 8, IC], F32, name="e2_sb")
                            nc.scalar.activation(t_sb[:], S_sb[:], AF.Exp,
                                                 scale=-sp1[h])
                            nc.scalar.activation(e2[:], S_sb[:], AF.Exp,
                                                 scale=-sp2[h])
                            nc.vector.tensor_add(t_sb[:], t_sb[:], e2[:])
                            nc.vector.tensor_scalar(t_sb[:], t_sb[:], p0[h],
                                                    None, ALU.mult)
                        nc.vector.tensor_add(t_sb[:], t_sb[:], R_sb[:])
                        u_sb = gup.tile([P, 8, IC], BF16, name="u_sb")
                        nc.scalar.activation(u_sb[:], t_sb[:], AF.Exp)
                        ps_s = pp.tile([P, 2], F32, name="ps_s")
                        for jc in range(8):
                            nc.tensor.matmul(ps_s[:], u_sb[:, jc],
                                             ones_colb[:],
                                             start=(jc == 0), stop=(jc == 7))
                        rs = sp_.tile([P, 1], F32, name="rs_t",
                                      bufs=min(n_groups + 1, H))
                        nc.vector.reciprocal(rs[:], ps_s[:, 0:1])
                        grp[hkeys[h]] = (u_sb, rs)

                    ps_o = [pp.tile([P, DV], F32, name=f"ps_o{b}")
                            for b in range(B)]
                    for jc in range(8):
                        lhsT = u_sb[:, jc]
                        for b in range(B):
                            nc.tensor.matmul(
                                ps_o[b][:], lhsT,
                                v_sb[:, b * 8 + jc, h * DV:(h + 1) * DV],
                                start=(jc == 0), stop=(jc == 7))
                    for b in range(B):
                        nc.vector.tensor_scalar(
                            O_sb[:, b, h * DV:(h + 1) * DV],
                            ps_o[b][:], rs[:], None, ALU.mult)

                    # once both heads of a 128-col chunk are done,
                    # transpose it to feature-major (adding bv; exact since
                    # softmax rows sum to 1)
                    if h % 2 == 1:
                        hc = h // 2
                        for b in range(B):
                            pt = pp.tile([P, P], F32, name="pt")
                            nc.tensor.transpose(
                                pt[:], O_sb[:, b, hc * P:(hc + 1) * P],
                                ident[:])
                            nc.vector.tensor_scalar(
                                OT[:, hc, b * IC:(b + 1) * IC], pt[:],
                                bv_sb[:, hc:hc + 1], None, ALU.add)


            # h2o weight stream (traced here so it doesn't outprioritize
            # the stage-A input loads; still prefetches during fc/FFN)
            h2oT_t = h2oT.ap().rearrange("(c p) v -> p c v", p=P)
            W_sbs = []
            for vi, (vs, vsz) in enumerate(VTILES):
                W_sb = wp.tile([P, DC, 512], BF16, name="W_sb")
                nc.sync.dma_start(W_sb[:, :, :vsz], h2oT_t[:, :, vs:vs + vsz])
                W_sbs.append(W_sb)

            # ---------------- stage C: fc + LN1 ----------------
            yp = s1.enter_context(tc.tile_pool(name="ypool", bufs=1))
            Y_sb = yp.tile([P, DC, ROWS], BF16, name="Y_sb")
            with contextlib.ExitStack() as sc:
                fp = sc.enter_context(tc.tile_pool(name="fcp", bufs=1))
                pc = sc.enter_context(tc.tile_pool(name="psC", bufs=4,
                                                   space="PSUM"))
                F1 = fp.tile([P, DC, ROWS], F32R, name="F1")
                for half in range(2):
                    psfs = [pc.tile([P, ROWS], F32, name=f"psf{q}", bufs=1)
                            for q in range(4)]
                    for hc in range(DC):
                        for q in range(4):
                            do = half * 4 + q
                            nc.tensor.matmul(
                                psfs[q][:],
                                fcT_sb[:, hc, do * P:(do + 1) * P],
                                OT[:, hc, :],
                                start=(hc == 0), stop=(hc == DC - 1))
                    for q in range(4):
                        do = half * 4 + q
                        nc.vector.tensor_scalar(F1[:, do], psfs[q][:],
                                                fcb_sb[:, do:do + 1], None,
                                                ALU.add)
                _layernorm_sb(nc, tc, F1, ln1g, ln1b, Y_sb, ones_col, "ln1")

            # ---------------- stage D: FFN ----------------
            with contextlib.ExitStack() as sd:
                hp = sd.enter_context(tc.tile_pool(name="hpool", bufs=1))
                wsp = sd.enter_context(tc.tile_pool(name="wstr", bufs=2))
                w2p = sd.enter_context(tc.tile_pool(name="w2str", bufs=5))
                cd = sd.enter_context(tc.tile_pool(name="cD", bufs=1))
                pd = sd.enter_context(tc.tile_pool(name="psD", bufs=2,
                                                   space="PSUM"))
                H_sb = hp.tile([P, HC, ROWS], BF16, name="H_sb")
                b1_sb = cd.tile([P, HC], F32, name="b1_sb")
                nc.sync.dma_start(b1_sb[:], b12.ap())
                b2_sb = cd.tile([P, DC], F32, name="b2_sb")
                nc.sync.dma_start(b2_sb[:], b22.ap())

                w1T_t = w1T.ap().rearrange("(c p) m -> p c m", p=P)
                for hs in range(8):           # 512-wide hid slices
                    W1t = wsp.tile([P, DC, 512], BF16, name="W1t")
                    nc.sync.dma_start(W1t[:],
                                      w1T_t[:, :, hs * 512:(hs + 1) * 512])
                    for m2 in range(4):       # 128-wide subchunks
                        psh = pd.tile([P, ROWS], F32, name="psh")
                        for dc in range(DC):
                            nc.tensor.matmul(
                                psh[:],
                                W1t[:, dc, m2 * P:(m2 + 1) * P],
                                Y_sb[:, dc, :],
                                start=(dc == 0), stop=(dc == DC - 1))
                        hcix = hs * 4 + m2
                        nc.scalar.activation(H_sb[:, hcix], psh[:], AF.Relu,
                                             bias=b1_sb[:, hcix:hcix + 1])

                FF = hp.tile([P, DC, ROWS], F32R, name="FF")
                w2T_t = w2T.ap().rearrange("(c p) m -> p c m", p=P)
                for ds2 in range(2):          # 512-wide d slices
                    W2ts = []
                    for g in range(4):
                        W2t = w2p.tile([P, 8, 512], BF16, name="W2t")
                        nc.sync.dma_start(
                            W2t[:],
                            w2T_t[:, g * 8:(g + 1) * 8,
                                  ds2 * 512:(ds2 + 1) * 512])
                        W2ts.append(W2t)
                    for m2 in range(4):
                        do = ds2 * 4 + m2
                        psw = pd.tile([P, ROWS], F32, name="psw")
                        for hc in range(HC):
                            nc.tensor.matmul(
                                psw[:],
                                W2ts[hc // 8][:, hc % 8, m2 * P:(m2 + 1) * P],
                                H_sb[:, hc, :],
                                start=(hc == 0), stop=(hc == HC - 1))
                        nc.vector.tensor_scalar(psw[:], psw[:],
                                                b2_sb[:, do:do + 1], None,
                                                ALU.add)
                        nc.vector.tensor_add(FF[:, do], psw[:], Y_sb[:, do])
                _layernorm_sb(nc, tc, FF, ln2g, ln2b, Z_sb, ones_col, "ln2")

        # ---------------- stage E: h2o + log_softmax ----------------
        with contextlib.ExitStack() as se:
            ep = se.enter_context(tc.tile_pool(name="h2o_e", bufs=3))
            op_ = se.enter_context(tc.tile_pool(name="h2o_o", bufs=4))
            lp_ = se.enter_context(tc.tile_pool(name="h2o_l", bufs=1))
            pp = se.enter_context(tc.tile_pool(name="h2o_p", bufs=4,
                                               space="PSUM"))

            L16 = lp_.tile([P, B, V], F16, name="L16")          # 16 MB
            parts = lp_.tile([P, B, len(VTILES)], F32, name="parts")
            if bias_on["h2o"]:
                ones_row = lp_.tile([1, ROWS], BF16, name="ones_row_z")
                nc.sync.dma_start(ones_row[:], onesr.ap())

            for vi, (vs, vsz) in enumerate(VTILES):
                W_sb = W_sbs[vi]
                if bias_on["h2o"]:
                    bias_sb = ep.tile([1, 512], BF16, name="bias_sb")
                    nc.sync.dma_start(bias_sb[:, :vsz],
                                      h2ob.ap()[:, vs:vs + vsz])
                for rt in range(B):
                    ps = pp.tile([P, 512], F32, name="ps_l")
                    for dc in range(DC):
                        nc.tensor.matmul(
                            ps[:, :vsz],
                            Z_sb[:, dc, rt * IC:(rt + 1) * IC],
                            W_sb[:, dc, :vsz],
                            start=(dc == 0),
                            stop=(dc == DC - 1 and not bias_on["h2o"]))
                    if bias_on["h2o"]:
                        nc.tensor.matmul(
                            ps[:, :vsz],
                            ones_row[:, rt * IC:(rt + 1) * IC],
                            bias_sb[:, :vsz],
                            start=False, stop=True)
                    nc.vector.tensor_copy(L16[:, rt, vs:vs + vsz],
                                          ps[:, :vsz])
                    esc = ep.tile([P, 512], F32, name="esc", bufs=2)
                    nc.scalar.activation(
                        esc[:, :vsz], ps[:, :vsz], AF.Exp,
                        accum_out=parts[:, rt, vi:vi + 1])

            for rt in range(B):
                s_t = ep.tile([P, 1], F32, name="s_t")
                nc.vector.reduce_sum(s_t[:], parts[:, rt, :], axis=AX.X)
                lse = ep.tile([P, 1], F32, name="lse_t")
                nc.scalar.activation(lse[:], s_t[:], AF.Ln)
                for ti, (vs, vsz) in enumerate(VTILES):
                    ot = op_.tile([P, 512], F16, name="ot")
                    eng = nc.gpsimd if ti % 5 == 4 else nc.vector
                    eng.tensor_scalar(ot[:, :vsz], L16[:, rt, vs:vs + vsz],
                                      lse[:], None, ALU.subtract)
                    nc.sync.dma_start(
                        y.ap()[rt * IC:(rt + 1) * IC, vs:vs + vsz],
                        ot[:, :vsz])

    nc.compile()
    return nc


_CACHE = {}


def _ppart(vec, chunks):
    """[chunks*P] -> [P, chunks] per-partition layout."""
    return np.ascontiguousarray(vec.reshape(chunks, P).T, np.float32)


def kernel(**inputs):
    f32 = np.float32
    bf16 = ml_dtypes.bfloat16
    x = np.asarray(inputs["x"], f32)
    wv = np.asarray(inputs["wv"], f32)
    bv = np.asarray(inputs["bv"], f32)
    fc_w = np.asarray(inputs["fc_w"], f32)
    fc_b = np.asarray(inputs["fc_b"], f32)
    ln1_g = np.asarray(inputs["ln1_g"], f32)
    ln1_b = np.asarray(inputs["ln1_b"], f32)
    w1 = np.asarray(inputs["w1"], f32)
    b1 = np.asarray(inputs["b1"], f32)
    w2 = np.asarray(inputs["w2"], f32)
    b2 = np.asarray(inputs["b2"], f32)
    ln2_g = np.asarray(inputs["ln2_g"], f32)
    ln2_b = np.asarray(inputs["ln2_b"], f32)
    h2o_w = np.asarray(inputs["h2o_w"], f32)
    h2o_b = np.asarray(inputs["h2o_b"], f32)
    p0 = np.asarray(inputs["p0"], np.float64)
    p1 = np.asarray(inputs["p1"], np.float64)
    p2 = np.asarray(inputs["p2"], np.float64)
    p3 = np.asarray(inputs["p3"], np.float64)
    # wk/bk deliberately unused: constant along the softmax axis.

    sp1 = np.float32(_softplus(p1)).astype(np.float64)
    sp2 = np.float32(_softplus(p2)).astype(np.float64)

    bias_on = {"h2o": bool(np.any(h2o_b))}

    key = (p0.tobytes(), sp1.tobytes(), sp2.tobytes(), p3.tobytes(),
           bias_on["h2o"])
    if key not in _CACHE:
        _CACHE[key] = _build(p0, sp1, sp2, p3, bias_on)
    nc = _CACHE[key]

    x2T = np.ascontiguousarray(x.reshape(B * L, D).T)
    shared = {
        "xT": np.ascontiguousarray(x2T.astype(bf16)),
        "wvT": np.ascontiguousarray(wv.T.astype(bf16)),
        "fcT": np.ascontiguousarray(fc_w.T.astype(bf16)),
        "w1T": np.ascontiguousarray(w1.T.astype(bf16)),
        "w2T": np.ascontiguousarray(w2.T.astype(bf16)),
        "h2oT": np.ascontiguousarray(h2o_w.T.astype(bf16)),
        "bv2": _ppart(bv, DC),
        "fcb2": _ppart(fc_b, DC),
        "b12": _ppart(b1, HC),
        "b22": _ppart(b2, DC),
        "onesc": np.ones((P, 2), f32),
        "onesb": np.ones((P, 2), bf16),
        "ln1g": _ppart(ln1_g, DC),
        "ln1b": _ppart(ln1_b, DC),
        "ln2g": _ppart(ln2_g, DC),
        "ln2b": _ppart(ln2_b, DC),
    }
    if bias_on["h2o"]:
        shared["h2ob"] = np.ascontiguousarray(h2o_b[None].astype(bf16))
        shared["onesr"] = np.ones((1, ROWS), bf16)

    p3_zero = bool(np.all(p3 == 0.0))
    ebv = np.zeros(H, np.float64)
    for h in range(H):
        if p0[h] > 0.0 and abs(sp1[h] - sp2[h]) < 1e-12:
            ebv[h] = math.log(2.0 * p0[h])
        elif p0[h] > 0.0:
            ebv[h] = math.log(p0[h])
    expb_host = np.ascontiguousarray(
        np.broadcast_to(ebv.astype(f32)[None, :], (P, H)))

    j = np.arange(L)
    in_maps = []
    for c in range(NCORES):
        i_idx = c * IC + np.arange(IC)
        Sji = np.abs(j[:, None] - i_idx[None, :]).astype(f32)       # [L, IC]
        eye = (Sji == 0).astype(f32)
        if p3_zero:
            Rs = [NEG_BIG * eye]
        else:
            Aji = (i_idx[None, :] < j[:, None]).astype(f32)
            Rs = [np.float32(p3[h]) * Aji + NEG_BIG * eye for h in range(H)]

        def tile_ji(a):  # [L, IC] -> [jp, jc, IC]
            return np.ascontiguousarray(
                a.reshape(8, P, IC).transpose(1, 0, 2), f32)

        m = dict(shared)
        m["S_in"] = tile_ji(Sji)
        m["expb"] = expb_host
        m["R_in"] = np.stack([tile_ji(R) for R in Rs], axis=0)
        in_maps.append(m)

    res = run_bass_kernel_spmd(nc, in_maps, core_ids=list(range(NCORES)))

    out = np.empty((B, L, V), f32)
    for c in range(NCORES):
        yc = res.results[c]["y"]
        for b in range(B):
            out[b, c * IC:(c + 1) * IC, :] = yc[b * IC:(b + 1) * IC, :]
    return out


# revision 27
# speedup vs baseline: 1.6993x; 1.1088x over previous
"""Trainium2 Bass kernel for a single-layer "BiTRF" dense transformer block.

Math (see reference):
  posi[h,i,j] = p0*(exp(-sp1*|i-j|) + exp(-sp2*|i-j|)) + p3*(i<j)   (sp=softplus(p))
  attn[h,b,i,j] = kproj[b,i,h] + posi[h,i,j], diag masked, softmax over j.
  Because kproj[b,i,h] is constant along the softmax axis j, softmax is
  invariant to it, so the wk/bk projection drops out entirely and the
  attention weights W[h,i,:] are shared across the batch (and across heads
  with identical (p0, sp1, sp2, p3) — computed once per unique group).
  out  = LN1(attnout @ fc_w.T + fc_b)
  out2 = LN2(relu(out @ w1.T + b1) @ w2.T + b2 + out)
  y    = log_softmax(out2 @ h2o_w.T + h2o_b)

Sharding: 8 cores, core c owns query rows i in [c*128,(c+1)*128) for BOTH
batches (256 row-instances).  v = x@wv.T is computed redundantly on every
core (avoids any collective); everything else is row-sharded, h2o is
row-sharded too (each core computes its rows x full 32000 vocab, so
log_softmax is fully local).

The whole pre-h2o chain (v-proj, attention, fc, LN1, FFN, LN2) lives in
SBUF — no DRAM round-trips between stages.  Activations are feature-major
[feat, row] (LN partition reductions via ones-column matmuls); attention
output is transposed back with PE-transposes.  Biases are applied as
per-partition scalars at PSUM-eviction time (bv rides through the
attention because softmax rows sum to 1, so it is added at the transpose
eviction).  The h2o weight stream pool lives at top scope so its first
tiles prefetch during earlier phases.

dtypes: matmuls run bf16 (weights pre-cast on host, fp32 PSUM
accumulation); LayerNorm statistics and log-sum-exp run in fp32; raw
logits are staged in fp16 for the final lse subtraction.
"""

import contextlib
import math

import ml_dtypes
import numpy as np

import concourse.mybir as mybir
import concourse.tile as tile
from concourse import bacc
from concourse.bass_utils import run_bass_kernel_spmd
from concourse.masks import make_identity

B, L, D, H, DV, HID, V = 2, 1024, 1024, 16, 64, 4096, 32000
NCORES = 8
IC = L // NCORES        # 128 query rows per core
ROWS = B * IC           # 256 row-instances per core
HD = H * DV             # 1024
P = 128
DC = D // P             # 8 feature chunks
HC = HID // P           # 32 hidden chunks
EPS = 1e-5
NEG_BIG = -1.0e9

F32 = mybir.dt.float32
F32R = mybir.dt.float32r
BF16 = mybir.dt.bfloat16
F16 = mybir.dt.float16
AF = mybir.ActivationFunctionType
ALU = mybir.AluOpType
AX = mybir.AxisListType

# h2o vocab tiling: 62 tiles of 512 + 1 tile of 256
VTILES = [(i * 512, 512) for i in range(62)] + [(62 * 512, 256)]


def _r(ap):
    return ap.bitcast(F32R)


def _softplus(x):
    return np.logaddexp(0.0, x.astype(np.float64))


def _layernorm_sb(nc, tc, F_sb, g_dram, b_dram, Y_sb, ones_col, tag):
    """LN over the feature (partition) axis, fully in SBUF.
    F_sb: [P, DC, ROWS] f32r source; Y_sb: [P, DC, ROWS] dst (any dtype)."""
    with contextlib.ExitStack() as ctx:
        lp = ctx.enter_context(tc.tile_pool(name=f"ln_{tag}", bufs=2))
        cp = ctx.enter_context(tc.tile_pool(name=f"lnc_{tag}", bufs=1))
        pp = ctx.enter_context(tc.tile_pool(name=f"lnp_{tag}", bufs=2, space="PSUM"))

        SQ = lp.tile([P, DC, ROWS], F32R, name=f"SQ_{tag}")
        nc.vector.tensor_mul(SQ[:], F_sb[:], F_sb[:])

        g_sb = cp.tile([P, DC], F32, name=f"g_{tag}")
        nc.sync.dma_start(g_sb[:], g_dram.ap())
        b_sb = cp.tile([P, DC], F32, name=f"b_{tag}")
        nc.sync.dma_start(b_sb[:], b_dram.ap())

        ps_sum = pp.tile([2, ROWS], F32, name=f"pssum_{tag}")
        ps_sq = pp.tile([2, ROWS], F32, name=f"pssq_{tag}")
        for dc in range(DC):
            nc.tensor.matmul(ps_sum[:], ones_col[:], F_sb[:, dc],
                             start=(dc == 0), stop=(dc == DC - 1))
            nc.tensor.matmul(ps_sq[:], ones_col[:], SQ[:, dc],
                             start=(dc == 0), stop=(dc == DC - 1))

        mean = lp.tile([1, ROWS], F32, name=f"mean_{tag}")
        nc.vector.tensor_scalar(mean[:], ps_sum[0:1, :], 1.0 / D, None, ALU.mult)
        ex2 = lp.tile([1, ROWS], F32, name=f"ex2_{tag}")
        nc.vector.tensor_scalar(ex2[:], ps_sq[0:1, :], 1.0 / D, None, ALU.mult)
        var = lp.tile([1, ROWS], F32, name=f"var_{tag}")
        nc.vector.tensor_mul(var[:], mean[:], mean[:])
        nc.vector.tensor_sub(var[:], ex2[:], var[:])
        veps = lp.tile([1, ROWS], F32, name=f"veps_{tag}")
        nc.vector.tensor_scalar(veps[:], var[:], EPS, None, ALU.add)
        s0 = lp.tile([1, ROWS], F32, name=f"s0_{tag}")
        nc.scalar.activation(s0[:], veps[:], AF.Sqrt)
        r0 = lp.tile([1, ROWS], F32, name=f"r0_{tag}")
        nc.vector.reciprocal(r0[:], s0[:])
        s1 = lp.tile([1, ROWS], F32, name=f"s1_{tag}")
        nc.vector.tensor_mul(s1[:], veps[:], r0[:])
        nc.vector.tensor_add(s1[:], s1[:], s0[:])
        nc.vector.tensor_scalar(s1[:], s1[:], 0.5, None, ALU.mult)
        rstd = lp.tile([1, ROWS], F32, name=f"rstd_{tag}")
        nc.vector.reciprocal(rstd[:], s1[:])

        meanB = lp.tile([P, ROWS], F32, name=f"meanB_{tag}")
        nc.gpsimd.partition_broadcast(meanB[:], mean[:])
        rstdB = lp.tile([P, ROWS], F32, name=f"rstdB_{tag}")
        nc.gpsimd.partition_broadcast(rstdB[:], rstd[:])

        for dc in range(DC):
            t1 = lp.tile([P, ROWS], F32, name=f"t1_{tag}", bufs=3)
            nc.vector.tensor_sub(t1[:], F_sb[:, dc], meanB[:])
            nc.vector.tensor_mul(t1[:], t1[:], rstdB[:])
            nc.vector.tensor_scalar(Y_sb[:, dc], t1[:],
                                    g_sb[:, dc:dc + 1], b_sb[:, dc:dc + 1],
                                    ALU.mult, ALU.add)


def _build(p0, sp1, sp2, p3, bias_on):
    """Build + compile the SPMD program.  p0/sp1/sp2/p3 are [H] host floats
    baked into the NEFF as activation immediates; bias_on['h2o'] selects the
    rank-1 vocab-bias matmul (other biases are always applied, free)."""
    p3_zero = bool(np.all(p3 == 0.0))
    n_r = 1 if p3_zero else H

    nc = bacc.Bacc(None, target_bir_lowering=False, debug=False,
                   num_devices=NCORES)

    def inp(name, shape, dtype):
        return nc.dram_tensor(name, shape, dtype, kind="ExternalInput")

    xT = inp("xT", [D, B * L], BF16)
    wvT = inp("wvT", [D, HD], BF16)
    fcT = inp("fcT", [HD, D], BF16)
    w1T = inp("w1T", [D, HID], BF16)
    w2T = inp("w2T", [HID, D], BF16)
    h2oT = inp("h2oT", [D, V], BF16)
    bv2 = inp("bv2", [P, DC], F32)
    fcb2 = inp("fcb2", [P, DC], F32)
    b12 = inp("b12", [P, HC], F32)
    b22 = inp("b22", [P, DC], F32)
    if bias_on["h2o"]:
        h2ob = inp("h2ob", [1, V], BF16)
        onesr = inp("onesr", [1, ROWS], BF16)
    onesc = inp("onesc", [P, 2], F32R)
    onesb = inp("onesb", [P, 2], BF16)
    ln1g = inp("ln1g", [P, DC], F32)
    ln1b = inp("ln1b", [P, DC], F32)
    ln2g = inp("ln2g", [P, DC], F32)
    ln2b = inp("ln2b", [P, DC], F32)
    S_in = inp("S_in", [P, 8, IC], F32)          # |i-j| tiled [jp, jc, i]
    expb = inp("expb", [P, H], F32)              # per-head exp bias ln(2*p0)
    R_in = inp("R_in", [n_r, P, 8, IC], F32)     # p3*(i<j) - BIG*eye, per head
    # fp16 device output: logits are already fp16-staged; the extra
    # rounding is ~2^-11 * |out| (~8e-4 rel), and it halves the 32MB
    # output write that bounds the kernel tail.  Host casts back to f32.
    y = nc.dram_tensor("y", [ROWS, V], F16, kind="ExternalOutput")

    with tile.TileContext(nc) as tc, contextlib.ExitStack() as top:
        c0 = top.enter_context(tc.tile_pool(name="const0", bufs=1))
        wp = top.enter_context(tc.tile_pool(name="h2o_w", bufs=6))
        zp = top.enter_context(tc.tile_pool(name="zmid", bufs=1))

        ones_col = c0.tile([P, 2], F32R, name="ones_col")
        nc.sync.dma_start(ones_col[:], onesc.ap())
        ones_colb = c0.tile([P, 2], BF16, name="ones_colb")
        nc.sync.dma_start(ones_colb[:], onesb.ap())
        Z_sb = zp.tile([P, DC, ROWS], BF16, name="Z_sb")

        with contextlib.ExitStack() as s1:
            OT = s1.enter_context(tc.tile_pool(name="otp", bufs=1)).tile(
                [P, DC, ROWS], BF16, name="OT")
            # fc weights pool created before the A/B scope so pool
            # stack order holds; DMA traced here too (no deps, prefetches)
            fcp0 = s1.enter_context(tc.tile_pool(name="fcc", bufs=1))
            fcT_sb = fcp0.tile([P, DC, D], BF16, name="fcT_sb")
            fcb_sb = fcp0.tile([P, DC], F32, name="fcb_sb")

            sab = contextlib.ExitStack()
            # ---------------- stage A: v = x @ wv.T ----------------
            vp = sab.enter_context(tc.tile_pool(name="vpool", bufs=1))
            v_sb = vp.tile([P, B * L // P, HD], BF16, name="v_sb")
            with contextlib.ExitStack() as sa:
                rp = sa.enter_context(tc.tile_pool(name="resid", bufs=1))
                pa = sa.enter_context(tc.tile_pool(name="psA", bufs=4,
                                                   space="PSUM"))
                xT_sb = rp.tile([P, DC, B * L], BF16, name="xT_sb")
                xT_t = xT.ap().rearrange("(c p) r -> p c r", p=P)
                wvT_sb = rp.tile([P, DC, HD], BF16, name="wvT_sb")
                wvT_t = wvT.ap().rearrange("(c p) f -> p c f", p=P)
                for dc in range(DC):
                    nc.sync.dma_start(wvT_sb[:, dc], wvT_t[:, dc])
                for rcg in range(4):
                    for dc in range(DC):
                        nc.sync.dma_start(
                            xT_sb[:, dc, rcg * 512:(rcg + 1) * 512],
                            xT_t[:, dc, rcg * 512:(rcg + 1) * 512])
                for rc in range(B * L // P):
                    for nh in range(2):
                        psv = pa.tile([P, 512], F32, name="psv")
                        for dc in range(DC):
                            nc.tensor.matmul(
                                psv[:],
                                xT_sb[:, dc, rc * P:(rc + 1) * P],
                                wvT_sb[:, dc, nh * 512:(nh + 1) * 512],
                                start=(dc == 0), stop=(dc == DC - 1))
                        nc.vector.tensor_copy(
                            v_sb[:, rc, nh * 512:(nh + 1) * 512], psv[:])

            # fc weight prefetch: traced after stage A's input loads so it
            # doesn't delay them; lands during attention
            nc.sync.dma_start(fcT_sb[:],
                              fcT.ap().rearrange("(c p) f -> p c f", p=P))
            nc.sync.dma_start(fcb_sb[:], fcb2.ap())

            # ---------------- stage B: attention ----------------
            # (bv is added at the transpose eviction: softmax rows sum to 1)
            with sab, contextlib.ExitStack() as sb:
                up = sb.enter_context(tc.tile_pool(name="attn_u", bufs=2))
                sp_ = sb.enter_context(tc.tile_pool(name="attn_s", bufs=3))
                cp = sb.enter_context(tc.tile_pool(name="attn_c", bufs=1))
                ab = sb.enter_context(tc.tile_pool(name="attn_b", bufs=1))
                pp = sb.enter_context(tc.tile_pool(name="attn_p", bufs=2,
                                                   space="PSUM"))

                S_sb = cp.tile([P, 8, IC], F32, name="S_sb")
                nc.sync.dma_start(S_sb[:], S_in.ap())
                eb_sb = cp.tile([P, H], F32, name="eb_sb")
                nc.sync.dma_start(eb_sb[:], expb.ap())
                bv_sb = cp.tile([P, DC], F32, name="bv_sb")
                nc.sync.dma_start(bv_sb[:], bv2.ap())
                ident = cp.tile([P, P], F32, name="ident")
                make_identity(nc, ident[:])
                R_sb = None
                O_sb = ab.tile([P, B, HD], F32, name="O_sb")

                hkeys = [(float(p0[h]), float(sp1[h]), float(sp2[h]),
                          float(p3[h])) for h in range(H)]
                n_groups = len(set(hkeys))
                gup = sb.enter_context(
                    tc.tile_pool(name="attn_gu", bufs=min(n_groups + 1, H)))
                grp = {}
                for h in range(H):
                    if hkeys[h] in grp:
                        u_sb, rs = grp[hkeys[h]]
                    else:
                        if R_sb is None or n_r > 1:
                            R_sb = cp.tile([P, 8, IC], F32, name="R_sb",
                                           bufs=2)
                            nc.sync.dma_start(R_sb[:],
                                              R_in.ap()[min(h, n_r - 1)])
                        t_sb = up.tile([P, 8, IC], F32, name="t_sb")
                        if p0[h] > 0.0 and abs(sp1[h] - sp2[h]) < 1e-12:
                            nc.scalar.activation(t_sb[:], S_sb[:], AF.Exp,
                                                 scale=-sp1[h],
                                                 bias=eb_sb[:, h:h + 1])
                        elif p0[h] > 0.0:
                            e2 = up.tile([P, 8, IC], F32, name="e2_sb")
                            nc.scalar.activation(t_sb[:], S_sb[:], AF.Exp,
                                                 scale=-sp1[h],
                                                 bias=eb_sb[:, h:h + 1])
                            nc.scalar.activation(e2[:], S_sb[:], AF.Exp,
                                                 scale=-sp2[h],
                                                 bias=eb_sb[:, h:h + 1])
                            nc.vector.tensor_add(t_sb[:], t_sb[:], e2[:])
                        elif p0[h] == 0.0:
                            nc.any.memset(t_sb[:], 0.0)
                        else:
                            e2 = up.tile([P, 8, IC], F32, name="e2_sb")
                            nc.scalar.activation(t_sb[:], S_sb[:], AF.Exp,
                                                 scale=-sp1[h])
                            nc.scalar.activation(e2[:], S_sb[:], AF.Exp,
                                                 scale=-sp2[h])
                            nc.vector.tensor_add(t_sb[:], t_sb[:], e2[:])
                            nc.vector.tensor_scalar(t_sb[:], t_sb[:], p0[h],
                                                    None, ALU.mult)
                        nc.vector.tensor_add(t_sb[:], t_sb[:], R_sb[:])
                        u_sb = gup.tile([P, 8, IC], BF16, name="u_sb")
                        nc.scalar.activation(u_sb[:], t_sb[:], AF.Exp)
                        ps_s = pp.tile([P, 2], F32, name="ps_s")
                        for jc in range(8):
                            nc.tensor.matmul(ps_s[:], u_sb[:, jc],
                                             ones_colb[:],
                                             start=(jc == 0), stop=(jc == 7))
                        rs = sp_.tile([P, 1], F32, name="rs_t",
                                      bufs=min(n_groups + 1, H))
                        nc.vector.reciprocal(rs[:], ps_s[:, 0:1])
                        grp[hkeys[h]] = (u_sb, rs)

                    ps_o = [pp.tile([P, DV], F32, name=f"ps_o{b}")
                            for b in range(B)]
                    for jc in range(8):
                        lhsT = u_sb[:, jc]
                        for b in range(B):
                            nc.tensor.matmul(
                                ps_o[b][:], lhsT,
                                v_sb[:, b * 8 + jc, h * DV:(h + 1) * DV],
                                start=(jc == 0), stop=(jc == 7))
                    for b in range(B):
                        nc.vector.tensor_scalar(
                            O_sb[:, b, h * DV:(h + 1) * DV],
                            ps_o[b][:], rs[:], None, ALU.mult)

                    # once both heads of a 128-col chunk are done,
                    # transpose it to feature-major (adding bv; exact since
                    # softmax rows sum to 1)
                    if h % 2 == 1:
                        hc = h // 2
                        for b in range(B):
                            pt = pp.tile([P, P], F32, name="pt")
                            nc.tensor.transpose(
                                pt[:], O_sb[:, b, hc * P:(hc + 1) * P],
                                ident[:])
                            nc.vector.tensor_scalar(
                                OT[:, hc, b * IC:(b + 1) * IC], pt[:],
                                bv_sb[:, hc:hc + 1], None, ALU.add)


            # h2o weight stream (traced here so it doesn't outprioritize
            # the stage-A input loads; still prefetches during fc/FFN)
            h2oT_t = h2oT.ap().rearrange("(c p) v -> p c v", p=P)
            W_sbs = []
            for vi, (vs, vsz) in enumerate(VTILES):
                W_sb = wp.tile([P, DC, 512], BF16, name="W_sb")
                nc.sync.dma_start(W_sb[:, :, :vsz], h2oT_t[:, :, vs:vs + vsz])
                W_sbs.append(W_sb)

            # ---------------- stage C: fc + LN1 ----------------
            yp = s1.enter_context(tc.tile_pool(name="ypool", bufs=1))
            Y_sb = yp.tile([P, DC, ROWS], BF16, name="Y_sb")
            with contextlib.ExitStack() as sc:
                fp = sc.enter_context(tc.tile_pool(name="fcp", bufs=1))
                pc = sc.enter_context(tc.tile_pool(name="psC", bufs=4,
                                                   space="PSUM"))
                F1 = fp.tile([P, DC, ROWS], F32R, name="F1")
                for half in range(2):
                    psfs = [pc.tile([P, ROWS], F32, name=f"psf{q}", bufs=1)
                            for q in range(4)]
                    for hc in range(DC):
                        for q in range(4):
                            do = half * 4 + q
                            nc.tensor.matmul(
                                psfs[q][:],
                                fcT_sb[:, hc, do * P:(do + 1) * P],
                                OT[:, hc, :],
                                start=(hc == 0), stop=(hc == DC - 1))
                    for q in range(4):
                        do = half * 4 + q
                        nc.vector.tensor_scalar(F1[:, do], psfs[q][:],
                                                fcb_sb[:, do:do + 1], None,
                                                ALU.add)
                _layernorm_sb(nc, tc, F1, ln1g, ln1b, Y_sb, ones_col, "ln1")

            # ---------------- stage D: FFN ----------------
            with contextlib.ExitStack() as sd:
                hp = sd.enter_context(tc.tile_pool(name="hpool", bufs=1))
                wsp = sd.enter_context(tc.tile_pool(name="wstr", bufs=2))
                w2p = sd.enter_context(tc.tile_pool(name="w2str", bufs=5))
                cd = sd.enter_context(tc.tile_pool(name="cD", bufs=1))
                pd = sd.enter_context(tc.tile_pool(name="psD", bufs=2,
                                                   space="PSUM"))
                H_sb = hp.tile([P, HC, ROWS], BF16, name="H_sb")
                b1_sb = cd.tile([P, HC], F32, name="b1_sb")
                nc.sync.dma_start(b1_sb[:], b12.ap())
                b2_sb = cd.tile([P, DC], F32, name="b2_sb")
                nc.sync.dma_start(b2_sb[:], b22.ap())

                w1T_t = w1T.ap().rearrange("(c p) m -> p c m", p=P)
                for hs in range(8):           # 512-wide hid slices
                    W1t = wsp.tile([P, DC, 512], BF16, name="W1t")
                    nc.sync.dma_start(W1t[:],
                                      w1T_t[:, :, hs * 512:(hs + 1) * 512])
                    for m2 in range(4):       # 128-wide subchunks
                        psh = pd.tile([P, ROWS], F32, name="psh")
                        for dc in range(DC):
                            nc.tensor.matmul(
                                psh[:],
                                W1t[:, dc, m2 * P:(m2 + 1) * P],
                                Y_sb[:, dc, :],
                                start=(dc == 0), stop=(dc == DC - 1))
                        hcix = hs * 4 + m2
                        nc.scalar.activation(H_sb[:, hcix], psh[:], AF.Relu,
                                             bias=b1_sb[:, hcix:hcix + 1])

                FF = hp.tile([P, DC, ROWS], F32R, name="FF")
                w2T_t = w2T.ap().rearrange("(c p) m -> p c m", p=P)
                for ds2 in range(2):          # 512-wide d slices
                    W2ts = []
                    for g in range(4):
                        W2t = w2p.tile([P, 8, 512], BF16, name="W2t")
                        nc.sync.dma_start(
                            W2t[:],
                            w2T_t[:, g * 8:(g + 1) * 8,
                                  ds2 * 512:(ds2 + 1) * 512])
                        W2ts.append(W2t)
                    for m2 in range(4):
                        do = ds2 * 4 + m2
                        psw = pd.tile([P, ROWS], F32, name="psw")
                        for hc in range(HC):
                            nc.tensor.matmul(
                                psw[:],
                                W2ts[hc // 8][:, hc % 8, m2 * P:(m2 + 1) * P],
                                H_sb[:, hc, :],
                                start=(hc == 0), stop=(hc == HC - 1))
                        nc.vector.tensor_scalar(psw[:], psw[:],
                                                b2_sb[:, do:do + 1], None,
                                                ALU.add)
                        nc.vector.tensor_add(FF[:, do], psw[:], Y_sb[:, do])
                _layernorm_sb(nc, tc, FF, ln2g, ln2b, Z_sb, ones_col, "ln2")

        # ---------------- stage E: h2o + log_softmax ----------------
        with contextlib.ExitStack() as se:
            ep = se.enter_context(tc.tile_pool(name="h2o_e", bufs=3))
            op_ = se.enter_context(tc.tile_pool(name="h2o_o", bufs=2))
            lp_ = se.enter_context(tc.tile_pool(name="h2o_l", bufs=1))
            pp = se.enter_context(tc.tile_pool(name="h2o_p", bufs=4,
                                               space="PSUM"))

            L16 = lp_.tile([P, B, V], F16, name="L16")          # 16 MB
            parts = lp_.tile([P, B, len(VTILES)], F32, name="parts")
            if bias_on["h2o"]:
                ones_row = lp_.tile([1, ROWS], BF16, name="ones_row_z")
                nc.sync.dma_start(ones_row[:], onesr.ap())

            for vi, (vs, vsz) in enumerate(VTILES):
                W_sb = W_sbs[vi]
                if bias_on["h2o"]:
                    bias_sb = ep.tile([1, 512], BF16, name="bias_sb")
                    nc.sync.dma_start(bias_sb[:, :vsz],
                                      h2ob.ap()[:, vs:vs + vsz])
                for rt in range(B):
                    ps = pp.tile([P, 512], F32, name="ps_l")
                    for dc in range(DC):
                        nc.tensor.matmul(
                            ps[:, :vsz],
                            Z_sb[:, dc, rt * IC:(rt + 1) * IC],
                            W_sb[:, dc, :vsz],
                            start=(dc == 0),
                            stop=(dc == DC - 1 and not bias_on["h2o"]))
                    if bias_on["h2o"]:
                        nc.tensor.matmul(
                            ps[:, :vsz],
                            ones_row[:, rt * IC:(rt + 1) * IC],
                            bias_sb[:, :vsz],
                            start=False, stop=True)
                    nc.vector.tensor_copy(L16[:, rt, vs:vs + vsz],
                                          ps[:, :vsz])
                    esc = ep.tile([P, 512], F32, name="esc", bufs=2)
                    nc.scalar.activation(
                        esc[:, :vsz], ps[:, :vsz], AF.Exp,
                        accum_out=parts[:, rt, vi:vi + 1])

            # wide output staging: one DMA per 8 vocab tiles (the tail was
            # serialized by HWDGE descriptor-gen on 126 small DMAs)
            for rt in range(B):
                s_t = ep.tile([P, 1], F32, name="s_t")
                nc.vector.reduce_sum(s_t[:], parts[:, rt, :], axis=AX.X)
                lse = ep.tile([P, 1], F32, name="lse_t")
                nc.scalar.activation(lse[:], s_t[:], AF.Ln)
                for gi in range(0, len(VTILES), 8):
                    gtiles = VTILES[gi:gi + 8]
                    g0 = gtiles[0][0]
                    gw = gtiles[-1][0] + gtiles[-1][1] - g0
                    ot = op_.tile([P, 4096], F16, name="ot")
                    for ti, (vs, vsz) in enumerate(gtiles):
                        eng = nc.gpsimd if ti % 5 == 4 else nc.vector
                        eng.tensor_scalar(ot[:, vs - g0:vs - g0 + vsz],
                                          L16[:, rt, vs:vs + vsz],
                                          lse[:], None, ALU.subtract)
                    nc.sync.dma_start(
                        y.ap()[rt * IC:(rt + 1) * IC, g0:g0 + gw],
                        ot[:, :gw])

    nc.compile()
    return nc


_CACHE = {}


def _ppart(vec, chunks):
    """[chunks*P] -> [P, chunks] per-partition layout."""
    return np.ascontiguousarray(vec.reshape(chunks, P).T, np.float32)


def kernel(**inputs):
    f32 = np.float32
    bf16 = ml_dtypes.bfloat16
    x = np.asarray(inputs["x"], f32)
    wv = np.asarray(inputs["wv"], f32)
    bv = np.asarray(inputs["bv"], f32)
    fc_w = np.asarray(inputs["fc_w"], f32)
    fc_b = np.asarray(inputs["fc_b"], f32)
    ln1_g = np.asarray(inputs["ln1_g"], f32)
    ln1_b = np.asarray(inputs["ln1_b"], f32)
    w1 = np.asarray(inputs["w1"], f32)
    b1 = np.asarray(inputs["b1"], f32)
    w2 = np.asarray(inputs["w2"], f32)
    b2 = np.asarray(inputs["b2"], f32)
    ln2_g = np.asarray(inputs["ln2_g"], f32)
    ln2_b = np.asarray(inputs["ln2_b"], f32)
    h2o_w = np.asarray(inputs["h2o_w"], f32)
    h2o_b = np.asarray(inputs["h2o_b"], f32)
    p0 = np.asarray(inputs["p0"], np.float64)
    p1 = np.asarray(inputs["p1"], np.float64)
    p2 = np.asarray(inputs["p2"], np.float64)
    p3 = np.asarray(inputs["p3"], np.float64)
    # wk/bk deliberately unused: constant along the softmax axis.

    sp1 = np.float32(_softplus(p1)).astype(np.float64)
    sp2 = np.float32(_softplus(p2)).astype(np.float64)

    bias_on = {"h2o": bool(np.any(h2o_b))}

    key = (p0.tobytes(), sp1.tobytes(), sp2.tobytes(), p3.tobytes(),
           bias_on["h2o"])
    if key not in _CACHE:
        _CACHE[key] = _build(p0, sp1, sp2, p3, bias_on)
    nc = _CACHE[key]

    x2T = np.ascontiguousarray(x.reshape(B * L, D).T)
    shared = {
        "xT": np.ascontiguousarray(x2T.astype(bf16)),
        "wvT": np.ascontiguousarray(wv.T.astype(bf16)),
        "fcT": np.ascontiguousarray(fc_w.T.astype(bf16)),
        "w1T": np.ascontiguousarray(w1.T.astype(bf16)),
        "w2T": np.ascontiguousarray(w2.T.astype(bf16)),
        "h2oT": np.ascontiguousarray(h2o_w.T.astype(bf16)),
        "bv2": _ppart(bv, DC),
        "fcb2": _ppart(fc_b, DC),
        "b12": _ppart(b1, HC),
        "b22": _ppart(b2, DC),
        "onesc": np.ones((P, 2), f32),
        "onesb": np.ones((P, 2), bf16),
        "ln1g": _ppart(ln1_g, DC),
        "ln1b": _ppart(ln1_b, DC),
        "ln2g": _ppart(ln2_g, DC),
        "ln2b": _ppart(ln2_b, DC),
    }
    if bias_on["h2o"]:
        shared["h2ob"] = np.ascontiguousarray(h2o_b[None].astype(bf16))
        shared["onesr"] = np.ones((1, ROWS), bf16)

    p3_zero = bool(np.all(p3 == 0.0))
    ebv = np.zeros(H, np.float64)
    for h in range(H):
        if p0[h] > 0.0 and abs(sp1[h] - sp2[h]) < 1e-12:
            ebv[h] = math.log(2.0 * p0[h])
        elif p0[h] > 0.0:
            ebv[h] = math.log(p0[h])
    expb_host = np.ascontiguousarray(
        np.broadcast_to(ebv.astype(f32)[None, :], (P, H)))

    j = np.arange(L)
    in_maps = []
    for c in range(NCORES):
        i_idx = c * IC + np.arange(IC)
        Sji = np.abs(j[:, None] - i_idx[None, :]).astype(f32)       # [L, IC]
        eye = (Sji == 0).astype(f32)
        if p3_zero:
            Rs = [NEG_BIG * eye]
        else:
            Aji = (i_idx[None, :] < j[:, None]).astype(f32)
            Rs = [np.float32(p3[h]) * Aji + NEG_BIG * eye for h in range(H)]

        def tile_ji(a):  # [L, IC] -> [jp, jc, IC]
            return np.ascontiguousarray(
                a.reshape(8, P, IC).transpose(1, 0, 2), f32)

        m = dict(shared)
        m["S_in"] = tile_ji(Sji)
        m["expb"] = expb_host
        m["R_in"] = np.stack([tile_ji(R) for R in Rs], axis=0)
        in_maps.append(m)

    res = run_bass_kernel_spmd(nc, in_maps, core_ids=list(range(NCORES)))

    out = np.empty((B, L, V), f32)
    for c in range(NCORES):
        yc = res.results[c]["y"]
        for b in range(B):
            out[b, c * IC:(c + 1) * IC, :] = yc[b * IC:(b + 1) * IC, :]
    return out


# revision 28
# speedup vs baseline: 1.7000x; 1.0004x over previous
"""Trainium2 Bass kernel for a single-layer "BiTRF" dense transformer block.

Math (see reference):
  posi[h,i,j] = p0*(exp(-sp1*|i-j|) + exp(-sp2*|i-j|)) + p3*(i<j)   (sp=softplus(p))
  attn[h,b,i,j] = kproj[b,i,h] + posi[h,i,j], diag masked, softmax over j.
  Because kproj[b,i,h] is constant along the softmax axis j, softmax is
  invariant to it, so the wk/bk projection drops out entirely and the
  attention weights W[h,i,:] are shared across the batch (and across heads
  with identical (p0, sp1, sp2, p3) — computed once per unique group).
  out  = LN1(attnout @ fc_w.T + fc_b)
  out2 = LN2(relu(out @ w1.T + b1) @ w2.T + b2 + out)
  y    = log_softmax(out2 @ h2o_w.T + h2o_b)

Sharding: 8 cores, core c owns query rows i in [c*128,(c+1)*128) for BOTH
batches (256 row-instances).  v = x@wv.T is computed redundantly on every
core (avoids any collective); everything else is row-sharded, h2o is
row-sharded too (each core computes its rows x full 32000 vocab, so
log_softmax is fully local).

The whole pre-h2o chain (v-proj, attention, fc, LN1, FFN, LN2) lives in
SBUF — no DRAM round-trips between stages.  Activations are feature-major
[feat, row] (LN partition reductions via ones-column matmuls); attention
output is transposed back with PE-transposes.  Biases are applied as
per-partition scalars at PSUM-eviction time (bv rides through the
attention because softmax rows sum to 1, so it is added at the transpose
eviction).  The h2o weight stream pool lives at top scope so its first
tiles prefetch during earlier phases.

dtypes: matmuls run bf16 (weights pre-cast on host, fp32 PSUM
accumulation); LayerNorm statistics and log-sum-exp run in fp32; raw
logits are staged in fp16 for the final lse subtraction.
"""

import contextlib
import math

import ml_dtypes
import numpy as np

import concourse.mybir as mybir
import concourse.tile as tile
from concourse import bacc
from concourse.bass_utils import run_bass_kernel_spmd
from concourse.masks import make_identity

B, L, D, H, DV, HID, V = 2, 1024, 1024, 16, 64, 4096, 32000
NCORES = 8
IC = L // NCORES        # 128 query rows per core
ROWS = B * IC           # 256 row-instances per core
HD = H * DV             # 1024
P = 128
DC = D // P             # 8 feature chunks
HC = HID // P           # 32 hidden chunks
EPS = 1e-5
NEG_BIG = -1.0e9

F32 = mybir.dt.float32
F32R = mybir.dt.float32r
BF16 = mybir.dt.bfloat16
F16 = mybir.dt.float16
AF = mybir.ActivationFunctionType
ALU = mybir.AluOpType
AX = mybir.AxisListType

# h2o vocab tiling: 62 tiles of 512 + 1 tile of 256
VTILES = [(i * 512, 512) for i in range(62)] + [(62 * 512, 256)]


def _r(ap):
    return ap.bitcast(F32R)


def _softplus(x):
    return np.logaddexp(0.0, x.astype(np.float64))


def _layernorm_sb(nc, tc, F_sb, g_dram, b_dram, Y_sb, ones_col, tag):
    """LN over the feature (partition) axis, fully in SBUF.
    F_sb: [P, DC, ROWS] f32r source; Y_sb: [P, DC, ROWS] dst (any dtype)."""
    with contextlib.ExitStack() as ctx:
        lp = ctx.enter_context(tc.tile_pool(name=f"ln_{tag}", bufs=2))
        cp = ctx.enter_context(tc.tile_pool(name=f"lnc_{tag}", bufs=1))
        pp = ctx.enter_context(tc.tile_pool(name=f"lnp_{tag}", bufs=2, space="PSUM"))

        SQ = lp.tile([P, DC, ROWS], F32R, name=f"SQ_{tag}")
        nc.vector.tensor_mul(SQ[:], F_sb[:], F_sb[:])

        g_sb = cp.tile([P, DC], F32, name=f"g_{tag}")
        nc.sync.dma_start(g_sb[:], g_dram.ap())
        b_sb = cp.tile([P, DC], F32, name=f"b_{tag}")
        nc.sync.dma_start(b_sb[:], b_dram.ap())

        ps_sum = pp.tile([2, ROWS], F32, name=f"pssum_{tag}")
        ps_sq = pp.tile([2, ROWS], F32, name=f"pssq_{tag}")
        for dc in range(DC):
            nc.tensor.matmul(ps_sum[:], ones_col[:], F_sb[:, dc],
                             start=(dc == 0), stop=(dc == DC - 1))
            nc.tensor.matmul(ps_sq[:], ones_col[:], SQ[:, dc],
                             start=(dc == 0), stop=(dc == DC - 1))

        mean = lp.tile([1, ROWS], F32, name=f"mean_{tag}")
        nc.vector.tensor_scalar(mean[:], ps_sum[0:1, :], 1.0 / D, None, ALU.mult)
        ex2 = lp.tile([1, ROWS], F32, name=f"ex2_{tag}")
        nc.vector.tensor_scalar(ex2[:], ps_sq[0:1, :], 1.0 / D, None, ALU.mult)
        var = lp.tile([1, ROWS], F32, name=f"var_{tag}")
        nc.vector.tensor_mul(var[:], mean[:], mean[:])
        nc.vector.tensor_sub(var[:], ex2[:], var[:])
        veps = lp.tile([1, ROWS], F32, name=f"veps_{tag}")
        nc.vector.tensor_scalar(veps[:], var[:], EPS, None, ALU.add)
        s0 = lp.tile([1, ROWS], F32, name=f"s0_{tag}")
        nc.scalar.activation(s0[:], veps[:], AF.Sqrt)
        r0 = lp.tile([1, ROWS], F32, name=f"r0_{tag}")
        nc.vector.reciprocal(r0[:], s0[:])
        s1 = lp.tile([1, ROWS], F32, name=f"s1_{tag}")
        nc.vector.tensor_mul(s1[:], veps[:], r0[:])
        nc.vector.tensor_add(s1[:], s1[:], s0[:])
        nc.vector.tensor_scalar(s1[:], s1[:], 0.5, None, ALU.mult)
        rstd = lp.tile([1, ROWS], F32, name=f"rstd_{tag}")
        nc.vector.reciprocal(rstd[:], s1[:])

        meanB = lp.tile([P, ROWS], F32, name=f"meanB_{tag}")
        nc.gpsimd.partition_broadcast(meanB[:], mean[:])
        rstdB = lp.tile([P, ROWS], F32, name=f"rstdB_{tag}")
        nc.gpsimd.partition_broadcast(rstdB[:], rstd[:])

        for dc in range(DC):
            t1 = lp.tile([P, ROWS], F32, name=f"t1_{tag}", bufs=3)
            nc.vector.tensor_sub(t1[:], F_sb[:, dc], meanB[:])
            nc.vector.tensor_mul(t1[:], t1[:], rstdB[:])
            nc.vector.tensor_scalar(Y_sb[:, dc], t1[:],
                                    g_sb[:, dc:dc + 1], b_sb[:, dc:dc + 1],
                                    ALU.mult, ALU.add)


def _build(p0, sp1, sp2, p3, bias_on):
    """Build + compile the SPMD program.  p0/sp1/sp2/p3 are [H] host floats
    baked into the NEFF as activation immediates; bias_on['h2o'] selects the
    rank-1 vocab-bias matmul (other biases are always applied, free)."""
    p3_zero = bool(np.all(p3 == 0.0))
    n_r = 1 if p3_zero else H

    nc = bacc.Bacc(None, target_bir_lowering=False, debug=False,
                   num_devices=NCORES)

    def inp(name, shape, dtype):
        return nc.dram_tensor(name, shape, dtype, kind="ExternalInput")

    xT = inp("xT", [D, B * L], BF16)
    wvT = inp("wvT", [D, HD], BF16)
    fcT = inp("fcT", [HD, D], BF16)
    w1T = inp("w1T", [D, HID], BF16)
    w2T = inp("w2T", [HID, D], BF16)
    h2oT = inp("h2oT", [D, V], BF16)
    bv2 = inp("bv2", [P, DC], F32)
    fcb2 = inp("fcb2", [P, DC], F32)
    b12 = inp("b12", [P, HC], F32)
    b22 = inp("b22", [P, DC], F32)
    if bias_on["h2o"]:
        h2ob = inp("h2ob", [1, V], BF16)
        onesr = inp("onesr", [1, ROWS], BF16)
    onesc = inp("onesc", [P, 2], F32R)
    onesb = inp("onesb", [P, 2], BF16)
    ln1g = inp("ln1g", [P, DC], F32)
    ln1b = inp("ln1b", [P, DC], F32)
    ln2g = inp("ln2g", [P, DC], F32)
    ln2b = inp("ln2b", [P, DC], F32)
    S_in = inp("S_in", [P, 8, IC], F32)          # |i-j| tiled [jp, jc, i]
    expb = inp("expb", [P, H], F32)              # per-head exp bias ln(2*p0)
    R_in = inp("R_in", [n_r, P, 8, IC], F32)     # p3*(i<j) - BIG*eye, per head
    # fp16 device output: logits are already fp16-staged; the extra
    # rounding is ~2^-11 * |out| (~8e-4 rel), and it halves the 32MB
    # output write that bounds the kernel tail.  Host casts back to f32.
    y = nc.dram_tensor("y", [ROWS, V], F16, kind="ExternalOutput")

    with tile.TileContext(nc) as tc, contextlib.ExitStack() as top:
        c0 = top.enter_context(tc.tile_pool(name="const0", bufs=1))
        wp = top.enter_context(tc.tile_pool(name="h2o_w", bufs=6))
        zp = top.enter_context(tc.tile_pool(name="zmid", bufs=1))

        ones_col = c0.tile([P, 2], F32R, name="ones_col")
        nc.sync.dma_start(ones_col[:], onesc.ap())
        ones_colb = c0.tile([P, 2], BF16, name="ones_colb")
        nc.sync.dma_start(ones_colb[:], onesb.ap())
        Z_sb = zp.tile([P, DC, ROWS], BF16, name="Z_sb")

        with contextlib.ExitStack() as s1:
            OT = s1.enter_context(tc.tile_pool(name="otp", bufs=1)).tile(
                [P, DC, ROWS], BF16, name="OT")
            # fc weights pool created before the A/B scope so pool
            # stack order holds; DMA traced here too (no deps, prefetches)
            fcp0 = s1.enter_context(tc.tile_pool(name="fcc", bufs=1))
            fcT_sb = fcp0.tile([P, DC, D], BF16, name="fcT_sb")
            fcb_sb = fcp0.tile([P, DC], F32, name="fcb_sb")

            sab = contextlib.ExitStack()
            # ---------------- stage A: v = x @ wv.T ----------------
            vp = sab.enter_context(tc.tile_pool(name="vpool", bufs=1))
            v_sb = vp.tile([P, B * L // P, HD], BF16, name="v_sb")
            with contextlib.ExitStack() as sa:
                rp = sa.enter_context(tc.tile_pool(name="resid", bufs=1))
                pa = sa.enter_context(tc.tile_pool(name="psA", bufs=4,
                                                   space="PSUM"))
                xT_sb = rp.tile([P, DC, B * L], BF16, name="xT_sb")
                xT_t = xT.ap().rearrange("(c p) r -> p c r", p=P)
                wvT_sb = rp.tile([P, DC, HD], BF16, name="wvT_sb")
                wvT_t = wvT.ap().rearrange("(c p) f -> p c f", p=P)
                for dc in range(DC):
                    nc.sync.dma_start(wvT_sb[:, dc], wvT_t[:, dc])
                for rcg in range(4):
                    for dc in range(DC):
                        nc.sync.dma_start(
                            xT_sb[:, dc, rcg * 512:(rcg + 1) * 512],
                            xT_t[:, dc, rcg * 512:(rcg + 1) * 512])
                for rc in range(B * L // P):
                    for nh in range(2):
                        psv = pa.tile([P, 512], F32, name="psv")
                        for dc in range(DC):
                            nc.tensor.matmul(
                                psv[:],
                                xT_sb[:, dc, rc * P:(rc + 1) * P],
                                wvT_sb[:, dc, nh * 512:(nh + 1) * 512],
                                start=(dc == 0), stop=(dc == DC - 1))
                        nc.vector.tensor_copy(
                            v_sb[:, rc, nh * 512:(nh + 1) * 512], psv[:])

            # fc weight prefetch: traced after stage A's input loads so it
            # doesn't delay them; lands during attention
            nc.sync.dma_start(fcT_sb[:],
                              fcT.ap().rearrange("(c p) f -> p c f", p=P))
            nc.sync.dma_start(fcb_sb[:], fcb2.ap())

            # ---------------- stage B: attention ----------------
            # (bv is added at the transpose eviction: softmax rows sum to 1)
            with sab, contextlib.ExitStack() as sb:
                up = sb.enter_context(tc.tile_pool(name="attn_u", bufs=2))
                sp_ = sb.enter_context(tc.tile_pool(name="attn_s", bufs=3))
                cp = sb.enter_context(tc.tile_pool(name="attn_c", bufs=1))
                ab = sb.enter_context(tc.tile_pool(name="attn_b", bufs=1))
                pp = sb.enter_context(tc.tile_pool(name="attn_p", bufs=2,
                                                   space="PSUM"))

                S_sb = cp.tile([P, 8, IC], F32, name="S_sb")
                nc.sync.dma_start(S_sb[:], S_in.ap())
                eb_sb = cp.tile([P, H], F32, name="eb_sb")
                nc.sync.dma_start(eb_sb[:], expb.ap())
                bv_sb = cp.tile([P, DC], F32, name="bv_sb")
                nc.sync.dma_start(bv_sb[:], bv2.ap())
                ident = cp.tile([P, P], F32, name="ident")
                make_identity(nc, ident[:])
                R_sb = None
                O_sb = ab.tile([P, B, HD], F32, name="O_sb")

                hkeys = [(float(p0[h]), float(sp1[h]), float(sp2[h]),
                          float(p3[h])) for h in range(H)]
                n_groups = len(set(hkeys))
                gup = sb.enter_context(
                    tc.tile_pool(name="attn_gu", bufs=min(n_groups + 1, H)))
                grp = {}
                for h in range(H):
                    if hkeys[h] in grp:
                        u_sb, rs = grp[hkeys[h]]
                    else:
                        if R_sb is None or n_r > 1:
                            R_sb = cp.tile([P, 8, IC], F32, name="R_sb",
                                           bufs=2)
                            nc.sync.dma_start(R_sb[:],
                                              R_in.ap()[min(h, n_r - 1)])
                        t_sb = up.tile([P, 8, IC], F32, name="t_sb")
                        if p0[h] > 0.0 and abs(sp1[h] - sp2[h]) < 1e-12:
                            nc.scalar.activation(t_sb[:], S_sb[:], AF.Exp,
                                                 scale=-sp1[h],
                                                 bias=eb_sb[:, h:h + 1])
                        elif p0[h] > 0.0:
                            e2 = up.tile([P, 8, IC], F32, name="e2_sb")
                            nc.scalar.activation(t_sb[:], S_sb[:], AF.Exp,
                                                 scale=-sp1[h],
                                                 bias=eb_sb[:, h:h + 1])
                            nc.scalar.activation(e2[:], S_sb[:], AF.Exp,
                                                 scale=-sp2[h],
                                                 bias=eb_sb[:, h:h + 1])
                            nc.vector.tensor_add(t_sb[:], t_sb[:], e2[:])
                        elif p0[h] == 0.0:
                            nc.any.memset(t_sb[:], 0.0)
                        else:
                            e2 = up.tile([P, 8, IC], F32, name="e2_sb")
                            nc.scalar.activation(t_sb[:], S_sb[:], AF.Exp,
                                                 scale=-sp1[h])
                            nc.scalar.activation(e2[:], S_sb[:], AF.Exp,
                                                 scale=-sp2[h])
                            nc.vector.tensor_add(t_sb[:], t_sb[:], e2[:])
                            nc.vector.tensor_scalar(t_sb[:], t_sb[:], p0[h],
                                                    None, ALU.mult)
                        nc.vector.tensor_add(t_sb[:], t_sb[:], R_sb[:])
                        u_sb = gup.tile([P, 8, IC], BF16, name="u_sb")
                        nc.scalar.activation(u_sb[:], t_sb[:], AF.Exp)
                        ps_s = pp.tile([P, 2], F32, name="ps_s")
                        for jc in range(8):
                            nc.tensor.matmul(ps_s[:], u_sb[:, jc],
                                             ones_colb[:],
                                             start=(jc == 0), stop=(jc == 7))
                        rs = sp_.tile([P, 1], F32, name="rs_t",
                                      bufs=min(n_groups + 1, H))
                        nc.vector.reciprocal(rs[:], ps_s[:, 0:1])
                        grp[hkeys[h]] = (u_sb, rs)

                    ps_o = [pp.tile([P, DV], F32, name=f"ps_o{b}")
                            for b in range(B)]
                    for jc in range(8):
                        lhsT = u_sb[:, jc]
                        for b in range(B):
                            nc.tensor.matmul(
                                ps_o[b][:], lhsT,
                                v_sb[:, b * 8 + jc, h * DV:(h + 1) * DV],
                                start=(jc == 0), stop=(jc == 7))
                    for b in range(B):
                        nc.vector.tensor_scalar(
                            O_sb[:, b, h * DV:(h + 1) * DV],
                            ps_o[b][:], rs[:], None, ALU.mult)

                    # once both heads of a 128-col chunk are done,
                    # transpose it to feature-major (adding bv; exact since
                    # softmax rows sum to 1)
                    if h % 2 == 1:
                        hc = h // 2
                        for b in range(B):
                            pt = pp.tile([P, P], F32, name="pt")
                            nc.tensor.transpose(
                                pt[:], O_sb[:, b, hc * P:(hc + 1) * P],
                                ident[:])
                            nc.vector.tensor_scalar(
                                OT[:, hc, b * IC:(b + 1) * IC], pt[:],
                                bv_sb[:, hc:hc + 1], None, ALU.add)


            # h2o weight stream (traced here so it doesn't outprioritize
            # the stage-A input loads; still prefetches during fc/FFN)
            h2oT_t = h2oT.ap().rearrange("(c p) v -> p c v", p=P)
            W_sbs = []
            for vi, (vs, vsz) in enumerate(VTILES):
                W_sb = wp.tile([P, DC, 512], BF16, name="W_sb")
                nc.sync.dma_start(W_sb[:, :, :vsz], h2oT_t[:, :, vs:vs + vsz])
                W_sbs.append(W_sb)

            # ---------------- stage C: fc + LN1 ----------------
            yp = s1.enter_context(tc.tile_pool(name="ypool", bufs=1))
            Y_sb = yp.tile([P, DC, ROWS], BF16, name="Y_sb")
            with contextlib.ExitStack() as sc:
                fp = sc.enter_context(tc.tile_pool(name="fcp", bufs=1))
                pc = sc.enter_context(tc.tile_pool(name="psC", bufs=4,
                                                   space="PSUM"))
                F1 = fp.tile([P, DC, ROWS], F32R, name="F1")
                for half in range(2):
                    psfs = [pc.tile([P, ROWS], F32, name=f"psf{q}", bufs=1)
                            for q in range(4)]
                    for hc in range(DC):
                        for q in range(4):
                            do = half * 4 + q
                            nc.tensor.matmul(
                                psfs[q][:],
                                fcT_sb[:, hc, do * P:(do + 1) * P],
                                OT[:, hc, :],
                                start=(hc == 0), stop=(hc == DC - 1))
                    for q in range(4):
                        do = half * 4 + q
                        nc.vector.tensor_scalar(F1[:, do], psfs[q][:],
                                                fcb_sb[:, do:do + 1], None,
                                                ALU.add)
                _layernorm_sb(nc, tc, F1, ln1g, ln1b, Y_sb, ones_col, "ln1")

            # ---------------- stage D: FFN ----------------
            with contextlib.ExitStack() as sd:
                hp = sd.enter_context(tc.tile_pool(name="hpool", bufs=1))
                wsp = sd.enter_context(tc.tile_pool(name="wstr", bufs=2))
                w2p = sd.enter_context(tc.tile_pool(name="w2str", bufs=5))
                cd = sd.enter_context(tc.tile_pool(name="cD", bufs=1))
                pd = sd.enter_context(tc.tile_pool(name="psD", bufs=2,
                                                   space="PSUM"))
                H_sb = hp.tile([P, HC, ROWS], BF16, name="H_sb")
                b1_sb = cd.tile([P, HC], F32, name="b1_sb")
                nc.sync.dma_start(b1_sb[:], b12.ap())
                b2_sb = cd.tile([P, DC], F32, name="b2_sb")
                nc.sync.dma_start(b2_sb[:], b22.ap())

                w1T_t = w1T.ap().rearrange("(c p) m -> p c m", p=P)
                for hs in range(8):           # 512-wide hid slices
                    W1t = wsp.tile([P, DC, 512], BF16, name="W1t")
                    nc.sync.dma_start(W1t[:],
                                      w1T_t[:, :, hs * 512:(hs + 1) * 512])
                    for m2 in range(4):       # 128-wide subchunks
                        psh = pd.tile([P, ROWS], F32, name="psh")
                        for dc in range(DC):
                            nc.tensor.matmul(
                                psh[:],
                                W1t[:, dc, m2 * P:(m2 + 1) * P],
                                Y_sb[:, dc, :],
                                start=(dc == 0), stop=(dc == DC - 1))
                        hcix = hs * 4 + m2
                        nc.scalar.activation(H_sb[:, hcix], psh[:], AF.Relu,
                                             bias=b1_sb[:, hcix:hcix + 1])

                FF = hp.tile([P, DC, ROWS], F32R, name="FF")
                w2T_t = w2T.ap().rearrange("(c p) m -> p c m", p=P)
                for ds2 in range(2):          # 512-wide d slices
                    W2ts = []
                    for g in range(4):
                        W2t = w2p.tile([P, 8, 512], BF16, name="W2t")
                        nc.sync.dma_start(
                            W2t[:],
                            w2T_t[:, g * 8:(g + 1) * 8,
                                  ds2 * 512:(ds2 + 1) * 512])
                        W2ts.append(W2t)
                    for m2 in range(4):
                        do = ds2 * 4 + m2
                        psw = pd.tile([P, ROWS], F32, name="psw")
                        for hc in range(HC):
                            nc.tensor.matmul(
                                psw[:],
                                W2ts[hc // 8][:, hc % 8, m2 * P:(m2 + 1) * P],
                                H_sb[:, hc, :],
                                start=(hc == 0), stop=(hc == HC - 1))
                        nc.vector.tensor_scalar(psw[:], psw[:],
                                                b2_sb[:, do:do + 1], None,
                                                ALU.add)
                        nc.vector.tensor_add(FF[:, do], psw[:], Y_sb[:, do])
                _layernorm_sb(nc, tc, FF, ln2g, ln2b, Z_sb, ones_col, "ln2")

        # ---------------- stage E: h2o + log_softmax ----------------
        with contextlib.ExitStack() as se:
            ep = se.enter_context(tc.tile_pool(name="h2o_e", bufs=3))
            op_ = se.enter_context(tc.tile_pool(name="h2o_o", bufs=2))
            lp_ = se.enter_context(tc.tile_pool(name="h2o_l", bufs=1))
            pp = se.enter_context(tc.tile_pool(name="h2o_p", bufs=4,
                                               space="PSUM"))

            L16 = lp_.tile([P, B, V], F16, name="L16")          # 16 MB
            parts = lp_.tile([P, B, len(VTILES)], F32, name="parts")
            if bias_on["h2o"]:
                ones_row = lp_.tile([1, ROWS], BF16, name="ones_row_z")
                nc.sync.dma_start(ones_row[:], onesr.ap())

            for vi, (vs, vsz) in enumerate(VTILES):
                W_sb = W_sbs[vi]
                if bias_on["h2o"]:
                    bias_sb = ep.tile([1, 512], BF16, name="bias_sb")
                    nc.sync.dma_start(bias_sb[:, :vsz],
                                      h2ob.ap()[:, vs:vs + vsz])
                for rt in range(B):
                    ps = pp.tile([P, 512], F32, name="ps_l")
                    for dc in range(DC):
                        nc.tensor.matmul(
                            ps[:, :vsz],
                            Z_sb[:, dc, rt * IC:(rt + 1) * IC],
                            W_sb[:, dc, :vsz],
                            start=(dc == 0),
                            stop=(dc == DC - 1 and not bias_on["h2o"]))
                    if bias_on["h2o"]:
                        nc.tensor.matmul(
                            ps[:, :vsz],
                            ones_row[:, rt * IC:(rt + 1) * IC],
                            bias_sb[:, :vsz],
                            start=False, stop=True)
                    nc.vector.tensor_copy(L16[:, rt, vs:vs + vsz],
                                          ps[:, :vsz])
                    esc = ep.tile([P, 512], F32, name="esc", bufs=2)
                    nc.scalar.activation(
                        esc[:, :vsz], ps[:, :vsz], AF.Exp,
                        accum_out=parts[:, rt, vi:vi + 1])

            # wide output staging: one DMA per 8 vocab tiles (the tail was
            # serialized by HWDGE descriptor-gen on 126 small DMAs)
            for rt in range(B):
                s_t = ep.tile([P, 1], F32, name="s_t")
                nc.vector.reduce_sum(s_t[:], parts[:, rt, :], axis=AX.X)
                lse = ep.tile([P, 1], F32, name="lse_t")
                nc.scalar.activation(lse[:], s_t[:], AF.Ln)
                nlse = ep.tile([P, 1], F32, name="nlse_t")
                nc.vector.tensor_scalar(nlse[:], lse[:], -1.0, None, ALU.mult)
                for gi in range(0, len(VTILES), 8):
                    gtiles = VTILES[gi:gi + 8]
                    g0 = gtiles[0][0]
                    gw = gtiles[-1][0] + gtiles[-1][1] - g0
                    ot = op_.tile([P, 4096], F16, name="ot")
                    for ti, (vs, vsz) in enumerate(gtiles):
                        # three-way engine split by measured rates
                        # (DVE ~0.19us, ACT ~0.72us, Pool ~0.81us per tile)
                        dst = ot[:, vs - g0:vs - g0 + vsz]
                        srcv = L16[:, rt, vs:vs + vsz]
                        if ti % 8 in (2, 6):
                            nc.scalar.activation(dst, srcv, AF.Identity,
                                                 bias=nlse[:])
                        elif ti % 8 == 4:
                            nc.gpsimd.tensor_scalar(dst, srcv, lse[:], None,
                                                    ALU.subtract)
                        else:
                            nc.vector.tensor_scalar(dst, srcv, lse[:], None,
                                                    ALU.subtract)
                    nc.sync.dma_start(
                        y.ap()[rt * IC:(rt + 1) * IC, g0:g0 + gw],
                        ot[:, :gw])

    nc.compile()
    return nc


_CACHE = {}


def _ppart(vec, chunks):
    """[chunks*P] -> [P, chunks] per-partition layout."""
    return np.ascontiguousarray(vec.reshape(chunks, P).T, np.float32)


def kernel(**inputs):
    f32 = np.float32
    bf16 = ml_dtypes.bfloat16
    x = np.asarray(inputs["x"], f32)
    wv = np.asarray(inputs["wv"], f32)
    bv = np.asarray(inputs["bv"], f32)
    fc_w = np.asarray(inputs["fc_w"], f32)
    fc_b = np.asarray(inputs["fc_b"], f32)
    ln1_g = np.asarray(inputs["ln1_g"], f32)
    ln1_b = np.asarray(inputs["ln1_b"], f32)
    w1 = np.asarray(inputs["w1"], f32)
    b1 = np.asarray(inputs["b1"], f32)
    w2 = np.asarray(inputs["w2"], f32)
    b2 = np.asarray(inputs["b2"], f32)
    ln2_g = np.asarray(inputs["ln2_g"], f32)
    ln2_b = np.asarray(inputs["ln2_b"], f32)
    h2o_w = np.asarray(inputs["h2o_w"], f32)
    h2o_b = np.asarray(inputs["h2o_b"], f32)
    p0 = np.asarray(inputs["p0"], np.float64)
    p1 = np.asarray(inputs["p1"], np.float64)
    p2 = np.asarray(inputs["p2"], np.float64)
    p3 = np.asarray(inputs["p3"], np.float64)
    # wk/bk deliberately unused: constant along the softmax axis.

    sp1 = np.float32(_softplus(p1)).astype(np.float64)
    sp2 = np.float32(_softplus(p2)).astype(np.float64)

    bias_on = {"h2o": bool(np.any(h2o_b))}

    key = (p0.tobytes(), sp1.tobytes(), sp2.tobytes(), p3.tobytes(),
           bias_on["h2o"])
    if key not in _CACHE:
        _CACHE[key] = _build(p0, sp1, sp2, p3, bias_on)
    nc = _CACHE[key]

    x2T = np.ascontiguousarray(x.reshape(B * L, D).T)
    shared = {
        "xT": np.ascontiguousarray(x2T.astype(bf16)),
        "wvT": np.ascontiguousarray(wv.T.astype(bf16)),
        "fcT": np.ascontiguousarray(fc_w.T.astype(bf16)),
        "w1T": np.ascontiguousarray(w1.T.astype(bf16)),
        "w2T": np.ascontiguousarray(w2.T.astype(bf16)),
        "h2oT": np.ascontiguousarray(h2o_w.T.astype(bf16)),
        "bv2": _ppart(bv, DC),
        "fcb2": _ppart(fc_b, DC),
        "b12": _ppart(b1, HC),
        "b22": _ppart(b2, DC),
        "onesc": np.ones((P, 2), f32),
        "onesb": np.ones((P, 2), bf16),
        "ln1g": _ppart(ln1_g, DC),
        "ln1b": _ppart(ln1_b, DC),
        "ln2g": _ppart(ln2_g, DC),
        "ln2b": _ppart(ln2_b, DC),
    }
    if bias_on["h2o"]:
        shared["h2ob"] = np.ascontiguousarray(h2o_b[None].astype(bf16))
        shared["onesr"] = np.ones((1, ROWS), bf16)

    p3_zero = bool(np.all(p3 == 0.0))
    ebv = np.zeros(H, np.float64)
    for h in range(H):
        if p0[h] > 0.0 and abs(sp1[h] - sp2[h]) < 1e-12:
            ebv[h] = math.log(2.0 * p0[h])
        elif p0[h] > 0.0:
            ebv[h] = math.log(p0[h])
    expb_host = np.ascontiguousarray(
        np.broadcast_to(ebv.astype(f32)[None, :], (P, H)))

    j = np.arange(L)
    in_maps = []
    for c in range(NCORES):
        i_idx = c * IC + np.arange(IC)
        Sji = np.abs(j[:, None] - i_idx[None, :]).astype(f32)       # [L, IC]
        eye = (Sji == 0).astype(f32)
        if p3_zero:
            Rs = [NEG_BIG * eye]
        else:
            Aji = (i_idx[None, :] < j[:, None]).astype(f32)
            Rs = [np.float32(p3[h]) * Aji + NEG_BIG * eye for h in range(H)]

        def tile_ji(a):  # [L, IC] -> [jp, jc, IC]
            return np.ascontiguousarray(
                a.reshape(8, P, IC).transpose(1, 0, 2), f32)

        m = dict(shared)
        m["S_in"] = tile_ji(Sji)
        m["expb"] = expb_host
        m["R_in"] = np.stack([tile_ji(R) for R in Rs], axis=0)
        in_maps.append(m)

    res = run_bass_kernel_spmd(nc, in_maps, core_ids=list(range(NCORES)))

    out = np.empty((B, L, V), f32)
    for c in range(NCORES):
        yc = res.results[c]["y"]
        for b in range(B):
            out[b, c * IC:(c + 1) * IC, :] = yc[b * IC:(b + 1) * IC, :]
    return out
